# revision 1
# baseline (speedup 1.0000x reference)
"""Trainium2 Bass kernel for the HAN-based cognitive-diagnosis net.

Strategy (8 NeuronCores, SPMD — one program, per-core data):
  * Batch (2048) split 8x256 across cores. Only the gathered rows of the
    student/exercise HAN outputs are ever used, so each core computes GAT
    outputs only for its own batch-slice node list ("b-slots"), plus a 1/8
    share of all exercise nodes needed for the (global-mean) semantic
    attention statistics.  The 4-float statistic is AllReduce'd on-device.
  * GAT edge phase: ELL layout (128 node-rows on partitions x padded degree
    slots on the free dim), built on the host from dst-sorted edge lists.
    Per-edge rows [z(64xfp16) | el(8xfp32) | pad] = 256B are fetched with
    dma_gather from per-core DRAM tables computed on-device (z = x@W,
    el = x@(W folded with a_l)).  Softmax + weighted aggregation run on
    DVE/ACT/GPSIMD; everything fp32 except the 16-bit table/weight values.
  * Predictor: pre(b)[j,k] = sigma(Q^T + c1 + M1-term) built per 4-batch
    group in PSUM via accumulated matmuls, sigmoid on ACT (fp16 out),
    D = pref-diff on DVE, W3-contraction back on PE into an o[128k, 256b]
    PSUM tile, final sigmoid + kn_r weighting, [1,256] out per core.
"""

import os
import numpy as np

import concourse.bass as bass
import concourse.bacc as bacc
import concourse.mybir as mybir
import concourse.tile as tile
from concourse import library_config
from concourse.masks import make_identity
from concourse import bass_utils

F32 = mybir.dt.float32
F16 = mybir.dt.float16
U16 = mybir.dt.uint16
I16 = mybir.dt.int16

NC = 8
B = 2048
BC = B // NC          # 256 batch rows per core
K = 128
H, D, FD = 8, 8, 64
SEM = 128
S_N, E_N = 10000, 20000
P = 128

SLOT_BUDGET = 96     # max slot-columns per gather chunk

AX = mybir.AxisListType
OP = mybir.AluOpType
AF = mybir.ActivationFunctionType


# ----------------------------------------------------------------------------
# Host-side preprocessing (integer / layout only)
# ----------------------------------------------------------------------------

def _csr_by_dst(src, dst, n):
    order = np.argsort(dst, kind="stable")
    ss = src[order].astype(np.int64)
    counts = np.bincount(dst, minlength=n)
    rowptr = np.zeros(n + 1, np.int64)
    np.cumsum(counts, out=rowptr[1:])
    return ss, rowptr, counts


class GraphPlan:
    """Compile-time shared plan for one gather group (graph/metapath)."""

    def __init__(self, tiles_dt, chunks, nslot, ntiles):
        self.tiles_dt = tiles_dt      # per-tile Dt (shared across cores)
        self.chunks = chunks          # list of (tile_lo, ntiles_in_chunk, Dt)
        self.nslot = nslot            # total slot columns
        self.ntiles = ntiles


def _plan_chunks(tiles_dt):
    """Group tiles into chunks with a uniform Dt (the chunk max)."""
    chunks = []
    i = 0
    nslot = 0
    while i < len(tiles_dt):
        dt = max(int(tiles_dt[i]), 1)
        j = i + 1
        while j < len(tiles_dt):
            nd = max(dt, int(tiles_dt[j]), 1)
            if (j - i + 1) * nd > max(SLOT_BUDGET, nd):
                break
            dt = nd
            j += 1
        chunks.append((i, j - i, dt))
        nslot += (j - i) * dt
        i = j
    return GraphPlan(tiles_dt, chunks, nslot, len(tiles_dt))


def _build_idx(plan, node_tiles, ss, rowptr, counts, zero_row):
    """Build the int16 gather index array for one core+graph.

    node_tiles: list of arrays (<=128 node ids each), aligned with plan tiles.
    Returns [128, nslot*8] int16 in the dma_gather 16-wrap layout.
    """
    flat = np.full((plan.nslot, P), zero_row, np.int64)  # [slotcol, partition]
    col = 0
    for (t_lo, t_n, dt) in plan.chunks:
        for t in range(t_lo, t_lo + t_n):
            nodes = node_tiles[t]
            for pi, node in enumerate(nodes):
                deg = int(counts[node])
                if deg:
                    lo = rowptr[node]
                    flat[col:col + deg, pi] = ss[lo:lo + deg]
            col += dt
    assert col == plan.nslot
    arr = flat.reshape(-1)                     # i = col*128 + p
    n = arr.shape[0]
    idx16 = np.full((16, n // 16), zero_row, np.int16)
    ii = np.arange(n)
    idx16[ii % 16, ii // 16] = arr.astype(np.int16)
    return np.tile(idx16, (8, 1))


def _tiles_of(nodes):
    out = []
    for i in range(0, len(nodes), P):
        out.append(np.asarray(nodes[i:i + P]))
    return out


def _tile_dts(node_tiles, counts):
    return [int(max(1, counts[t].max() if len(t) else 1)) for t in node_tiles]


def _xtp(x, node_tiles, ntiles):
    """x^T columns for a node list, padded to ntiles*128 cols, fp16."""
    kdim = x.shape[1]
    out = np.zeros((kdim, ntiles * P), np.float16)
    for t, nodes in enumerate(node_tiles):
        out[:, t * P:t * P + len(nodes)] = x[nodes].T.astype(np.float16)
    return out


def preprocess(inputs):
    inp = {k: np.asarray(v) for k, v in inputs.items()}
    stu_id = inp["stu_id"].astype(np.int64)
    exer_id = inp["exer_id"].astype(np.int64)

    # CSRs (dst-sorted)
    g_st = _csr_by_dst(inp["ss0"].astype(np.int64), inp["sd0"].astype(np.int64), S_N)
    g_e0 = _csr_by_dst(inp["es0"].astype(np.int64), inp["ed0"].astype(np.int64), E_N)
    g_e1 = _csr_by_dst(inp["es1"].astype(np.int64), inp["ed1"].astype(np.int64), E_N)
    g_kn = _csr_by_dst(inp["ks0"].astype(np.int64), inp["kd0"].astype(np.int64), K)

    # ------- node lists per core -------
    # exercise share: per metapath, nodes globally degree-sorted, strided by core
    share_lists = {}
    for mp, g in ((0, g_e0), (1, g_e1)):
        order = np.argsort(-g[2], kind="stable")
        share_lists[mp] = [order[c::NC] for c in range(NC)]
        assert all(len(s) == E_N // NC for s in share_lists[mp])

    SH = E_N // NC                      # 2500
    SH_TILES = (SH + P - 1) // P        # 20
    BS_TILES = BC // P                  # 2

    # per-core node tile lists
    ex_tiles = {0: [], 1: []}           # mp -> [core][tile] node arrays
    st_tiles = []
    for c in range(NC):
        bsl = slice(c * BC, (c + 1) * BC)
        for mp in (0, 1):
            tl = _tiles_of(share_lists[mp][c])
            tl += _tiles_of(exer_id[bsl])
            ex_tiles[mp].append(tl)
        st_tiles.append(_tiles_of(stu_id[bsl]))
    kn_tiles = [_tiles_of(np.arange(K))] * NC

    # shared per-tile Dt = max over cores
    plans = {}
    for mp in (0, 1):
        g = (g_e0, g_e1)[mp]
        dts = np.max([_tile_dts(ex_tiles[mp][c], g[2]) for c in range(NC)], axis=0)
        plans["ex%d" % mp] = _plan_chunks(dts)
    dts = np.max([_tile_dts(st_tiles[c], g_st[2]) for c in range(NC)], axis=0)
    plans["st"] = _plan_chunks(dts)
    plans["kn"] = _plan_chunks(_tile_dts(kn_tiles[0], g_kn[2]))
    for pl in plans.values():
        assert max(d for (_, _, d) in pl.chunks) <= 128

    NT_EX = (E_N + P - 1) // P          # 157 z-table tiles
    NT_ST = (S_N + P - 1) // P          # 79
    ZR_EX = NT_EX * P                   # zero row index
    ZR_ST = NT_ST * P
    ZR_KN = K

    meta = dict(plans=plans, SH=SH, SH_TILES=SH_TILES, BS_TILES=BS_TILES,
                NT_EX=NT_EX, NT_ST=NT_ST, ZR_EX=ZR_EX, ZR_ST=ZR_ST, ZR_KN=ZR_KN)

    # ------- shared input arrays -------
    def padT(x, nt):  # [N, K] -> x^T [K, nt*128] fp16
        out = np.zeros((x.shape[1], nt * P), np.float16)
        out[:, :x.shape[0]] = x.T.astype(np.float16)
        return out

    zrow = np.zeros((1, 128), np.uint16)
    zrow[0, 64:80] = np.full(8, -1e30, np.float32).view(np.uint16)

    shared = {
        "xt_ex": padT(inp["exer_t"], NT_EX),
        "xt_st": padT(inp["stu_t"], NT_ST),
        "xt_kn": inp["kn_t"].T.astype(np.float16).copy(),
        "w_ex0": inp["f3W0"].astype(np.float16),
        "w_ex1": inp["f3W1"].astype(np.float16),
        "w_st": inp["f1W0"].astype(np.float16),
        "w_kn": inp["f5W0"].astype(np.float16),
        "alr_ex0": np.concatenate([inp["f3al0"].reshape(1, 64), inp["f3ar0"].reshape(1, 64)], 1),
        "alr_ex1": np.concatenate([inp["f3al1"].reshape(1, 64), inp["f3ar1"].reshape(1, 64)], 1),
        "alr_st": np.concatenate([inp["f1al0"].reshape(1, 64), inp["f1ar0"].reshape(1, 64)], 1),
        "alr_kn": np.concatenate([inp["f5al0"].reshape(1, 64), inp["f5ar0"].reshape(1, 64)], 1),
        "semW": inp["f3sW"].astype(np.float32),
        "semb_col": inp["f3sb"].reshape(SEM, 1).astype(np.float32),
        "semq_col": inp["f3sq"].reshape(SEM, 1).astype(np.float32),
        "pWT_st": inp["f1pW"].T.astype(np.float32).copy(),
        "pb_st": inp["f1pb"].reshape(K, 1).astype(np.float32),
        "pWT_ex": inp["f3pW"].T.astype(np.float32).copy(),
        "pb_ex": inp["f3pb"].reshape(K, 1).astype(np.float32),
        "pW_kn": inp["f5pW"].astype(np.float32),
        "pb_kn_row": inp["f5pb"].reshape(1, K).astype(np.float32),
        "W1a": inp["W1"][:K].astype(np.float32),
        "W1b": inp["W1"][K:].astype(np.float32),
        "W2a": inp["W2"][:K].astype(np.float32),
        "W2b": inp["W2"][K:].astype(np.float32),
        "W3h": inp["W3"].astype(np.float16),
        "b3": inp["b3"].reshape(1, 1).astype(np.float32),
        "zrow": zrow,
    }

    # ------- per-core arrays -------
    in_maps = []
    for c in range(NC):
        bsl = slice(c * BC, (c + 1) * BC)
        m = dict(shared)
        m["idx_ex0"] = _build_idx(plans["ex0"], ex_tiles[0][c], g_e0[0], g_e0[1], g_e0[2], ZR_EX)
        m["idx_ex1"] = _build_idx(plans["ex1"], ex_tiles[1][c], g_e1[0], g_e1[1], g_e1[2], ZR_EX)
        m["idx_st"] = _build_idx(plans["st"], st_tiles[c], g_st[0], g_st[1], g_st[2], ZR_ST)
        m["idx_kn"] = _build_idx(plans["kn"], kn_tiles[c], g_kn[0], g_kn[1], g_kn[2], ZR_KN)
        m["xtp_ex0"] = _xtp(inp["exer_t"], ex_tiles[0][c], SH_TILES + BS_TILES)
        m["xtp_ex1"] = _xtp(inp["exer_t"], ex_tiles[1][c], SH_TILES + BS_TILES)
        m["xtp_st"] = _xtp(inp["stu_t"], st_tiles[c], BS_TILES)
        m["kn_rT"] = inp["kn_r"][bsl].T.astype(np.float32).copy()
        in_maps.append(m)

    return meta, in_maps


# ----------------------------------------------------------------------------
# Bass program
# ----------------------------------------------------------------------------

def build_program(meta):
    nc = bacc.Bacc("TRN2", num_devices=NC)
    plans = meta["plans"]
    NT_EX, NT_ST = meta["NT_EX"], meta["NT_ST"]
    SH_TILES, BS_TILES = meta["SH_TILES"], meta["BS_TILES"]
    NTP_EX = SH_TILES + BS_TILES
    SH = meta["SH"]

    ein = {}
    def EIN(name, shape, dt):
        ein[name] = nc.dram_tensor(name, list(shape), dt, kind="ExternalInput")
        return ein[name]

    EIN("xt_ex", (K, NT_EX * P), F16)
    EIN("xt_st", (K, NT_ST * P), F16)
    EIN("xt_kn", (K, K), F16)
    EIN("w_ex0", (K, FD), F16); EIN("w_ex1", (K, FD), F16)
    EIN("w_st", (K, FD), F16); EIN("w_kn", (K, FD), F16)
    for g in ("ex0", "ex1", "st", "kn"):
        EIN("alr_" + g, (1, 128), F32)
    EIN("semW", (FD, SEM), F32); EIN("semb_col", (SEM, 1), F32); EIN("semq_col", (SEM, 1), F32)
    EIN("pWT_st", (K, FD), F32); EIN("pb_st", (K, 1), F32)
    EIN("pWT_ex", (K, FD), F32); EIN("pb_ex", (K, 1), F32)
    EIN("pW_kn", (FD, K), F32); EIN("pb_kn_row", (1, K), F32)
    EIN("W1a", (K, K), F32); EIN("W1b", (K, K), F32)
    EIN("W2a", (K, K), F32); EIN("W2b", (K, K), F32)
    EIN("W3h", (K, 1), F16); EIN("b3", (1, 1), F32)
    EIN("zrow", (1, 128), U16)
    for g in ("ex0", "ex1", "st", "kn"):
        EIN("idx_" + g, (P, plans[g].nslot * 8), I16)
    EIN("xtp_ex0", (K, NTP_EX * P), F16)
    EIN("xtp_ex1", (K, NTP_EX * P), F16)
    EIN("xtp_st", (K, BS_TILES * P), F16)
    EIN("kn_rT", (K, BC), F32)

    out_d = nc.dram_tensor("out", [1, BC], F32, kind="ExternalOutput")

    # tables (per-core private DRAM)
    tbl = {
        "ex0": nc.dram_tensor("tbl_ex0", [NT_EX * P + 1, 128], U16, kind="Internal"),
        "ex1": nc.dram_tensor("tbl_ex1", [NT_EX * P + 1, 128], U16, kind="Internal"),
        "st": nc.dram_tensor("tbl_st", [NT_ST * P + 1, 128], U16, kind="Internal"),
        "kn": nc.dram_tensor("tbl_kn", [K + 1, 128], U16, kind="Internal"),
    }
    cc_in = nc.dram_tensor("cc_in", [1, 16], F32, kind="Internal")
    cc_out = nc.dram_tensor("cc_out", [1, 16], F32, kind="Internal", addr_space="Shared")

    with tile.TileContext(nc) as tc:
        with tc.tile_pool(name="const", bufs=1) as cst, \
             tc.tile_pool(name="slab", bufs=1) as slab:
            nc.gpsimd.load_library(library_config.mlp)

            ident = cst.tile([P, P], F32, tag="ident", name="ident")
            make_identity(nc, ident[:])
            ones_col = cst.tile([P, 1], F32, tag="ones_col", name="ones_col")
            nc.vector.memset(ones_col[:], 1.0)
            ones_row = cst.tile([1, P], F32, tag="ones_row", name="ones_row")
            nc.vector.memset(ones_row[:], 1.0)

            # ---- load small weights ----
            def load(name, shape, dt):
                t = cst.tile(list(shape), dt, tag="ld_" + name, name="ld_" + name)
                nc.sync.dma_start(t[:], ein[name][:])
                return t
            w_g = {g: load("w_" + g, (K, FD), F16) for g in ("ex0", "ex1", "st", "kn")}
            alr = {g: load("alr_" + g, (1, 128), F32) for g in ("ex0", "ex1", "st", "kn")}
            semW = load("semW", (FD, SEM), F32)
            semb_col = load("semb_col", (SEM, 1), F32)
            semq_col = load("semq_col", (SEM, 1), F32)
            pWT_st = load("pWT_st", (K, FD), F32); pb_st = load("pb_st", (K, 1), F32)
            pWT_ex = load("pWT_ex", (K, FD), F32); pb_ex = load("pb_ex", (K, 1), F32)
            pW_kn = load("pW_kn", (FD, K), F32); pb_kn_row = load("pb_kn_row", (1, K), F32)
            W1a = load("W1a", (K, K), F32); W1b = load("W1b", (K, K), F32)
            W2a = load("W2a", (K, K), F32); W2b = load("W2b", (K, K), F32)
            W3h = load("W3h", (K, 1), F16); b3 = load("b3", (1, 1), F32)
            zrow_sb = load("zrow", (1, 128), U16)
            kn_rT = load("kn_rT", (K, BC), F32)
            idx_sb = {g: load("idx_" + g, (P, plans[g].nslot * 8), I16)
                      for g in ("ex0", "ex1", "st", "kn")}

            # ---- fold al/ar into W: Wcat[g] = [W | Wal] fp16 (+ War separately) ----
            wcat = {}   # [128, 80] f16: cols 0:64 W, 64:72 Wal
            war = {}    # [128, 8] f16
            with tc.tile_pool(name="bc_ps", bufs=2, space="PSUM") as bcp:
              for g in ("ex0", "ex1", "st", "kn"):
                alb = cst.tile([P, 128], F32, tag="alb", name="alb")
                alb_ps = bcp.tile([P, 128], F32, space="PSUM", tag="alb_ps", name="alb_ps")
                nc.tensor.matmul(alb_ps[:], lhsT=ones_row[:], rhs=alr[g][:])
                nc.vector.tensor_copy(alb[:], alb_ps[:])
                wf = cst.tile([P, FD], F32, tag="wf", name="wf")
                nc.vector.tensor_copy(wf[:], w_g[g][:])
                wtmp = cst.tile([P, FD], F32, tag="wtmp", name="wtmp")
                wc = cst.tile([P, 80], F16, tag="wcat_" + g, name="wcat_" + g)
                wcat[g] = wc
                nc.vector.memset(wc[:, 72:80], 0.0)
                nc.vector.tensor_copy(wc[:, 0:64], w_g[g][:])
                # Wal
                with nc.allow_low_precision(reason="8-elem head fold of fp16 weights"):
                    nc.vector.tensor_tensor(out=wtmp[:], in0=wf[:], in1=alb[:, 0:64], op=OP.mult)
                    nc.vector.tensor_reduce(out=wc[:, 64:72].bitcast(F16),
                                            in_=wtmp[:].rearrange("p (h f) -> p h f", h=H),
                                            axis=AX.X, op=OP.add)
                    # War
                    wr = cst.tile([P, 8], F16, tag="war_" + g, name="war_" + g)
                    war[g] = wr
                    nc.vector.tensor_tensor(out=wtmp[:], in0=wf[:], in1=alb[:, 64:128], op=OP.mult)
                    nc.vector.tensor_reduce(out=wr[:], in_=wtmp[:].rearrange("p (h f) -> p h f", h=H),
                                            axis=AX.X, op=OP.add)

            # ---- Phase A: z/el tables ----
            zgrp = [("ex0", ein["xt_ex"], NT_EX), ("ex1", ein["xt_ex"], NT_EX),
                    ("st", ein["xt_st"], NT_ST), ("kn", ein["xt_kn"], 1)]
            DMA_T = 24   # xt tiles per input DMA
            with tc.tile_pool(name="pA", bufs=3) as pa, \
                 tc.tile_pool(name="pA_ps", bufs=4, space="PSUM") as pap:
                for g, xt_d, nt in zgrp:
                    for lo in range(0, nt, DMA_T):
                        n_here = min(DMA_T, nt - lo)
                        xt_sb = pa.tile([P, DMA_T * P], F16, tag="xt_sb", name="xt_sb")
                        nc.sync.dma_start(xt_sb[:, 0:n_here * P],
                                          xt_d[:, lo * P:(lo + n_here) * P])
                        for g0 in range(0, n_here, 3):
                            g_n = min(3, n_here - g0)
                            zps = pap.tile([P, 3, 80], F32, space="PSUM", tag="zps", name="zps")
                            for t in range(g_n):
                                nc.tensor.matmul(zps[:, t, :],
                                                 lhsT=xt_sb[:, (g0 + t) * P:(g0 + t + 1) * P],
                                                 rhs=wcat[g][:])
                            zu = pa.tile([P, 3, 128], U16, tag="zu", name="zu")
                            nc.gpsimd.memset(zu[:, :, 80:128], 0)
                            eng = nc.scalar if (g0 // 3) % 2 == 0 else nc.vector
                            if eng is nc.scalar:
                                nc.scalar.activation(out=zu[:, 0:g_n, 0:64].bitcast(F16),
                                                     in_=zps[:, 0:g_n, 0:64], func=AF.Copy)
                                nc.scalar.activation(out=zu[:, 0:g_n, 64:80].bitcast(F32),
                                                     in_=zps[:, 0:g_n, 64:72], func=AF.Copy)
                            else:
                                nc.vector.tensor_copy(zu[:, 0:g_n, 0:64].bitcast(F16),
                                                      zps[:, 0:g_n, 0:64])
                                nc.vector.tensor_copy(zu[:, 0:g_n, 64:80].bitcast(F32),
                                                      zps[:, 0:g_n, 64:72])
                            r0 = (lo + g0) * P
                            nc.sync.dma_start(
                                tbl[g][r0:r0 + g_n * P, :].rearrange("(t p) c -> p t c", p=P),
                                zu[:, 0:g_n, :])
                    # zero row
                    zr = {"ex0": NT_EX * P, "ex1": NT_EX * P, "st": NT_ST * P, "kn": K}[g]
                    nc.sync.dma_start(tbl[g][zr:zr + 1, :], zrow_sb[:])

            # ---- Phase A2: er per graph ----
            er = {}
            with tc.tile_pool(name="pE", bufs=2) as pe, \
                 tc.tile_pool(name="pE_ps", bufs=2, space="PSUM") as pep:
                for g, xtp_d, ntp in (("ex0", ein["xtp_ex0"], NTP_EX),
                                      ("ex1", ein["xtp_ex1"], NTP_EX),
                                      ("st", ein["xtp_st"], BS_TILES),
                                      ("kn", ein["xt_kn"], 1)):
                    er_sb = slab.tile([P, ntp, 8], F32, tag="er_" + g, name="er_" + g)
                    er[g] = er_sb
                    xtp_sb = pe.tile([P, NTP_EX * P], F16, tag="xtp_sb", name="xtp_sb")
                    nc.sync.dma_start(xtp_sb[:, 0:ntp * P], xtp_d[:])
                    for t in range(ntp):
                        eps = pep.tile([P, 8], F32, space="PSUM", tag="eps", name="eps")
                        nc.tensor.matmul(eps[:], lhsT=xtp_sb[:, t * P:(t + 1) * P],
                                         rhs=war[g][:])
                        nc.vector.tensor_copy(er_sb[:, t, :], eps[:])

            # ---- Phase B: gathers + edge softmax + aggregation ----
            zs = {"ex0": slab.tile([P, NTP_EX, FD], F32, tag="zs_ex0", name="zs_ex0"),
                  "ex1": slab.tile([P, NTP_EX, FD], F32, tag="zs_ex1", name="zs_ex1"),
                  "st": slab.tile([P, BS_TILES, FD], F32, tag="zs_st", name="zs_st"),
                  "kn": slab.tile([P, 1, FD], F32, tag="zs_kn", name="zs_kn")}

            with tc.tile_pool(name="pB", bufs=2) as pb, \
                 tc.tile_pool(name="pBs", bufs=2) as pbs:
                for g in ("ex0", "ex1", "st", "kn"):
                    plan = plans[g]
                    col0 = 0
                    for (t_lo, T, Dt) in plan.chunks:
                        NIDX = P * T * Dt
                        gat = pb.tile([P, T * Dt, 128], U16, tag="gat", name="gat")
                        nc.gpsimd.dma_gather(
                            gat[:], tbl[g][:, :],
                            idx_sb[g][:, col0 * 8:(col0 + T * Dt) * 8],
                            NIDX, NIDX, 128, single_packet=False)
                        zf = gat[:].bitcast(F16)
                        elg = gat[:].bitcast(F32)[:, :, 32:40].rearrange(
                            "p (t d) h -> p t d h", t=T)
                        e = pbs.tile([P, T, Dt, 8], F32, tag="e_buf", name="e_buf")
                        nc.vector.tensor_tensor(
                            out=e[:], in0=elg,
                            in1=er[g][:, t_lo:t_lo + T, :].unsqueeze(2).to_broadcast(
                                [P, T, Dt, 8]),
                            op=OP.add)
                        e2 = pbs.tile([P, T, Dt, 8], F32, tag="e2_buf", name="e2_buf")
                        nc.vector.tensor_scalar_mul(e2[:], e[:], 0.2)
                        nc.vector.tensor_tensor(out=e2[:], in0=e2[:], in1=e[:], op=OP.max)
                        m = pbs.tile([P, T, 8], F32, tag="m_buf", name="m_buf")
                        nc.vector.tensor_reduce(out=m[:], in_=e2[:].transpose([0, 1, 3, 2]),
                                                axis=AX.X, op=OP.max)
                        nc.vector.tensor_tensor(
                            out=e2[:], in0=e2[:],
                            in1=m[:].unsqueeze(2).to_broadcast([P, T, Dt, 8]),
                            op=OP.subtract)
                        exb = pbs.tile([P, T, Dt, 8], F16, tag="exb_buf", name="exb_buf")
                        nc.scalar.activation(out=exb[:], in_=e2[:], func=AF.Exp)
                        s = pbs.tile([P, T, 8], F32, tag="s_buf", name="s_buf")
                        nc.vector.tensor_reduce(out=s[:], in_=exb[:].transpose([0, 1, 3, 2]),
                                                axis=AX.X, op=OP.add)
                        rs = pbs.tile([P, T, 8], F32, tag="rs_buf", name="rs_buf")
                        nc.vector.tensor_scalar_add(s[:], s[:], 1e-9)
                        nc.vector.reciprocal(rs[:], s[:])
                        w = pbs.tile([P, T * Dt, 64], F16, tag="w_buf", name="w_buf")
                        nc.vector.tensor_tensor(
                            out=w[:].rearrange("p s (h f) -> p s h f", h=8),
                            in0=zf[:, :, 0:64].rearrange("p s (h f) -> p s h f", h=8),
                            in1=exb[:].rearrange("p t d h -> p (t d) h").unsqueeze(3)
                            .to_broadcast([P, T * Dt, 8, 8]),
                            op=OP.mult)
                        exe = pbs.tile([P, T * Dt, 64], F16, tag="exe_buf", name="exe_buf")
                        # per-tile tree reduction over d, then normalize by 1/s
                        for t in range(T):
                            wt = w[:, t * Dt:(t + 1) * Dt, :]
                            dcur = Dt
                            scratch = exe  # dead after the w-mult; reuse as tree scratch
                            cur = wt
                            while dcur > 1:
                                half = dcur // 2
                                dst = scratch[:, 0:(dcur + 1) // 2, :]
                                nc.vector.tensor_tensor(
                                    out=dst[:, 0:half, :],
                                    in0=cur[:, 0:2 * half:2, :],
                                    in1=cur[:, 1:2 * half:2, :], op=OP.add)
                                if dcur % 2:
                                    nc.vector.tensor_copy(dst[:, half:half + 1, :],
                                                          cur[:, dcur - 1:dcur, :])
                                cur = dst
                                dcur = (dcur + 1) // 2
                            out_t = zs[g][:, t_lo + t, :]
                            nc.vector.tensor_tensor(
                                out=out_t.rearrange("p (h f) -> p h f", h=H),
                                in0=cur[:, 0, :].rearrange("p (h f) -> p h f", h=H),
                                in1=rs[:, t, :].unsqueeze(2).to_broadcast([P, H, D]),
                                op=OP.mult)
                        # elu on this chunk's node rows
                        v = zs[g][:, t_lo:t_lo + T, :]
                        t1 = pbs.tile([P, T, FD], F32, tag="elu1", name="elu1")
                        nc.vector.tensor_scalar_min(t1[:], v, 0.0)
                        t2 = pbs.tile([P, T, FD], F32, tag="elu2", name="elu2")
                        nc.scalar.activation(out=t2[:], in_=t1[:], func=AF.Exp)
                        nc.vector.tensor_tensor(out=v, in0=v, in1=t1[:], op=OP.subtract)
                        nc.vector.scalar_tensor_tensor(out=v, in0=t2[:], scalar=-1.0,
                                                       in1=v, op0=OP.add, op1=OP.add)
                        col0 += T * Dt

            # ---- Phase C: transposes + semantic attention stats ----
            zsT = {"ex0": slab.tile([FD, NTP_EX * P], F32, tag="zsT_ex0", name="zsT_ex0"),
                   "ex1": slab.tile([FD, NTP_EX * P], F32, tag="zsT_ex1", name="zsT_ex1"),
                   "st": slab.tile([FD, BS_TILES * P], F32, tag="zsT_st", name="zsT_st"),
                   "kn": slab.tile([FD, K], F32, tag="zsT_kn", name="zsT_kn")}
            with tc.tile_pool(name="pC_ps", bufs=4, space="PSUM") as pcp:
                for g, ntp in (("ex0", NTP_EX), ("ex1", NTP_EX), ("st", BS_TILES), ("kn", 1)):
                    for t in range(ntp):
                        tp = pcp.tile([FD, P], F32, space="PSUM", tag="tp_ps", name="tp_ps")
                        nc.tensor.transpose(out=tp[:], in_=zs[g][:, t, :], identity=ident[:])
                        eng = nc.scalar if t % 2 == 0 else nc.vector
                        if eng is nc.scalar:
                            nc.scalar.copy(zsT[g][:, t * P:(t + 1) * P], tp[:])
                        else:
                            nc.vector.tensor_copy(zsT[g][:, t * P:(t + 1) * P], tp[:])

            stats = cst.tile([1, 16], F32, tag="stats", name="stats")
            nc.vector.memset(stats[:], 0.0)
            with tc.tile_pool(name="pD", bufs=2) as pd, \
                 tc.tile_pool(name="pD_ps", bufs=4, space="PSUM") as pdp:
                nch = 0
                parts = cst.tile([1, 16], F32, tag="parts", name="parts")
                for mi, g in enumerate(("ex0", "ex1")):
                    cw_list = []
                    lo = 0
                    while lo < SH:
                        cw = min(512, SH - lo)
                        cw_list.append((lo, cw))
                        lo += cw
                    for ci, (lo, cw) in enumerate(cw_list):
                        tps = pdp.tile([SEM, 512], F32, space="PSUM", tag="tps", name="tps")
                        nc.tensor.matmul(tps[:, 0:cw], lhsT=semW[:], rhs=zsT[g][:, lo:lo + cw])
                        tsb = pd.tile([SEM, 512], F32, tag="tsb", name="tsb")
                        nc.scalar.activation(out=tsb[:, 0:cw], in_=tps[:, 0:cw],
                                             func=AF.Tanh, bias=semb_col[:])
                        rps = pdp.tile([1, 512], F32, space="PSUM", tag="rps", name="rps")
                        nc.tensor.matmul(rps[:, 0:cw], lhsT=semq_col[:], rhs=tsb[:, 0:cw])
                        nc.vector.tensor_reduce(out=parts[:, mi * 8 + ci:mi * 8 + ci + 1],
                                                in_=rps[:, 0:cw], axis=AX.X, op=OP.add)
                    nc.vector.tensor_reduce(
                        out=stats[:, mi:mi + 1],
                        in_=parts[:, mi * 8:mi * 8 + len(cw_list)], axis=AX.X, op=OP.add)
                    nch = len(cw_list)

            # ---- AllReduce the 2 stats scalars ----
            nc.sync.dma_start(cc_in[:, 0:16], stats[:])
            nc.gpsimd.collective_compute(
                "AllReduce", OP.add,
                replica_groups=[list(range(NC))],
                ins=[cc_in[:, :]], outs=[cc_out[:, :]])
            gstats = cst.tile([1, 16], F32, tag="gstats", name="gstats")
            nc.sync.dma_start(gstats[:], cc_out[:, :])

            # ---- Phase E: predictor prep ----
            beta_col = cst.tile([P, 2], F32, tag="beta_col", name="beta_col")
            bd = cst.tile([1, 2], F32, tag="bd", name="bd")
            nc.vector.tensor_tensor(out=bd[:, 0:1], in0=gstats[:, 0:1],
                                    in1=gstats[:, 1:2], op=OP.subtract)
            btmp = cst.tile([1, 2], F32, tag="btmp", name="btmp")
            _bsc = float(os.environ.get("KERNEL_BETA_SCALE", "1.0"))
            nc.scalar.activation(out=btmp[:, 0:1], in_=bd[:, 0:1], func=AF.Sigmoid,
                                 scale=_bsc / E_N)
            nc.scalar.activation(out=btmp[:, 1:2], in_=bd[:, 0:1], func=AF.Sigmoid,
                                 scale=-_bsc / E_N)
            b3_col = cst.tile([P, 1], F32, tag="b3_col", name="b3_col")
            with tc.tile_pool(name="bc2_ps", bufs=2, space="PSUM") as bc2:
                bb_ps = bc2.tile([P, 4], F32, space="PSUM", tag="bb_ps", name="bb_ps")
                nc.tensor.matmul(bb_ps[:, 0:2], lhsT=ones_row[:], rhs=btmp[:])
                nc.tensor.matmul(bb_ps[:, 2:3], lhsT=ones_row[:], rhs=b3[:])
                nc.vector.tensor_copy(beta_col[:], bb_ps[:, 0:2])
                nc.vector.tensor_copy(b3_col[:], bb_ps[:, 2:3])

            # fused exercise b-slot features: zsFT = b0*zsT_ex0 + b1*zsT_ex1
            zsFT = cst.tile([FD, BC], F32, tag="zsFT", name="zsFT")
            bcol = SH_TILES * P
            nc.vector.tensor_scalar(out=zsFT[:], in0=zsT["ex0"][:, bcol:bcol + BC],
                                    scalar1=beta_col[0:FD, 0:1], scalar2=None,
                                    op0=OP.mult)
            nc.vector.scalar_tensor_tensor(out=zsFT[:], in0=zsT["ex1"][:, bcol:bcol + BC],
                                           scalar=beta_col[0:FD, 1:2], in1=zsFT[:],
                                           op0=OP.mult, op1=OP.add)

            qt_sb = cst.tile([P, K], F32, tag="qt_sb", name="qt_sb")
            st_sb = cst.tile([P, K], F32, tag="st_sb", name="st_sb")
            m1_sb = cst.tile([FD, K], F32, tag="m1_sb", name="m1_sb")
            m2_sb = cst.tile([FD, K], F32, tag="m2_sb", name="m2_sb")
            c1t = cst.tile([P, 1], F32, tag="c1t", name="c1t")
            c2t = cst.tile([P, 1], F32, tag="c2t", name="c2t")
            kn1T = cst.tile([P, K], F32, tag="kn1T", name="kn1T")
            with tc.tile_pool(name="pF_ps", bufs=2, space="PSUM") as pfp:
                kn1_ps = pfp.tile([P, K], F32, space="PSUM", tag="prep_ps", name="kn1_ps")
                nc.tensor.matmul(kn1_ps[:], lhsT=zsT["kn"][:], rhs=pW_kn[:],
                                 start=True, stop=False)
                nc.tensor.matmul(kn1_ps[:], lhsT=ones_row[:], rhs=pb_kn_row[:],
                                 start=False, stop=True)
                kn1_sb = cst.tile([P, K], F32, tag="kn1_sb", name="kn1_sb")
                nc.scalar.copy(kn1_sb[:], kn1_ps[:])
                kn1T_ps = pfp.tile([P, K], F32, space="PSUM", tag="prep_ps", name="kn1T_ps")
                nc.tensor.transpose(out=kn1T_ps[:], in_=kn1_sb[:], identity=ident[:])
                nc.scalar.copy(kn1T[:], kn1T_ps[:])

                qs_ps = pfp.tile([P, K], F32, space="PSUM", tag="prep_ps", name="qs_ps")
                nc.tensor.matmul(qs_ps[:], lhsT=W1b[:], rhs=kn1T[:])
                nc.scalar.copy(qt_sb[:], qs_ps[:])
                qs2_ps = pfp.tile([P, K], F32, space="PSUM", tag="prep_ps", name="qs2_ps")
                nc.tensor.matmul(qs2_ps[:], lhsT=W2b[:], rhs=kn1T[:])
                nc.scalar.copy(st_sb[:], qs2_ps[:])

                m1_ps = pfp.tile([FD, K], F32, space="PSUM", tag="prep_ps", name="m1_ps")
                nc.tensor.matmul(m1_ps[:], lhsT=pWT_st[:], rhs=W1a[:])
                nc.scalar.copy(m1_sb[:], m1_ps[:])
                m2_ps = pfp.tile([FD, K], F32, space="PSUM", tag="prep_ps", name="m2_ps")
                nc.tensor.matmul(m2_ps[:], lhsT=pWT_ex[:], rhs=W2a[:])
                nc.scalar.copy(m2_sb[:], m2_ps[:])
                c1_ps = pfp.tile([P, 1], F32, space="PSUM", tag="prep_ps", name="c1_ps")
                nc.tensor.matmul(c1_ps[:], lhsT=W1a[:], rhs=pb_st[:])
                nc.vector.tensor_copy(c1t[:], c1_ps[:])
                c2_ps = pfp.tile([P, 1], F32, space="PSUM", tag="prep_ps", name="c2_ps")
                nc.tensor.matmul(c2_ps[:], lhsT=W2a[:], rhs=pb_ex[:])
                nc.vector.tensor_copy(c2t[:], c2_ps[:])

            # ---- Phase F: predictor main loop ----
            GRP = 4   # batch rows per psum group
            with tc.tile_pool(name="pG", bufs=3) as pg, \
                 tc.tile_pool(name="pG_ps", bufs=2, space="PSUM") as pgp, \
                 tc.tile_pool(name="pO_ps", bufs=1, space="PSUM") as pop:
                o_ps = pop.tile([P, BC], F32, space="PSUM", tag="o_ps", name="o_ps")
                for grp in range(BC // GRP):
                    b0 = grp * GRP
                    pr_ps = pgp.tile([P, GRP * K], F32, space="PSUM", tag="pr_ps", name="pr_ps")
                    nc.tensor.matmul(pr_ps[:], lhsT=W1b[:],
                                     rhs=kn1T[:].unsqueeze(1).to_broadcast([P, GRP, K]),
                                     start=True, stop=False)
                    nc.tensor.matmul(pr_ps[:], lhsT=m1_sb[:],
                                     rhs=zsT["st"][:, b0:b0 + GRP].unsqueeze(2)
                                     .to_broadcast([FD, GRP, K]),
                                     start=False, stop=True)
                    pr_sb = pg.tile([P, GRP * K], F16, tag="pr_sb", name="pr_sb")
                    nc.scalar.activation(out=pr_sb[:], in_=pr_ps[:], func=AF.Sigmoid,
                                         bias=c1t[:])
                    df_ps = pgp.tile([P, GRP * K], F32, space="PSUM", tag="df_ps", name="df_ps")
                    nc.tensor.matmul(df_ps[:], lhsT=W2b[:],
                                     rhs=kn1T[:].unsqueeze(1).to_broadcast([P, GRP, K]),
                                     start=True, stop=False)
                    nc.tensor.matmul(df_ps[:], lhsT=m2_sb[:],
                                     rhs=zsFT[:, b0:b0 + GRP].unsqueeze(2)
                                     .to_broadcast([FD, GRP, K]),
                                     start=False, stop=True)
                    df_sb = pg.tile([P, GRP * K], F16, tag="df_sb", name="df_sb")
                    nc.scalar.activation(out=df_sb[:], in_=df_ps[:], func=AF.Sigmoid,
                                         bias=c2t[:])
                    d_sb = pg.tile([P, GRP * K], F16, tag="d_sb", name="d_sb")
                    nc.vector.tensor_tensor(out=d_sb[:], in0=pr_sb[:], in1=df_sb[:],
                                            op=OP.subtract)
                    for lb in range(GRP):
                        nc.tensor.matmul(o_ps[:, b0 + lb:b0 + lb + 1],
                                         lhsT=d_sb[:, lb * K:(lb + 1) * K], rhs=W3h[:])

                # ---- Phase G: final ----
                o_sb = pg.tile([P, BC], F32, tag="o_sb", name="o_sb")
                nc.scalar.activation(out=o_sb[:], in_=o_ps[:], func=AF.Sigmoid,
                                     bias=b3_col[:])
                om = pg.tile([P, BC], F32, tag="om", name="om")
                nc.vector.tensor_tensor(out=om[:], in0=o_sb[:], in1=kn_rT[:], op=OP.mult)
                nd_ps = pgp.tile([1, 2 * BC], F32, space="PSUM", tag="nd_ps", name="nd_ps")
                nc.tensor.matmul(nd_ps[:, 0:BC], lhsT=ones_col[:], rhs=om[:])
                nc.tensor.matmul(nd_ps[:, BC:2 * BC], lhsT=ones_col[:], rhs=kn_rT[:])
                rcp = pg.tile([1, BC], F32, tag="rcp", name="rcp")
                nc.vector.reciprocal(rcp[:], nd_ps[:, BC:2 * BC])
                res = pg.tile([1, BC], F32, tag="res", name="res")
                nc.vector.tensor_tensor(out=res[:], in0=nd_ps[:, 0:BC], in1=rcp[:],
                                        op=OP.mult)
                nc.sync.dma_start(out_d[:], res[:])

    nc.compile()
    return nc


# ----------------------------------------------------------------------------
# Entry point
# ----------------------------------------------------------------------------

_TRACE = bool(int(os.environ.get("KERNEL_TRACE", "0")))


def kernel(**inputs):
    meta, in_maps = preprocess(inputs)
    nc = build_program(meta)
    res = bass_utils.run_bass_kernel_spmd(
        nc, in_maps, core_ids=list(range(NC)), trace=_TRACE)
    out = np.concatenate([r["out"].reshape(-1) for r in res.results])
    kernel.last_results = res
    return out.reshape(B, 1).astype(np.float32)



# revision 7
# speedup vs baseline: 4.4467x; 4.4467x over previous
"""Trainium2 Bass kernel for the HAN-based cognitive-diagnosis net.

Strategy (8 NeuronCores, SPMD — one program, per-core data):
  * Batch (2048) split 8x256 across cores. Each core computes GAT outputs
    for its own batch-slice node list plus a 1/8 share of all exercise
    nodes (for the global-mean semantic attention stats, AllReduce'd).
  * Edge phase without any device gather: the edge lists are known on the
    host, so the host pre-expands x^T into the ELL slot layout (one
    x-column per edge slot, zero column for pad slots).  The device
    computes per-edge [z(64) | el(8)] directly with PE matmuls
    (lhsT = xts slot tile, rhs = W folded with a_l), then runs the
    edge softmax + weighted aggregation on DVE/ACT with dst nodes on
    partitions.  Pad slots have z=0 (exact numerator); the softmax
    denominator is corrected analytically:
        s_real = s - npad[d] * exp(leaky(er[d]) - m[d])
    since every pad slot contributes exactly that one value.
  * Predictor exploits the rank structure pref[b,j,k] =
    sigmoid(U1[b,k] + Q1[j,k] + c1[k]): two small matmuls build U/Q once,
    the [B,K,K] tensors are pure broadcast-add + sigmoid, and the W3
    contraction runs on PE per batch row.
"""

import os
import numpy as np

import concourse.bass as bass
import concourse.bacc as bacc
import concourse.mybir as mybir
import concourse.tile as tile
from concourse import library_config
from concourse.masks import make_identity
from concourse import bass_utils

F32 = mybir.dt.float32
F16 = mybir.dt.float16

NC = 8
B = 2048
BC = B // NC          # 256 batch rows per core
K = 128
H, D, FD = 8, 8, 64
SEM = 128
S_N, E_N = 10000, 20000
P = 128

AX = mybir.AxisListType
OP = mybir.AluOpType
AF = mybir.ActivationFunctionType

GRAPHS = ("ex0", "ex1", "st", "kn")


# ----------------------------------------------------------------------------
# Host-side preprocessing (integer / layout only)
# ----------------------------------------------------------------------------

def _csr_by_dst(src, dst, n):
    order = np.argsort(dst, kind="stable")
    ss = src[order].astype(np.int64)
    counts = np.bincount(dst, minlength=n)
    rowptr = np.zeros(n + 1, np.int64)
    np.cumsum(counts, out=rowptr[1:])
    return ss, rowptr, counts


def _tiles_of(nodes):
    return [np.asarray(nodes[i:i + P]) for i in range(0, len(nodes), P)]


def _flat_src(node_tiles, dts, ss, rowptr, counts, npad_row):
    """Flat per-edge-slot src index list (tile-major, col-major, 128 lanes)
    with -1 for pad slots, plus the [128, ntiles] pad-count array."""
    nslot = int(np.sum(dts))
    flat = np.full((nslot, P), -1, np.int64)
    npad = np.zeros((P, len(node_tiles)), np.float32)
    col = 0
    for t, nodes in enumerate(node_tiles):
        dt = int(dts[t])
        for pi, node in enumerate(nodes):
            deg = int(counts[node])
            if deg:
                lo = rowptr[node]
                flat[col:col + deg, pi] = ss[lo:lo + deg]
            npad[pi, t] = -(dt - deg)
        for pi in range(len(nodes), P):
            npad[pi, t] = -dt
        col += dt
    assert col == nslot
    return flat.reshape(-1), npad


def _xts(x_aug_T, flat):
    """[K, nslot*128] fp16 ELL-expanded x^T (zero column for pad slots)."""
    return np.ascontiguousarray(x_aug_T[:, flat])


def _xtp(x, node_tiles, ntiles):
    """x^T columns for a node list, padded to ntiles*128 cols, fp16."""
    kdim = x.shape[1]
    out = np.zeros((kdim, ntiles * P), np.float16)
    for t, nodes in enumerate(node_tiles):
        out[:, t * P:t * P + len(nodes)] = x[nodes].T.astype(np.float16)
    return out


def preprocess(inputs):
    inp = {k: np.asarray(v) for k, v in inputs.items()}
    stu_id = inp["stu_id"].astype(np.int64)
    exer_id = inp["exer_id"].astype(np.int64)

    csr = {
        "st": _csr_by_dst(inp["ss0"].astype(np.int64), inp["sd0"].astype(np.int64), S_N),
        "ex0": _csr_by_dst(inp["es0"].astype(np.int64), inp["ed0"].astype(np.int64), E_N),
        "ex1": _csr_by_dst(inp["es1"].astype(np.int64), inp["ed1"].astype(np.int64), E_N),
        "kn": _csr_by_dst(inp["ks0"].astype(np.int64), inp["kd0"].astype(np.int64), K),
    }

    # per-core node tile lists; exercise share nodes degree-sorted + strided
    SH = E_N // NC                      # 2500
    SH_TILES = (SH + P - 1) // P        # 20
    BS_TILES = BC // P                  # 2
    NTP = {"ex0": SH_TILES + BS_TILES, "ex1": SH_TILES + BS_TILES,
           "st": BS_TILES, "kn": 1}

    tiles = {g: [] for g in GRAPHS}     # g -> [core][tile] node arrays
    for g in ("ex0", "ex1"):
        order = np.argsort(-csr[g][2], kind="stable")
        for c in range(NC):
            share = order[c::NC]
            assert len(share) == SH
            tiles[g].append(_tiles_of(share) +
                            _tiles_of(exer_id[c * BC:(c + 1) * BC]))
    for c in range(NC):
        tiles["st"].append(_tiles_of(stu_id[c * BC:(c + 1) * BC]))
        tiles["kn"].append(_tiles_of(np.arange(K)))

    # shared per-tile Dt = max over cores (SPMD: one program)
    dts = {}
    for g in GRAPHS:
        counts = csr[g][2]
        dts[g] = np.max(
            [[max(1, int(counts[t].max()) if len(t) else 1) for t in tiles[g][c]]
             for c in range(NC)], axis=0)

    meta = dict(dts=dts, NTP=NTP, SH=SH, SH_TILES=SH_TILES, BS_TILES=BS_TILES,
                nslot={g: int(dts[g].sum()) for g in GRAPHS})

    # ------- shared input arrays -------
    shared = {
        "w_ex0": inp["f3W0"].astype(np.float16),
        "w_ex1": inp["f3W1"].astype(np.float16),
        "w_st": inp["f1W0"].astype(np.float16),
        "w_kn": inp["f5W0"].astype(np.float16),
        "alr_ex0": np.concatenate([inp["f3al0"].reshape(1, 64), inp["f3ar0"].reshape(1, 64)], 1),
        "alr_ex1": np.concatenate([inp["f3al1"].reshape(1, 64), inp["f3ar1"].reshape(1, 64)], 1),
        "alr_st": np.concatenate([inp["f1al0"].reshape(1, 64), inp["f1ar0"].reshape(1, 64)], 1),
        "alr_kn": np.concatenate([inp["f5al0"].reshape(1, 64), inp["f5ar0"].reshape(1, 64)], 1),
        "semW": inp["f3sW"].astype(np.float32),
        "semb_col": inp["f3sb"].reshape(SEM, 1).astype(np.float32),
        "semq_col": inp["f3sq"].reshape(SEM, 1).astype(np.float32),
        "pWT_st": inp["f1pW"].T.astype(np.float32).copy(),
        "pb_st": inp["f1pb"].reshape(K, 1).astype(np.float32),
        "pWT_ex": inp["f3pW"].T.astype(np.float32).copy(),
        "pb_ex": inp["f3pb"].reshape(K, 1).astype(np.float32),
        "pW_kn": inp["f5pW"].astype(np.float32),
        "pb_kn_row": inp["f5pb"].reshape(1, K).astype(np.float32),
        "W1a": inp["W1"][:K].astype(np.float32),
        "W1b": inp["W1"][K:].astype(np.float32),
        "W2a": inp["W2"][:K].astype(np.float32),
        "W2b": inp["W2"][K:].astype(np.float32),
        "W3h": inp["W3"].astype(np.float16),
        "b3": inp["b3"].reshape(1, 1).astype(np.float32),
    }

    xsrc = {"ex0": inp["exer_t"], "ex1": inp["exer_t"],
            "st": inp["stu_t"], "kn": inp["kn_t"]}
    x_aug_T = {}
    for g in GRAPHS:
        xa = np.vstack([xsrc[g], np.zeros((1, K), np.float32)]).astype(np.float16)
        x_aug_T[g] = np.ascontiguousarray(xa.T)   # [K, N+1], col N = zeros

    # ------- per-core arrays -------
    in_maps = []
    for c in range(NC):
        m = dict(shared)
        for g in GRAPHS:
            ss, rowptr, counts = csr[g]
            flat, npad = _flat_src(tiles[g][c], dts[g], ss, rowptr, counts, None)
            flat = np.where(flat < 0, xsrc[g].shape[0], flat)
            m["xts_" + g] = _xts(x_aug_T[g], flat)
            m["npad_" + g] = npad
            m["xtp_" + g] = _xtp(xsrc[g], tiles[g][c], NTP[g])
        m["kn_rT"] = inp["kn_r"][c * BC:(c + 1) * BC].T.astype(np.float32).copy()
        in_maps.append(m)

    return meta, in_maps


# ----------------------------------------------------------------------------
# Bass program
# ----------------------------------------------------------------------------

DMA_COLS = 32    # xts slot-cols per input DMA
PS_COLS = 16     # slot-cols per PSUM tile ([P,16,128] f32 = 4 banks, no straddle)
FG = 8           # predictor batch rows per group


def build_program(meta):
    nc = bacc.Bacc("TRN2", num_devices=NC)
    dts = meta["dts"]
    NTP = meta["NTP"]
    SH = meta["SH"]
    SH_TILES, BS_TILES = meta["SH_TILES"], meta["BS_TILES"]
    nslot = meta["nslot"]
    MAXDT = -(-max(int(dts[g].max()) for g in GRAPHS) // 8) * 8

    ein = {}
    def EIN(name, shape, dt):
        ein[name] = nc.dram_tensor(name, list(shape), dt, kind="ExternalInput")
        return ein[name]

    for g in GRAPHS:
        EIN("w_" + g, (K, FD), F16)
        EIN("alr_" + g, (1, 128), F32)
        EIN("xts_" + g, (K, nslot[g] * P), F16)
        EIN("npad_" + g, (P, len(dts[g])), F32)
        EIN("xtp_" + g, (K, NTP[g] * P), F16)
    EIN("semW", (FD, SEM), F32); EIN("semb_col", (SEM, 1), F32); EIN("semq_col", (SEM, 1), F32)
    EIN("pWT_st", (K, FD), F32); EIN("pb_st", (K, 1), F32)
    EIN("pWT_ex", (K, FD), F32); EIN("pb_ex", (K, 1), F32)
    EIN("pW_kn", (FD, K), F32); EIN("pb_kn_row", (1, K), F32)
    EIN("W1a", (K, K), F32); EIN("W1b", (K, K), F32)
    EIN("W2a", (K, K), F32); EIN("W2b", (K, K), F32)
    EIN("W3h", (K, 1), F16); EIN("b3", (1, 1), F32)
    EIN("kn_rT", (K, BC), F32)

    out_d = nc.dram_tensor("out", [1, BC], F32, kind="ExternalOutput")

    cc_in = nc.dram_tensor("cc_in", [1, 16], F32, kind="Internal")
    cc_out = nc.dram_tensor("cc_out", [1, 16], F32, kind="Internal", addr_space="Shared")

    with tile.TileContext(nc) as tc:
        with tc.tile_pool(name="const", bufs=1) as cst, \
             tc.tile_pool(name="slab", bufs=1) as slab:
            nc.gpsimd.load_library(library_config.mlp)

            ident = cst.tile([P, P], F32, tag="ident", name="ident")
            make_identity(nc, ident[:])
            ones_col = cst.tile([P, 1], F32, tag="ones_col", name="ones_col")
            nc.vector.memset(ones_col[:], 1.0)
            ones_row = cst.tile([1, P], F32, tag="ones_row", name="ones_row")
            nc.vector.memset(ones_row[:], 1.0)

            # ---- load small weights ----
            def load(name, shape, dt):
                t = cst.tile(list(shape), dt, tag="ld_" + name, name="ld_" + name)
                nc.sync.dma_start(t[:], ein[name][:])
                return t
            w_g = {g: load("w_" + g, (K, FD), F16) for g in GRAPHS}
            alr = {g: load("alr_" + g, (1, 128), F32) for g in GRAPHS}
            npad_sb = {g: load("npad_" + g, (P, len(dts[g])), F32) for g in GRAPHS}
            semW = load("semW", (FD, SEM), F32)
            semb_col = load("semb_col", (SEM, 1), F32)
            semq_col = load("semq_col", (SEM, 1), F32)
            pWT_st = load("pWT_st", (K, FD), F32); pb_st = load("pb_st", (K, 1), F32)
            pWT_ex = load("pWT_ex", (K, FD), F32); pb_ex = load("pb_ex", (K, 1), F32)
            pW_kn = load("pW_kn", (FD, K), F32); pb_kn_row = load("pb_kn_row", (1, K), F32)
            W1a = load("W1a", (K, K), F32); W1b = load("W1b", (K, K), F32)
            W2a = load("W2a", (K, K), F32); W2b = load("W2b", (K, K), F32)
            W3h = load("W3h", (K, 1), F16); b3 = load("b3", (1, 1), F32)
            kn_rT = load("kn_rT", (K, BC), F32)

            # ---- fold al/ar into W: wcat = [W | W@al] fp16, war = W@ar ----
            wcat = {}   # [128, 72] f16: cols 0:64 W, 64:72 Wal
            war = {}    # [128, 8] f16
            with tc.tile_pool(name="bc_ps", bufs=2, space="PSUM") as bcp:
              for g in GRAPHS:
                alb = cst.tile([P, 128], F32, tag="alb", name="alb")
                alb_ps = bcp.tile([P, 128], F32, space="PSUM", tag="alb_ps", name="alb_ps")
                nc.tensor.matmul(alb_ps[:], lhsT=ones_row[:], rhs=alr[g][:])
                nc.vector.tensor_copy(alb[:], alb_ps[:])
                wf = cst.tile([P, FD], F32, tag="wf", name="wf")
                nc.vector.tensor_copy(wf[:], w_g[g][:])
                wtmp = cst.tile([P, FD], F32, tag="wtmp", name="wtmp")
                wc = cst.tile([P, 72], F16, tag="wcat_" + g, name="wcat_" + g)
                wcat[g] = wc
                nc.vector.tensor_copy(wc[:, 0:64], w_g[g][:])
                with nc.allow_low_precision(reason="8-elem head fold of fp16 weights"):
                    nc.vector.tensor_tensor(out=wtmp[:], in0=wf[:], in1=alb[:, 0:64], op=OP.mult)
                    nc.vector.tensor_reduce(out=wc[:, 64:72].bitcast(F16),
                                            in_=wtmp[:].rearrange("p (h f) -> p h f", h=H),
                                            axis=AX.X, op=OP.add)
                    wr = cst.tile([P, 8], F16, tag="war_" + g, name="war_" + g)
                    war[g] = wr
                    nc.vector.tensor_tensor(out=wtmp[:], in0=wf[:], in1=alb[:, 64:128], op=OP.mult)
                    nc.vector.tensor_reduce(out=wr[:], in_=wtmp[:].rearrange("p (h f) -> p h f", h=H),
                                            axis=AX.X, op=OP.add)

            # ---- er per (graph, tile): er[d, t, h] = (x[d] @ war)[h] ----
            er = {}
            with tc.tile_pool(name="pE", bufs=2) as pe, \
                 tc.tile_pool(name="pE_ps", bufs=4, space="PSUM") as pep:
                for g in GRAPHS:
                    ntp = NTP[g]
                    er_sb = slab.tile([P, ntp, 8], F32, tag="er_" + g, name="er_" + g)
                    er[g] = er_sb
                    xtp_sb = pe.tile([P, NTP["ex0"] * P], F16, tag="xtp_sb", name="xtp_sb")
                    nc.sync.dma_start(xtp_sb[:, 0:ntp * P], ein["xtp_" + g][:])
                    for t in range(ntp):
                        eps = pep.tile([P, 8], F32, space="PSUM", tag="eps", name="eps")
                        nc.tensor.matmul(eps[:], lhsT=xtp_sb[:, t * P:(t + 1) * P],
                                         rhs=war[g][:])
                        nc.vector.tensor_copy(er_sb[:, t, :], eps[:])

            # ---- Phase B: per-edge z via PE + edge softmax + aggregation ----
            zs = {g: slab.tile([P, NTP[g], FD], F32, tag="zs_" + g, name="zs_" + g)
                  for g in GRAPHS}

            def do_graph(g, pb, pbs, pzp):
                """Emit z matmuls + softmax for every tile of graph g."""
                col0 = 0
                for t in range(NTP[g]):
                    Dt = int(dts[g][t])
                    DtP = -(-Dt // 8) * 8
                    z_el = pbs.tile([P, MAXDT, 64], F16, tag="z_el", name="z_el")
                    el_t = pbs.tile([P, 8, MAXDT], F32, tag="el_t", name="el_t")
                    # load xts cols + compute z into PSUM, evacuate
                    ncopy = 0
                    for lo in range(0, Dt, DMA_COLS):
                        n_here = min(DMA_COLS, Dt - lo)
                        xts_sb = pb.tile([P, DMA_COLS * P], F16, tag="xts_sb", name="xts_sb")
                        nc.sync.dma_start(
                            xts_sb[:, 0:n_here * P],
                            ein["xts_" + g][:, (col0 + lo) * P:(col0 + lo + n_here) * P])
                        for s0 in range(0, n_here, PS_COLS):
                            s_n = min(PS_COLS, n_here - s0)
                            zps = pzp.tile([P, PS_COLS, 128], F32, space="PSUM",
                                           tag="zps", name="zps")
                            for ci in range(s_n):
                                nc.tensor.matmul(
                                    zps[:, ci, 0:72],
                                    lhsT=xts_sb[:, (s0 + ci) * P:(s0 + ci + 1) * P],
                                    rhs=wcat[g][:])
                            c0 = lo + s0
                            # z copy: 2 scalar : 1 vector; el copy head-major
                            if ncopy % 3 != 2:
                                nc.scalar.activation(out=z_el[:, c0:c0 + s_n, :],
                                                     in_=zps[:, 0:s_n, 0:64], func=AF.Copy)
                            else:
                                nc.vector.tensor_copy(z_el[:, c0:c0 + s_n, :],
                                                      zps[:, 0:s_n, 0:64])
                            nc.scalar.activation(
                                out=el_t[:].transpose([0, 2, 1])[:, c0:c0 + s_n, :],
                                in_=zps[:, 0:s_n, 64:72], func=AF.Copy)
                            ncopy += 1
                    # ---- edge softmax over the Dt slots (head-major) ----
                    e = pbs.tile([P, 8, MAXDT], F32, tag="e_buf", name="e_buf")
                    erb = er[g][:, t, :].unsqueeze(2).to_broadcast([P, 8, Dt])
                    nc.vector.tensor_tensor(out=e[:, :, 0:Dt], in0=el_t[:, :, 0:Dt],
                                            in1=erb, op=OP.add)
                    e2 = pbs.tile([P, 8, MAXDT], F32, tag="e2_buf", name="e2_buf")
                    nc.vector.scalar_tensor_tensor(out=e2[:, :, 0:Dt], in0=e[:, :, 0:Dt],
                                                   scalar=0.2, in1=e[:, :, 0:Dt],
                                                   op0=OP.mult, op1=OP.max)
                    m = pbs.tile([P, 8], F32, tag="m_buf", name="m_buf")
                    nc.vector.tensor_reduce(out=m[:], in_=e2[:, :, 0:Dt],
                                            axis=AX.X, op=OP.max)
                    nc.gpsimd.tensor_tensor(
                        out=e2[:, :, 0:Dt], in0=e2[:, :, 0:Dt],
                        in1=m[:].unsqueeze(2).to_broadcast([P, 8, Dt]), op=OP.subtract)
                    exb = pbs.tile([P, 8, MAXDT], F16, tag="exb_buf", name="exb_buf")
                    nc.scalar.activation(out=exb[:, :, 0:Dt], in_=e2[:, :, 0:Dt], func=AF.Exp)
                    s = pbs.tile([P, 8], F32, tag="s_buf", name="s_buf")
                    nc.vector.tensor_reduce(out=s[:], in_=exb[:, :, 0:Dt],
                                            axis=AX.X, op=OP.add)
                    # pad correction: s += npadneg * exp(leaky(er) - m)
                    ep = pbs.tile([P, 8], F32, tag="ep_buf", name="ep_buf")
                    nc.vector.scalar_tensor_tensor(out=ep[:], in0=er[g][:, t, :],
                                                   scalar=0.2, in1=er[g][:, t, :],
                                                   op0=OP.mult, op1=OP.max)
                    nc.vector.tensor_tensor(out=ep[:], in0=ep[:], in1=m[:], op=OP.subtract)
                    pex = pbs.tile([P, 8], F16, tag="pex_buf", name="pex_buf")
                    nc.scalar.activation(out=pex[:], in_=ep[:], func=AF.Exp)
                    pex32 = pbs.tile([P, 8], F32, tag="pex32_buf", name="pex32_buf")
                    nc.vector.tensor_copy(pex32[:], pex[:])
                    nc.vector.scalar_tensor_tensor(out=s[:], in0=pex32[:],
                                                   scalar=npad_sb[g][:, t:t + 1],
                                                   in1=s[:], op0=OP.mult, op1=OP.add)
                    nc.vector.tensor_scalar_add(s[:], s[:], 1e-9)
                    rs = pbs.tile([P, 8], F32, tag="rs_buf", name="rs_buf")
                    nc.vector.reciprocal(rs[:], s[:])
                    # weighted aggregation: w = z * exb (gpsimd), zero tree pad
                    nc.gpsimd.tensor_tensor(
                        out=z_el[:, 0:Dt, :].rearrange("p s (h f) -> p s h f", h=H),
                        in0=z_el[:, 0:Dt, :].rearrange("p s (h f) -> p s h f", h=H),
                        in1=exb[:, :, 0:Dt].transpose([0, 2, 1]).unsqueeze(3)
                        .to_broadcast([P, Dt, 8, 8]),
                        op=OP.mult)
                    if DtP > Dt:
                        nc.gpsimd.memset(z_el[:, Dt:DtP, :], 0.0)
                    # 3 contiguous halvings then short strided tail reduce
                    cur = DtP
                    while cur > DtP // 8:
                        h2 = cur // 2
                        nc.vector.tensor_tensor(out=z_el[:, 0:h2, :],
                                                in0=z_el[:, 0:h2, :],
                                                in1=z_el[:, h2:cur, :], op=OP.add)
                        cur = h2
                    v = zs[g][:, t, :]
                    nc.vector.tensor_reduce(
                        out=v, in_=z_el[:, 0:cur, :].transpose([0, 2, 1]),
                        axis=AX.X, op=OP.add)
                    nc.vector.tensor_tensor(
                        out=v.rearrange("p (h f) -> p h f", h=H),
                        in0=v.rearrange("p (h f) -> p h f", h=H),
                        in1=rs[:].unsqueeze(2).to_broadcast([P, H, D]), op=OP.mult)
                    # elu
                    t1 = pbs.tile([P, FD], F32, tag="elu1", name="elu1")
                    nc.vector.tensor_scalar_min(t1[:], v, 0.0)
                    t2 = pbs.tile([P, FD], F32, tag="elu2", name="elu2")
                    nc.scalar.activation(out=t2[:], in_=t1[:], func=AF.Exp)
                    nc.vector.tensor_tensor(out=v, in0=v, in1=t1[:], op=OP.subtract)
                    nc.vector.scalar_tensor_tensor(out=v, in0=t2[:], scalar=-1.0,
                                                   in1=v, op0=OP.add, op1=OP.add)
                    col0 += Dt

            zsT = {g: slab.tile([FD, NTP[g] * P], F32, tag="zsT_" + g, name="zsT_" + g)
                   for g in GRAPHS}

            def do_transposes(g, pcp):
                for t in range(NTP[g]):
                    tp = pcp.tile([FD, P], F32, space="PSUM", tag="tp_ps", name="tp_ps")
                    nc.tensor.transpose(out=tp[:], in_=zs[g][:, t, :], identity=ident[:])
                    if t % 2 == 0:
                        nc.scalar.copy(zsT[g][:, t * P:(t + 1) * P], tp[:])
                    else:
                        nc.vector.tensor_copy(zsT[g][:, t * P:(t + 1) * P], tp[:])

            with tc.tile_pool(name="pB", bufs=3) as pb, \
                 tc.tile_pool(name="pBs", bufs=2) as pbs, \
                 tc.tile_pool(name="pB_ps", bufs=2, space="PSUM") as pzp:
                do_graph("ex0", pb, pbs, pzp)
                do_graph("ex1", pb, pbs, pzp)

            with tc.tile_pool(name="pC_ps", bufs=4, space="PSUM") as pcp:
                do_transposes("ex0", pcp)
                do_transposes("ex1", pcp)

            # ---- semantic attention stats over the exercise share ----
            stats = cst.tile([1, 16], F32, tag="stats", name="stats")
            nc.vector.memset(stats[:], 0.0)
            with tc.tile_pool(name="pD", bufs=2) as pd, \
                 tc.tile_pool(name="pD_ps", bufs=4, space="PSUM") as pdp:
                parts = cst.tile([1, 16], F32, tag="parts", name="parts")
                for mi, g in enumerate(("ex0", "ex1")):
                    cw_list = []
                    lo = 0
                    while lo < SH:
                        cw = min(512, SH - lo)
                        cw_list.append((lo, cw))
                        lo += cw
                    for ci, (lo, cw) in enumerate(cw_list):
                        tps = pdp.tile([SEM, 512], F32, space="PSUM", tag="tps", name="tps")
                        nc.tensor.matmul(tps[:, 0:cw], lhsT=semW[:], rhs=zsT[g][:, lo:lo + cw])
                        tsb = pd.tile([SEM, 512], F32, tag="tsb", name="tsb")
                        nc.scalar.activation(out=tsb[:, 0:cw], in_=tps[:, 0:cw],
                                             func=AF.Tanh, bias=semb_col[:])
                        rps = pdp.tile([1, 512], F32, space="PSUM", tag="rps", name="rps")
                        nc.tensor.matmul(rps[:, 0:cw], lhsT=semq_col[:], rhs=tsb[:, 0:cw])
                        nc.vector.tensor_reduce(out=parts[:, mi * 8 + ci:mi * 8 + ci + 1],
                                                in_=rps[:, 0:cw], axis=AX.X, op=OP.add)
                    nc.vector.tensor_reduce(
                        out=stats[:, mi:mi + 1],
                        in_=parts[:, mi * 8:mi * 8 + len(cw_list)], axis=AX.X, op=OP.add)

            # ---- AllReduce the 2 stats scalars (overlapped with st/kn) ----
            nc.sync.dma_start(cc_in[:, 0:16], stats[:])
            nc.gpsimd.collective_compute(
                "AllReduce", OP.add,
                replica_groups=[list(range(NC))],
                ins=[cc_in[:, :]], outs=[cc_out[:, :]])

            with tc.tile_pool(name="pB2", bufs=3) as pb, \
                 tc.tile_pool(name="pB2s", bufs=2) as pbs, \
                 tc.tile_pool(name="pB2_ps", bufs=2, space="PSUM") as pzp:
                do_graph("st", pb, pbs, pzp)
                do_graph("kn", pb, pbs, pzp)

            with tc.tile_pool(name="pC2_ps", bufs=4, space="PSUM") as pcp:
                do_transposes("st", pcp)
                do_transposes("kn", pcp)

            gstats = cst.tile([1, 16], F32, tag="gstats", name="gstats")
            nc.sync.dma_start(gstats[:], cc_out[:, :])

            # ---- predictor prep ----
            beta_col = cst.tile([P, 2], F32, tag="beta_col", name="beta_col")
            bd = cst.tile([1, 2], F32, tag="bd", name="bd")
            nc.vector.tensor_tensor(out=bd[:, 0:1], in0=gstats[:, 0:1],
                                    in1=gstats[:, 1:2], op=OP.subtract)
            btmp = cst.tile([1, 2], F32, tag="btmp", name="btmp")
            nc.scalar.activation(out=btmp[:, 0:1], in_=bd[:, 0:1], func=AF.Sigmoid,
                                 scale=1.0 / E_N)
            nc.scalar.activation(out=btmp[:, 1:2], in_=bd[:, 0:1], func=AF.Sigmoid,
                                 scale=-1.0 / E_N)
            b3_col = cst.tile([P, 1], F32, tag="b3_col", name="b3_col")
            with tc.tile_pool(name="bc2_ps", bufs=2, space="PSUM") as bc2:
                bb_ps = bc2.tile([P, 4], F32, space="PSUM", tag="bb_ps", name="bb_ps")
                nc.tensor.matmul(bb_ps[:, 0:2], lhsT=ones_row[:], rhs=btmp[:])
                nc.tensor.matmul(bb_ps[:, 2:3], lhsT=ones_row[:], rhs=b3[:])
                nc.vector.tensor_copy(beta_col[:], bb_ps[:, 0:2])
                nc.vector.tensor_copy(b3_col[:], bb_ps[:, 2:3])

            # fused exercise b-slot features: zsFT = b0*zsT_ex0 + b1*zsT_ex1
            zsFT = cst.tile([FD, BC], F32, tag="zsFT", name="zsFT")
            bcol = SH_TILES * P
            nc.vector.tensor_scalar(out=zsFT[:], in0=zsT["ex0"][:, bcol:bcol + BC],
                                    scalar1=beta_col[0:FD, 0:1], scalar2=None,
                                    op0=OP.mult)
            nc.vector.scalar_tensor_tensor(out=zsFT[:], in0=zsT["ex1"][:, bcol:bcol + BC],
                                           scalar=beta_col[0:FD, 1:2], in1=zsFT[:],
                                           op0=OP.mult, op1=OP.add)

            qt_sb = cst.tile([P, K], F32, tag="qt_sb", name="qt_sb")   # Q1T [k, j]
            st_sb = cst.tile([P, K], F32, tag="st_sb", name="st_sb")   # Q2T [k, j]
            m1_sb = cst.tile([FD, K], F32, tag="m1_sb", name="m1_sb")
            m2_sb = cst.tile([FD, K], F32, tag="m2_sb", name="m2_sb")
            c1t = cst.tile([P, 1], F32, tag="c1t", name="c1t")
            c2t = cst.tile([P, 1], F32, tag="c2t", name="c2t")
            kn1T = cst.tile([P, K], F32, tag="kn1T", name="kn1T")
            U1T = cst.tile([P, BC], F32, tag="U1T", name="U1T")
            U2T = cst.tile([P, BC], F32, tag="U2T", name="U2T")
            with tc.tile_pool(name="pF_ps", bufs=2, space="PSUM") as pfp:
                kn1_ps = pfp.tile([P, K], F32, space="PSUM", tag="prep_ps", name="kn1_ps")
                nc.tensor.matmul(kn1_ps[:], lhsT=zsT["kn"][:], rhs=pW_kn[:],
                                 start=True, stop=False)
                nc.tensor.matmul(kn1_ps[:], lhsT=ones_row[:], rhs=pb_kn_row[:],
                                 start=False, stop=True)
                kn1_sb = cst.tile([P, K], F32, tag="kn1_sb", name="kn1_sb")
                nc.scalar.copy(kn1_sb[:], kn1_ps[:])
                kn1T_ps = pfp.tile([P, K], F32, space="PSUM", tag="prep_ps", name="kn1T_ps")
                nc.tensor.transpose(out=kn1T_ps[:], in_=kn1_sb[:], identity=ident[:])
                nc.scalar.copy(kn1T[:], kn1T_ps[:])

                qs_ps = pfp.tile([P, K], F32, space="PSUM", tag="prep_ps", name="qs_ps")
                nc.tensor.matmul(qs_ps[:], lhsT=W1b[:], rhs=kn1T[:])
                nc.scalar.copy(qt_sb[:], qs_ps[:])
                qs2_ps = pfp.tile([P, K], F32, space="PSUM", tag="prep_ps", name="qs2_ps")
                nc.tensor.matmul(qs2_ps[:], lhsT=W2b[:], rhs=kn1T[:])
                nc.scalar.copy(st_sb[:], qs2_ps[:])

                m1_ps = pfp.tile([FD, K], F32, space="PSUM", tag="prep_ps", name="m1_ps")
                nc.tensor.matmul(m1_ps[:], lhsT=pWT_st[:], rhs=W1a[:])
                nc.scalar.copy(m1_sb[:], m1_ps[:])
                m2_ps = pfp.tile([FD, K], F32, space="PSUM", tag="prep_ps", name="m2_ps")
                nc.tensor.matmul(m2_ps[:], lhsT=pWT_ex[:], rhs=W2a[:])
                nc.scalar.copy(m2_sb[:], m2_ps[:])
                c1_ps = pfp.tile([P, 1], F32, space="PSUM", tag="prep_ps", name="c1_ps")
                nc.tensor.matmul(c1_ps[:], lhsT=W1a[:], rhs=pb_st[:])
                nc.vector.tensor_copy(c1t[:], c1_ps[:])
                c2_ps = pfp.tile([P, 1], F32, space="PSUM", tag="prep_ps", name="c2_ps")
                nc.tensor.matmul(c2_ps[:], lhsT=W2a[:], rhs=pb_ex[:])
                nc.vector.tensor_copy(c2t[:], c2_ps[:])

                u1_ps = pfp.tile([P, BC], F32, space="PSUM", tag="u_ps", name="u1_ps")
                nc.tensor.matmul(u1_ps[:], lhsT=m1_sb[:], rhs=zsT["st"][:, 0:BC])
                nc.vector.tensor_copy(U1T[:], u1_ps[:])
                u2_ps = pfp.tile([P, BC], F32, space="PSUM", tag="u_ps", name="u2_ps")
                nc.tensor.matmul(u2_ps[:], lhsT=m2_sb[:], rhs=zsFT[:])
                nc.vector.tensor_copy(U2T[:], u2_ps[:])

            # ---- predictor main loop ----
            # pref[b,j,k] = sig(U1[b,k] + Q1[j,k] + c1[k]); layout [k, (g,j)]
            with tc.tile_pool(name="pG", bufs=3) as pg, \
                 tc.tile_pool(name="pO_ps", bufs=1, space="PSUM") as pop:
                o_ps = pop.tile([P, BC], F32, space="PSUM", tag="o_ps", name="o_ps")
                for grp in range(BC // FG):
                    b0 = grp * FG
                    pr_lin = pg.tile([P, FG, K], F32, tag="pr_lin", name="pr_lin")
                    nc.vector.tensor_tensor(
                        out=pr_lin[:],
                        in0=qt_sb[:].unsqueeze(1).to_broadcast([P, FG, K]),
                        in1=U1T[:, b0:b0 + FG].unsqueeze(2).to_broadcast([P, FG, K]),
                        op=OP.add)
                    pr_sb = pg.tile([P, FG, K], F16, tag="pr_sb", name="pr_sb")
                    nc.scalar.activation(out=pr_sb[:], in_=pr_lin[:], func=AF.Sigmoid,
                                         bias=c1t[:])
                    df_lin = pg.tile([P, FG, K], F32, tag="df_lin", name="df_lin")
                    nc.vector.tensor_tensor(
                        out=df_lin[:],
                        in0=st_sb[:].unsqueeze(1).to_broadcast([P, FG, K]),
                        in1=U2T[:, b0:b0 + FG].unsqueeze(2).to_broadcast([P, FG, K]),
                        op=OP.add)
                    df_sb = pg.tile([P, FG, K], F16, tag="df_sb", name="df_sb")
                    nc.scalar.activation(out=df_sb[:], in_=df_lin[:], func=AF.Sigmoid,
                                         bias=c2t[:])
                    d_sb = pg.tile([P, FG, K], F16, tag="d_sb", name="d_sb")
                    nc.gpsimd.tensor_tensor(out=d_sb[:], in0=pr_sb[:], in1=df_sb[:],
                                            op=OP.subtract)
                    for lb in range(FG):
                        nc.tensor.matmul(o_ps[:, b0 + lb:b0 + lb + 1],
                                         lhsT=d_sb[:, lb, :], rhs=W3h[:])

                # ---- final ----
                with tc.tile_pool(name="pN_ps", bufs=1, space="PSUM") as pnp:
                    o_sb = pg.tile([P, BC], F32, tag="o_sb", name="o_sb")
                    nc.scalar.activation(out=o_sb[:], in_=o_ps[:], func=AF.Sigmoid,
                                         bias=b3_col[:])
                    om = pg.tile([P, BC], F32, tag="om", name="om")
                    nc.vector.tensor_tensor(out=om[:], in0=o_sb[:], in1=kn_rT[:], op=OP.mult)
                    nd_ps = pnp.tile([1, 2 * BC], F32, space="PSUM", tag="nd_ps", name="nd_ps")
                    nc.tensor.matmul(nd_ps[:, 0:BC], lhsT=ones_col[:], rhs=om[:])
                    nc.tensor.matmul(nd_ps[:, BC:2 * BC], lhsT=ones_col[:], rhs=kn_rT[:])
                    rcp = pg.tile([1, BC], F32, tag="rcp", name="rcp")
                    nc.vector.reciprocal(rcp[:], nd_ps[:, BC:2 * BC])
                    res = pg.tile([1, BC], F32, tag="res", name="res")
                    nc.vector.tensor_tensor(out=res[:], in0=nd_ps[:, 0:BC], in1=rcp[:],
                                            op=OP.mult)
                    nc.sync.dma_start(out_d[:], res[:])

    nc.compile()
    return nc


# ----------------------------------------------------------------------------
# Entry point
# ----------------------------------------------------------------------------

_TRACE = bool(int(os.environ.get("KERNEL_TRACE", "0")))


def kernel(**inputs):
    meta, in_maps = preprocess(inputs)
    nc = build_program(meta)
    res = bass_utils.run_bass_kernel_spmd(
        nc, in_maps, core_ids=list(range(NC)), trace=_TRACE)
    out = np.concatenate([r["out"].reshape(-1) for r in res.results])
    kernel.last_results = res
    return out.reshape(B, 1).astype(np.float32)


# revision 10
# speedup vs baseline: 5.5690x; 1.2524x over previous
"""Trainium2 Bass kernel for the HAN-based cognitive-diagnosis net.

Strategy (8 NeuronCores, SPMD — one program, per-core data):
  * Batch (2048) split 8x256 across cores. Each core computes GAT outputs
    for its own batch-slice node list plus a 1/8 share of all exercise
    nodes (for the global-mean semantic attention stats, AllReduce'd).
  * Edge phase without any device gather: the edge lists are known on the
    host, so the host pre-expands x^T into the ELL slot layout (one
    x-column per edge slot, zero column for pad slots).  The device
    computes per-edge [z(64) | el(8)] directly with PE matmuls
    (lhsT = xts slot tile, rhs = W folded with a_l), then runs the
    edge softmax + weighted aggregation on DVE/ACT with dst nodes on
    partitions.  Pad slots have z=0 (exact numerator); the softmax
    denominator is corrected analytically:
        s_real = s - npad[d] * exp(leaky(er[d]) - m[d])
    since every pad slot contributes exactly that one value.
  * Predictor exploits the rank structure pref[b,j,k] =
    sigmoid(U1[b,k] + Q1[j,k] + c1[k]): two small matmuls build U/Q once,
    the [B,K,K] tensors are pure broadcast-add + sigmoid, and the W3
    contraction runs on PE per batch row.
"""

import os
import numpy as np

import concourse.bass as bass
import concourse.bacc as bacc
import concourse.mybir as mybir
import concourse.tile as tile
from concourse import library_config
from concourse.masks import make_identity
from concourse import bass_utils

F32 = mybir.dt.float32
F16 = mybir.dt.float16

NC = 8
B = 2048
BC = B // NC          # 256 batch rows per core
K = 128
H, D, FD = 8, 8, 64
SEM = 128
S_N, E_N = 10000, 20000
P = 128

AX = mybir.AxisListType
OP = mybir.AluOpType
AF = mybir.ActivationFunctionType

GRAPHS = ("ex0", "ex1", "st", "kn")


# ----------------------------------------------------------------------------
# Host-side preprocessing (integer / layout only)
# ----------------------------------------------------------------------------

def _csr_by_dst(src, dst, n):
    order = np.argsort(dst, kind="stable")
    ss = src[order].astype(np.int64)
    counts = np.bincount(dst, minlength=n)
    rowptr = np.zeros(n + 1, np.int64)
    np.cumsum(counts, out=rowptr[1:])
    return ss, rowptr, counts


def _tiles_of(nodes):
    return [np.asarray(nodes[i:i + P]) for i in range(0, len(nodes), P)]


def _flat_src(node_tiles, dts, ss, rowptr, counts, npad_row):
    """Flat per-edge-slot src index list (tile-major, col-major, 128 lanes)
    with -1 for pad slots, plus the [128, ntiles] pad-count array."""
    nslot = int(np.sum(dts))
    flat = np.full((nslot, P), -1, np.int64)
    npad = np.zeros((P, len(node_tiles)), np.float32)
    col = 0
    for t, nodes in enumerate(node_tiles):
        dt = int(dts[t])
        for pi, node in enumerate(nodes):
            deg = int(counts[node])
            if deg:
                lo = rowptr[node]
                flat[col:col + deg, pi] = ss[lo:lo + deg]
            npad[pi, t] = -(dt - deg)
        for pi in range(len(nodes), P):
            npad[pi, t] = -dt
        col += dt
    assert col == nslot
    return flat.reshape(-1), npad


def _xts(x_aug_T, flat):
    """[K, nslot*128] fp16 ELL-expanded x^T (zero column for pad slots)."""
    return np.ascontiguousarray(x_aug_T[:, flat])


def _xtp(x, node_tiles, ntiles):
    """x^T columns for a node list, padded to ntiles*128 cols, fp16."""
    kdim = x.shape[1]
    out = np.zeros((kdim, ntiles * P), np.float16)
    for t, nodes in enumerate(node_tiles):
        out[:, t * P:t * P + len(nodes)] = x[nodes].T.astype(np.float16)
    return out


def preprocess(inputs):
    inp = {k: np.asarray(v) for k, v in inputs.items()}
    stu_id = inp["stu_id"].astype(np.int64)
    exer_id = inp["exer_id"].astype(np.int64)

    csr = {
        "st": _csr_by_dst(inp["ss0"].astype(np.int64), inp["sd0"].astype(np.int64), S_N),
        "ex0": _csr_by_dst(inp["es0"].astype(np.int64), inp["ed0"].astype(np.int64), E_N),
        "ex1": _csr_by_dst(inp["es1"].astype(np.int64), inp["ed1"].astype(np.int64), E_N),
        "kn": _csr_by_dst(inp["ks0"].astype(np.int64), inp["kd0"].astype(np.int64), K),
    }

    # per-core node tile lists; exercise share nodes degree-sorted + strided
    SH = E_N // NC                      # 2500
    SH_TILES = (SH + P - 1) // P        # 20
    BS_TILES = BC // P                  # 2
    NTP = {"ex0": SH_TILES + BS_TILES, "ex1": SH_TILES + BS_TILES,
           "st": BS_TILES, "kn": 1}

    tiles = {g: [] for g in GRAPHS}     # g -> [core][tile] node arrays
    for g in ("ex0", "ex1"):
        order = np.argsort(-csr[g][2], kind="stable")
        for c in range(NC):
            share = order[c::NC]
            assert len(share) == SH
            tiles[g].append(_tiles_of(share) +
                            _tiles_of(exer_id[c * BC:(c + 1) * BC]))
    for c in range(NC):
        tiles["st"].append(_tiles_of(stu_id[c * BC:(c + 1) * BC]))
        tiles["kn"].append(_tiles_of(np.arange(K)))

    # shared per-tile Dt = max over cores (SPMD: one program)
    dts = {}
    for g in GRAPHS:
        counts = csr[g][2]
        dts[g] = np.max(
            [[max(1, int(counts[t].max()) if len(t) else 1) for t in tiles[g][c]]
             for c in range(NC)], axis=0)

    meta = dict(dts=dts, NTP=NTP, SH=SH, SH_TILES=SH_TILES, BS_TILES=BS_TILES,
                nslot={g: int(dts[g].sum()) for g in GRAPHS})

    # ------- shared input arrays -------
    shared = {
        "w_ex0": inp["f3W0"].astype(np.float16),
        "w_ex1": inp["f3W1"].astype(np.float16),
        "w_st": inp["f1W0"].astype(np.float16),
        "w_kn": inp["f5W0"].astype(np.float16),
        "alr_ex0": np.concatenate([inp["f3al0"].reshape(1, 64), inp["f3ar0"].reshape(1, 64)], 1),
        "alr_ex1": np.concatenate([inp["f3al1"].reshape(1, 64), inp["f3ar1"].reshape(1, 64)], 1),
        "alr_st": np.concatenate([inp["f1al0"].reshape(1, 64), inp["f1ar0"].reshape(1, 64)], 1),
        "alr_kn": np.concatenate([inp["f5al0"].reshape(1, 64), inp["f5ar0"].reshape(1, 64)], 1),
        "semW": inp["f3sW"].astype(np.float32),
        "semb_col": inp["f3sb"].reshape(SEM, 1).astype(np.float32),
        "semq_col": inp["f3sq"].reshape(SEM, 1).astype(np.float32),
        "pWT_st": inp["f1pW"].T.astype(np.float32).copy(),
        "pb_st": inp["f1pb"].reshape(K, 1).astype(np.float32),
        "pWT_ex": inp["f3pW"].T.astype(np.float32).copy(),
        "pb_ex": inp["f3pb"].reshape(K, 1).astype(np.float32),
        "pW_kn": inp["f5pW"].astype(np.float32),
        "pb_kn_row": inp["f5pb"].reshape(1, K).astype(np.float32),
        "W1a": inp["W1"][:K].astype(np.float32),
        "W1b": inp["W1"][K:].astype(np.float32),
        "W2a": inp["W2"][:K].astype(np.float32),
        "W2b": inp["W2"][K:].astype(np.float32),
        "W3h": inp["W3"].astype(np.float16),
        "b3": inp["b3"].reshape(1, 1).astype(np.float32),
    }

    xsrc = {"ex0": inp["exer_t"], "ex1": inp["exer_t"],
            "st": inp["stu_t"], "kn": inp["kn_t"]}
    x_aug_T = {}
    for g in GRAPHS:
        xa = np.vstack([xsrc[g], np.zeros((1, K), np.float32)]).astype(np.float16)
        x_aug_T[g] = np.ascontiguousarray(xa.T)   # [K, N+1], col N = zeros

    # ------- per-core arrays -------
    in_maps = []
    for c in range(NC):
        m = dict(shared)
        for g in GRAPHS:
            ss, rowptr, counts = csr[g]
            flat, npad = _flat_src(tiles[g][c], dts[g], ss, rowptr, counts, None)
            flat = np.where(flat < 0, xsrc[g].shape[0], flat)
            m["xts_" + g] = _xts(x_aug_T[g], flat)
            m["npad_" + g] = npad
            m["xtp_" + g] = _xtp(xsrc[g], tiles[g][c], NTP[g])
        m["kn_rT"] = inp["kn_r"][c * BC:(c + 1) * BC].T.astype(np.float32).copy()
        in_maps.append(m)

    return meta, in_maps


# ----------------------------------------------------------------------------
# Bass program
# ----------------------------------------------------------------------------

PS_COLS = 24     # slot-cols per PSUM unit: z [P,24,64] (3 banks) + el [P,24,8]
FG = 8           # predictor batch rows per group


def build_program(meta):
    nc = bacc.Bacc("TRN2", num_devices=NC)
    dts = meta["dts"]
    NTP = meta["NTP"]
    SH = meta["SH"]
    SH_TILES, BS_TILES = meta["SH_TILES"], meta["BS_TILES"]
    nslot = meta["nslot"]
    MAXDT = -(-max(int(dts[g].max()) for g in GRAPHS) // 8) * 8

    ein = {}
    def EIN(name, shape, dt):
        ein[name] = nc.dram_tensor(name, list(shape), dt, kind="ExternalInput")
        return ein[name]

    for g in GRAPHS:
        EIN("w_" + g, (K, FD), F16)
        EIN("alr_" + g, (1, 128), F32)
        EIN("xts_" + g, (K, nslot[g] * P), F16)
        EIN("npad_" + g, (P, len(dts[g])), F32)
        EIN("xtp_" + g, (K, NTP[g] * P), F16)
    EIN("semW", (FD, SEM), F32); EIN("semb_col", (SEM, 1), F32); EIN("semq_col", (SEM, 1), F32)
    EIN("pWT_st", (K, FD), F32); EIN("pb_st", (K, 1), F32)
    EIN("pWT_ex", (K, FD), F32); EIN("pb_ex", (K, 1), F32)
    EIN("pW_kn", (FD, K), F32); EIN("pb_kn_row", (1, K), F32)
    EIN("W1a", (K, K), F32); EIN("W1b", (K, K), F32)
    EIN("W2a", (K, K), F32); EIN("W2b", (K, K), F32)
    EIN("W3h", (K, 1), F16); EIN("b3", (1, 1), F32)
    EIN("kn_rT", (K, BC), F32)

    out_d = nc.dram_tensor("out", [1, BC], F32, kind="ExternalOutput")

    cc_in = nc.dram_tensor("cc_in", [1, 16], F32, kind="Internal")
    cc_out = nc.dram_tensor("cc_out", [1, 16], F32, kind="Internal", addr_space="Shared")

    with tile.TileContext(nc) as tc:
        with tc.tile_pool(name="const", bufs=1) as cst, \
             tc.tile_pool(name="slab", bufs=1) as slab:
            nc.gpsimd.load_library(library_config.mlp)

            ident = cst.tile([P, P], F32, tag="ident", name="ident")
            make_identity(nc, ident[:])
            ones_col = cst.tile([P, 1], F32, tag="ones_col", name="ones_col")
            nc.vector.memset(ones_col[:], 1.0)
            ones_row = cst.tile([1, P], F32, tag="ones_row", name="ones_row")
            nc.vector.memset(ones_row[:], 1.0)

            # ---- load small weights ----
            def load(name, shape, dt):
                t = cst.tile(list(shape), dt, tag="ld_" + name, name="ld_" + name)
                nc.sync.dma_start(t[:], ein[name][:])
                return t
            w_g = {g: load("w_" + g, (K, FD), F16) for g in GRAPHS}
            alr = {g: load("alr_" + g, (1, 128), F32) for g in GRAPHS}
            npad_sb = {g: load("npad_" + g, (P, len(dts[g])), F32) for g in GRAPHS}
            semW = load("semW", (FD, SEM), F32)
            semb_col = load("semb_col", (SEM, 1), F32)
            semq_col = load("semq_col", (SEM, 1), F32)
            pWT_st = load("pWT_st", (K, FD), F32); pb_st = load("pb_st", (K, 1), F32)
            pWT_ex = load("pWT_ex", (K, FD), F32); pb_ex = load("pb_ex", (K, 1), F32)
            pW_kn = load("pW_kn", (FD, K), F32); pb_kn_row = load("pb_kn_row", (1, K), F32)
            W1a = load("W1a", (K, K), F32); W1b = load("W1b", (K, K), F32)
            W2a = load("W2a", (K, K), F32); W2b = load("W2b", (K, K), F32)
            W3h = load("W3h", (K, 1), F16); b3 = load("b3", (1, 1), F32)
            kn_rT = load("kn_rT", (K, BC), F32)

            # ---- fold al/ar into W: wcat = [W | W@al] fp16, war = W@ar ----
            wcat = {}   # [128, 72] f16: cols 0:64 W, 64:72 Wal
            war = {}    # [128, 8] f16
            with tc.tile_pool(name="bc_ps", bufs=2, space="PSUM") as bcp:
              for g in GRAPHS:
                alb = cst.tile([P, 128], F32, tag="alb", name="alb")
                alb_ps = bcp.tile([P, 128], F32, space="PSUM", tag="alb_ps", name="alb_ps")
                nc.tensor.matmul(alb_ps[:], lhsT=ones_row[:], rhs=alr[g][:])
                nc.vector.tensor_copy(alb[:], alb_ps[:])
                wf = cst.tile([P, FD], F32, tag="wf", name="wf")
                nc.vector.tensor_copy(wf[:], w_g[g][:])
                wtmp = cst.tile([P, FD], F32, tag="wtmp", name="wtmp")
                wc = cst.tile([P, 72], F16, tag="wcat_" + g, name="wcat_" + g)
                wcat[g] = wc
                nc.vector.tensor_copy(wc[:, 0:64], w_g[g][:])
                with nc.allow_low_precision(reason="8-elem head fold of fp16 weights"):
                    nc.vector.tensor_tensor(out=wtmp[:], in0=wf[:], in1=alb[:, 0:64], op=OP.mult)
                    nc.vector.tensor_reduce(out=wc[:, 64:72].bitcast(F16),
                                            in_=wtmp[:].rearrange("p (h f) -> p h f", h=H),
                                            axis=AX.X, op=OP.add)
                    wr = cst.tile([P, 8], F16, tag="war_" + g, name="war_" + g)
                    war[g] = wr
                    nc.vector.tensor_tensor(out=wtmp[:], in0=wf[:], in1=alb[:, 64:128], op=OP.mult)
                    nc.vector.tensor_reduce(out=wr[:], in_=wtmp[:].rearrange("p (h f) -> p h f", h=H),
                                            axis=AX.X, op=OP.add)

            # ---- er per (graph, tile): er[d, t, h] = (x[d] @ war)[h] ----
            er = {}
            with tc.tile_pool(name="pE", bufs=2) as pe, \
                 tc.tile_pool(name="pE_ps", bufs=4, space="PSUM") as pep:
                for g in GRAPHS:
                    ntp = NTP[g]
                    er_sb = slab.tile([P, ntp, 8], F32, tag="er_" + g, name="er_" + g)
                    er[g] = er_sb
                    xtp_sb = pe.tile([P, NTP["ex0"] * P], F16, tag="xtp_sb", name="xtp_sb")
                    nc.sync.dma_start(xtp_sb[:, 0:ntp * P], ein["xtp_" + g][:])
                    for t in range(ntp):
                        eps = pep.tile([P, 8], F32, space="PSUM", tag="eps", name="eps")
                        nc.tensor.matmul(eps[:], lhsT=xtp_sb[:, t * P:(t + 1) * P],
                                         rhs=war[g][:])
                        nc.vector.tensor_copy(er_sb[:, t, :], eps[:])

            # ---- Phase B: per-edge z via PE + edge softmax + aggregation ----
            zs = {g: slab.tile([P, NTP[g], FD], F32, tag="zs_" + g, name="zs_" + g)
                  for g in GRAPHS}

            def do_graph(g, pb, pbs, pzp):
                """Emit z matmuls + softmax for every tile of graph g.

                Small tiles (Dt <= PS_COLS): z and el stay resident in PSUM;
                the e-pipe reads el from PSUM and the weighted mult reads z
                from PSUM (no evacuation copies).  Big tiles copy per unit.
                """
                col0 = 0
                for t in range(NTP[g]):
                    Dt = int(dts[g][t])
                    DtP = -(-Dt // 8) * 8
                    small = Dt <= PS_COLS
                    z_el = pbs.tile([P, MAXDT, 64], F16, tag="z_el", name="z_el")
                    if not small:
                        el_t = pbs.tile([P, 8, MAXDT], F32, tag="el_t", name="el_t")
                    zp_keep = None
                    elp_keep = None
                    for lo in range(0, Dt, PS_COLS):
                        n_here = min(PS_COLS, Dt - lo)
                        xts_sb = pb.tile([P, PS_COLS * P], F16, tag="xts_sb", name="xts_sb")
                        nc.sync.dma_start(
                            xts_sb[:, 0:n_here * P],
                            ein["xts_" + g][:, (col0 + lo) * P:(col0 + lo + n_here) * P])
                        zp = pzp.tile([P, PS_COLS, 64], F32, space="PSUM",
                                      tag="zp", name="zp")
                        elp = pzp.tile([P, PS_COLS, 8], F32, space="PSUM",
                                       tag="elp", name="elp")
                        for ci in range(n_here):
                            lhsT = xts_sb[:, ci * P:(ci + 1) * P]
                            nc.tensor.matmul(zp[:, ci, :], lhsT=lhsT, rhs=wcat[g][:, 0:64])
                            nc.tensor.matmul(elp[:, ci, :], lhsT=lhsT, rhs=wcat[g][:, 64:72])
                        if small:
                            zp_keep, elp_keep = zp, elp
                        else:
                            nc.scalar.activation(out=z_el[:, lo:lo + n_here, :],
                                                 in_=zp[:, 0:n_here, :], func=AF.Copy)
                            nc.scalar.activation(
                                out=el_t[:].transpose([0, 2, 1])[:, lo:lo + n_here, :],
                                in_=elp[:, 0:n_here, :], func=AF.Copy)
                    # ---- edge softmax over the Dt slots (head-major e) ----
                    e = pbs.tile([P, 8, MAXDT], F32, tag="e_buf", name="e_buf")
                    if small:
                        nc.vector.tensor_tensor(
                            out=e[:].transpose([0, 2, 1])[:, 0:Dt, :],
                            in0=elp_keep[:, 0:Dt, :],
                            in1=er[g][:, t, :].unsqueeze(1).to_broadcast([P, Dt, 8]),
                            op=OP.add)
                    else:
                        nc.vector.tensor_tensor(
                            out=e[:, :, 0:Dt], in0=el_t[:, :, 0:Dt],
                            in1=er[g][:, t, :].unsqueeze(2).to_broadcast([P, 8, Dt]),
                            op=OP.add)
                    e2 = pbs.tile([P, 8, MAXDT], F32, tag="e2_buf", name="e2_buf")
                    nc.vector.scalar_tensor_tensor(out=e2[:, :, 0:Dt], in0=e[:, :, 0:Dt],
                                                   scalar=0.2, in1=e[:, :, 0:Dt],
                                                   op0=OP.mult, op1=OP.max)
                    m = pbs.tile([P, 8], F32, tag="m_buf", name="m_buf")
                    nc.vector.tensor_reduce(out=m[:], in_=e2[:, :, 0:Dt],
                                            axis=AX.X, op=OP.max)
                    nc.vector.tensor_tensor(
                        out=e2[:, :, 0:Dt], in0=e2[:, :, 0:Dt],
                        in1=m[:].unsqueeze(2).to_broadcast([P, 8, Dt]), op=OP.subtract)
                    exb = pbs.tile([P, 8, MAXDT], F16, tag="exb_buf", name="exb_buf")
                    nc.scalar.activation(out=exb[:, :, 0:Dt], in_=e2[:, :, 0:Dt], func=AF.Exp)
                    s = pbs.tile([P, 8], F32, tag="s_buf", name="s_buf")
                    nc.vector.tensor_reduce(out=s[:], in_=exb[:, :, 0:Dt],
                                            axis=AX.X, op=OP.add)
                    # pad correction: s += npadneg * exp(leaky(er) - m)
                    ep = pbs.tile([P, 8], F32, tag="ep_buf", name="ep_buf")
                    nc.vector.scalar_tensor_tensor(out=ep[:], in0=er[g][:, t, :],
                                                   scalar=0.2, in1=er[g][:, t, :],
                                                   op0=OP.mult, op1=OP.max)
                    nc.vector.tensor_tensor(out=ep[:], in0=ep[:], in1=m[:], op=OP.subtract)
                    pex = pbs.tile([P, 8], F16, tag="pex_buf", name="pex_buf")
                    nc.scalar.activation(out=pex[:], in_=ep[:], func=AF.Exp)
                    pex32 = pbs.tile([P, 8], F32, tag="pex32_buf", name="pex32_buf")
                    nc.vector.tensor_copy(pex32[:], pex[:])
                    nc.vector.scalar_tensor_tensor(out=s[:], in0=pex32[:],
                                                   scalar=npad_sb[g][:, t:t + 1],
                                                   in1=s[:], op0=OP.mult, op1=OP.add)
                    nc.vector.tensor_scalar_add(s[:], s[:], 1e-9)
                    rs = pbs.tile([P, 8], F32, tag="rs_buf", name="rs_buf")
                    nc.vector.reciprocal(rs[:], s[:])
                    # weighted w = z * exb into SBUF (z from PSUM for small tiles)
                    z_src = (zp_keep[:, 0:Dt, :] if small else z_el[:, 0:Dt, :])
                    nc.vector.tensor_tensor(
                        out=z_el[:, 0:Dt, :].rearrange("p s (h f) -> p s h f", h=H),
                        in0=z_src.rearrange("p s (h f) -> p s h f", h=H),
                        in1=exb[:, :, 0:Dt].transpose([0, 2, 1]).unsqueeze(3)
                        .to_broadcast([P, Dt, 8, 8]),
                        op=OP.mult)
                    if DtP > Dt:
                        nc.vector.memset(z_el[:, Dt:DtP, :], 0.0)
                    # 3 contiguous halvings then short strided tail reduce
                    cur = DtP
                    while cur > DtP // 8:
                        h2 = cur // 2
                        nc.vector.tensor_tensor(out=z_el[:, 0:h2, :],
                                                in0=z_el[:, 0:h2, :],
                                                in1=z_el[:, h2:cur, :], op=OP.add)
                        cur = h2
                    v = zs[g][:, t, :]
                    nc.vector.tensor_reduce(
                        out=v, in_=z_el[:, 0:cur, :].transpose([0, 2, 1]),
                        axis=AX.X, op=OP.add)
                    nc.vector.tensor_tensor(
                        out=v.rearrange("p (h f) -> p h f", h=H),
                        in0=v.rearrange("p (h f) -> p h f", h=H),
                        in1=rs[:].unsqueeze(2).to_broadcast([P, H, D]), op=OP.mult)
                    # elu
                    t1 = pbs.tile([P, FD], F32, tag="elu1", name="elu1")
                    nc.vector.tensor_scalar_min(t1[:], v, 0.0)
                    t2 = pbs.tile([P, FD], F32, tag="elu2", name="elu2")
                    nc.scalar.activation(out=t2[:], in_=t1[:], func=AF.Exp)
                    nc.vector.tensor_tensor(out=v, in0=v, in1=t1[:], op=OP.subtract)
                    nc.vector.scalar_tensor_tensor(out=v, in0=t2[:], scalar=-1.0,
                                                   in1=v, op0=OP.add, op1=OP.add)
                    col0 += Dt

            zsT = {g: slab.tile([FD, NTP[g] * P], F32, tag="zsT_" + g, name="zsT_" + g)
                   for g in GRAPHS}

            def do_transposes(g, pcp):
                for t in range(NTP[g]):
                    tp = pcp.tile([FD, P], F32, space="PSUM", tag="tp_ps", name="tp_ps")
                    nc.tensor.transpose(out=tp[:], in_=zs[g][:, t, :], identity=ident[:])
                    if t % 2 == 0:
                        nc.scalar.copy(zsT[g][:, t * P:(t + 1) * P], tp[:])
                    else:
                        nc.vector.tensor_copy(zsT[g][:, t * P:(t + 1) * P], tp[:])

            with tc.tile_pool(name="pB", bufs=3) as pb, \
                 tc.tile_pool(name="pBs", bufs=2) as pbs, \
                 tc.tile_pool(name="pB_ps", bufs=2, space="PSUM") as pzp:
                do_graph("ex0", pb, pbs, pzp)
                do_graph("ex1", pb, pbs, pzp)

            with tc.tile_pool(name="pC_ps", bufs=4, space="PSUM") as pcp:
                do_transposes("ex0", pcp)
                do_transposes("ex1", pcp)

            # ---- semantic attention stats over the exercise share ----
            stats = cst.tile([1, 16], F32, tag="stats", name="stats")
            nc.vector.memset(stats[:], 0.0)
            with tc.tile_pool(name="pD", bufs=2) as pd, \
                 tc.tile_pool(name="pD_ps", bufs=4, space="PSUM") as pdp:
                parts = cst.tile([1, 16], F32, tag="parts", name="parts")
                for mi, g in enumerate(("ex0", "ex1")):
                    cw_list = []
                    lo = 0
                    while lo < SH:
                        cw = min(512, SH - lo)
                        cw_list.append((lo, cw))
                        lo += cw
                    for ci, (lo, cw) in enumerate(cw_list):
                        tps = pdp.tile([SEM, 512], F32, space="PSUM", tag="tps", name="tps")
                        nc.tensor.matmul(tps[:, 0:cw], lhsT=semW[:], rhs=zsT[g][:, lo:lo + cw])
                        tsb = pd.tile([SEM, 512], F32, tag="tsb", name="tsb")
                        nc.scalar.activation(out=tsb[:, 0:cw], in_=tps[:, 0:cw],
                                             func=AF.Tanh, bias=semb_col[:])
                        rps = pdp.tile([1, 512], F32, space="PSUM", tag="rps", name="rps")
                        nc.tensor.matmul(rps[:, 0:cw], lhsT=semq_col[:], rhs=tsb[:, 0:cw])
                        nc.vector.tensor_reduce(out=parts[:, mi * 8 + ci:mi * 8 + ci + 1],
                                                in_=rps[:, 0:cw], axis=AX.X, op=OP.add)
                    nc.vector.tensor_reduce(
                        out=stats[:, mi:mi + 1],
                        in_=parts[:, mi * 8:mi * 8 + len(cw_list)], axis=AX.X, op=OP.add)

            # ---- AllReduce the 2 stats scalars (overlapped with st/kn) ----
            nc.sync.dma_start(cc_in[:, 0:16], stats[:])
            nc.gpsimd.collective_compute(
                "AllReduce", OP.add,
                replica_groups=[list(range(NC))],
                ins=[cc_in[:, :]], outs=[cc_out[:, :]])

            with tc.tile_pool(name="pB2", bufs=3) as pb, \
                 tc.tile_pool(name="pB2s", bufs=2) as pbs, \
                 tc.tile_pool(name="pB2_ps", bufs=2, space="PSUM") as pzp:
                do_graph("st", pb, pbs, pzp)
                do_graph("kn", pb, pbs, pzp)

            with tc.tile_pool(name="pC2_ps", bufs=4, space="PSUM") as pcp:
                do_transposes("st", pcp)
                do_transposes("kn", pcp)

            gstats = cst.tile([1, 16], F32, tag="gstats", name="gstats")
            nc.sync.dma_start(gstats[:], cc_out[:, :])

            # ---- predictor prep ----
            beta_col = cst.tile([P, 2], F32, tag="beta_col", name="beta_col")
            bd = cst.tile([1, 2], F32, tag="bd", name="bd")
            nc.vector.tensor_tensor(out=bd[:, 0:1], in0=gstats[:, 0:1],
                                    in1=gstats[:, 1:2], op=OP.subtract)
            btmp = cst.tile([1, 2], F32, tag="btmp", name="btmp")
            nc.scalar.activation(out=btmp[:, 0:1], in_=bd[:, 0:1], func=AF.Sigmoid,
                                 scale=1.0 / E_N)
            nc.scalar.activation(out=btmp[:, 1:2], in_=bd[:, 0:1], func=AF.Sigmoid,
                                 scale=-1.0 / E_N)
            b3_col = cst.tile([P, 1], F32, tag="b3_col", name="b3_col")
            with tc.tile_pool(name="bc2_ps", bufs=2, space="PSUM") as bc2:
                bb_ps = bc2.tile([P, 4], F32, space="PSUM", tag="bb_ps", name="bb_ps")
                nc.tensor.matmul(bb_ps[:, 0:2], lhsT=ones_row[:], rhs=btmp[:])
                nc.tensor.matmul(bb_ps[:, 2:3], lhsT=ones_row[:], rhs=b3[:])
                nc.vector.tensor_copy(beta_col[:], bb_ps[:, 0:2])
                nc.vector.tensor_copy(b3_col[:], bb_ps[:, 2:3])

            # fused exercise b-slot features: zsFT = b0*zsT_ex0 + b1*zsT_ex1
            zsFT = cst.tile([FD, BC], F32, tag="zsFT", name="zsFT")
            bcol = SH_TILES * P
            nc.vector.tensor_scalar(out=zsFT[:], in0=zsT["ex0"][:, bcol:bcol + BC],
                                    scalar1=beta_col[0:FD, 0:1], scalar2=None,
                                    op0=OP.mult)
            nc.vector.scalar_tensor_tensor(out=zsFT[:], in0=zsT["ex1"][:, bcol:bcol + BC],
                                           scalar=beta_col[0:FD, 1:2], in1=zsFT[:],
                                           op0=OP.mult, op1=OP.add)

            qt_sb = cst.tile([P, K], F16, tag="qt_sb", name="qt_sb")   # Q1T [k, j]
            st_sb = cst.tile([P, K], F16, tag="st_sb", name="st_sb")   # Q2T [k, j]
            m1_sb = cst.tile([FD, K], F32, tag="m1_sb", name="m1_sb")
            m2_sb = cst.tile([FD, K], F32, tag="m2_sb", name="m2_sb")
            c1t = cst.tile([P, 1], F32, tag="c1t", name="c1t")
            c2t = cst.tile([P, 1], F32, tag="c2t", name="c2t")
            kn1T = cst.tile([P, K], F32, tag="kn1T", name="kn1T")
            U1T = cst.tile([P, BC], F16, tag="U1T", name="U1T")
            U2T = cst.tile([P, BC], F16, tag="U2T", name="U2T")
            with tc.tile_pool(name="pF_ps", bufs=2, space="PSUM") as pfp:
                kn1_ps = pfp.tile([P, K], F32, space="PSUM", tag="prep_ps", name="kn1_ps")
                nc.tensor.matmul(kn1_ps[:], lhsT=zsT["kn"][:], rhs=pW_kn[:],
                                 start=True, stop=False)
                nc.tensor.matmul(kn1_ps[:], lhsT=ones_row[:], rhs=pb_kn_row[:],
                                 start=False, stop=True)
                kn1_sb = cst.tile([P, K], F32, tag="kn1_sb", name="kn1_sb")
                nc.scalar.copy(kn1_sb[:], kn1_ps[:])
                kn1T_ps = pfp.tile([P, K], F32, space="PSUM", tag="prep_ps", name="kn1T_ps")
                nc.tensor.transpose(out=kn1T_ps[:], in_=kn1_sb[:], identity=ident[:])
                nc.scalar.copy(kn1T[:], kn1T_ps[:])

                qs_ps = pfp.tile([P, K], F32, space="PSUM", tag="prep_ps", name="qs_ps")
                nc.tensor.matmul(qs_ps[:], lhsT=W1b[:], rhs=kn1T[:])
                nc.scalar.copy(qt_sb[:], qs_ps[:])
                qs2_ps = pfp.tile([P, K], F32, space="PSUM", tag="prep_ps", name="qs2_ps")
                nc.tensor.matmul(qs2_ps[:], lhsT=W2b[:], rhs=kn1T[:])
                nc.scalar.copy(st_sb[:], qs2_ps[:])

                m1_ps = pfp.tile([FD, K], F32, space="PSUM", tag="prep_ps", name="m1_ps")
                nc.tensor.matmul(m1_ps[:], lhsT=pWT_st[:], rhs=W1a[:])
                nc.scalar.copy(m1_sb[:], m1_ps[:])
                m2_ps = pfp.tile([FD, K], F32, space="PSUM", tag="prep_ps", name="m2_ps")
                nc.tensor.matmul(m2_ps[:], lhsT=pWT_ex[:], rhs=W2a[:])
                nc.scalar.copy(m2_sb[:], m2_ps[:])
                c1_ps = pfp.tile([P, 1], F32, space="PSUM", tag="prep_ps", name="c1_ps")
                nc.tensor.matmul(c1_ps[:], lhsT=W1a[:], rhs=pb_st[:])
                nc.vector.tensor_copy(c1t[:], c1_ps[:])
                c2_ps = pfp.tile([P, 1], F32, space="PSUM", tag="prep_ps", name="c2_ps")
                nc.tensor.matmul(c2_ps[:], lhsT=W2a[:], rhs=pb_ex[:])
                nc.vector.tensor_copy(c2t[:], c2_ps[:])

                u1_ps = pfp.tile([P, BC], F32, space="PSUM", tag="u_ps", name="u1_ps")
                nc.tensor.matmul(u1_ps[:], lhsT=m1_sb[:], rhs=zsT["st"][:, 0:BC])
                nc.vector.tensor_copy(U1T[:], u1_ps[:])
                u2_ps = pfp.tile([P, BC], F32, space="PSUM", tag="u_ps", name="u2_ps")
                nc.tensor.matmul(u2_ps[:], lhsT=m2_sb[:], rhs=zsFT[:])
                nc.vector.tensor_copy(U2T[:], u2_ps[:])

            # ---- predictor main loop ----
            # pref[b,j,k] = sig(U1[b,k] + Q1[j,k] + c1[k]); layout [k, (g,j)]
            with tc.tile_pool(name="pG", bufs=3) as pg, \
                 tc.tile_pool(name="pO_ps", bufs=1, space="PSUM") as pop:
                o_ps = pop.tile([P, BC], F32, space="PSUM", tag="o_ps", name="o_ps")
                for grp in range(BC // FG):
                    b0 = grp * FG
                    pr_lin = pg.tile([P, FG, K], F16, tag="pr_lin", name="pr_lin")
                    nc.vector.tensor_tensor(
                        out=pr_lin[:],
                        in0=qt_sb[:].unsqueeze(1).to_broadcast([P, FG, K]),
                        in1=U1T[:, b0:b0 + FG].unsqueeze(2).to_broadcast([P, FG, K]),
                        op=OP.add)
                    pr_sb = pg.tile([P, FG, K], F16, tag="pr_sb", name="pr_sb")
                    nc.scalar.activation(out=pr_sb[:], in_=pr_lin[:], func=AF.Sigmoid,
                                         bias=c1t[:])
                    df_lin = pg.tile([P, FG, K], F16, tag="df_lin", name="df_lin")
                    nc.vector.tensor_tensor(
                        out=df_lin[:],
                        in0=st_sb[:].unsqueeze(1).to_broadcast([P, FG, K]),
                        in1=U2T[:, b0:b0 + FG].unsqueeze(2).to_broadcast([P, FG, K]),
                        op=OP.add)
                    df_sb = pg.tile([P, FG, K], F16, tag="df_sb", name="df_sb")
                    nc.scalar.activation(out=df_sb[:], in_=df_lin[:], func=AF.Sigmoid,
                                         bias=c2t[:])
                    d_sb = pg.tile([P, FG, K], F16, tag="d_sb", name="d_sb")
                    nc.vector.tensor_tensor(out=d_sb[:], in0=pr_sb[:], in1=df_sb[:],
                                            op=OP.subtract)
                    for lb in range(FG):
                        nc.tensor.matmul(o_ps[:, b0 + lb:b0 + lb + 1],
                                         lhsT=d_sb[:, lb, :], rhs=W3h[:])

                # ---- final ----
                with tc.tile_pool(name="pN_ps", bufs=1, space="PSUM") as pnp:
                    o_sb = pg.tile([P, BC], F32, tag="o_sb", name="o_sb")
                    nc.scalar.activation(out=o_sb[:], in_=o_ps[:], func=AF.Sigmoid,
                                         bias=b3_col[:])
                    om = pg.tile([P, BC], F32, tag="om", name="om")
                    nc.vector.tensor_tensor(out=om[:], in0=o_sb[:], in1=kn_rT[:], op=OP.mult)
                    nd_ps = pnp.tile([1, 2 * BC], F32, space="PSUM", tag="nd_ps", name="nd_ps")
                    nc.tensor.matmul(nd_ps[:, 0:BC], lhsT=ones_col[:], rhs=om[:])
                    nc.tensor.matmul(nd_ps[:, BC:2 * BC], lhsT=ones_col[:], rhs=kn_rT[:])
                    rcp = pg.tile([1, BC], F32, tag="rcp", name="rcp")
                    nc.vector.reciprocal(rcp[:], nd_ps[:, BC:2 * BC])
                    res = pg.tile([1, BC], F32, tag="res", name="res")
                    nc.vector.tensor_tensor(out=res[:], in0=nd_ps[:, 0:BC], in1=rcp[:],
                                            op=OP.mult)
                    nc.sync.dma_start(out_d[:], res[:])

    nc.compile()
    return nc


# ----------------------------------------------------------------------------
# Entry point
# ----------------------------------------------------------------------------

_TRACE = bool(int(os.environ.get("KERNEL_TRACE", "0")))


def kernel(**inputs):
    meta, in_maps = preprocess(inputs)
    nc = build_program(meta)
    res = bass_utils.run_bass_kernel_spmd(
        nc, in_maps, core_ids=list(range(NC)), trace=_TRACE)
    out = np.concatenate([r["out"].reshape(-1) for r in res.results])
    kernel.last_results = res
    return out.reshape(B, 1).astype(np.float32)


# revision 14
# speedup vs baseline: 5.5821x; 1.0024x over previous
"""Trainium2 Bass kernel for the HAN-based cognitive-diagnosis net.

Strategy (8 NeuronCores, SPMD — one program, per-core data):
  * Batch (2048) split 8x256 across cores. Each core computes GAT outputs
    for its own batch-slice node list plus a 1/8 share of all exercise
    nodes (for the global-mean semantic attention stats, AllReduce'd).
  * Edge phase without any device gather: the edge lists are known on the
    host, so the host pre-expands x^T into the ELL slot layout (one
    x-column per edge slot, zero column for pad slots).  The device
    computes per-edge [z(64) | el(8)] directly with PE matmuls
    (lhsT = xts slot tile, rhs = W folded with a_l), then runs the
    edge softmax + weighted aggregation on DVE/ACT with dst nodes on
    partitions.  Pad slots have z=0 (exact numerator); the softmax
    denominator is corrected analytically:
        s_real = s - npad[d] * exp(leaky(er[d]) - m[d])
    since every pad slot contributes exactly that one value.
  * Predictor exploits the rank structure pref[b,j,k] =
    sigmoid(U1[b,k] + Q1[j,k] + c1[k]): two small matmuls build U/Q once,
    the [B,K,K] tensors are pure broadcast-add + sigmoid, and the W3
    contraction runs on PE per batch row.
"""

import os
import numpy as np

import concourse.bass as bass
import concourse.bacc as bacc
import concourse.mybir as mybir
import concourse.tile as tile
from concourse import library_config
from concourse.masks import make_identity
from concourse import bass_utils

F32 = mybir.dt.float32
F16 = mybir.dt.float16

NC = 8
B = 2048
BC = B // NC          # 256 batch rows per core
K = 128
H, D, FD = 8, 8, 64
SEM = 128
S_N, E_N = 10000, 20000
P = 128

AX = mybir.AxisListType
OP = mybir.AluOpType
AF = mybir.ActivationFunctionType

GRAPHS = ("ex0", "ex1", "st", "kn")


# ----------------------------------------------------------------------------
# Host-side preprocessing (integer / layout only)
# ----------------------------------------------------------------------------

def _csr_by_dst(src, dst, n):
    order = np.argsort(dst, kind="stable")
    ss = src[order].astype(np.int64)
    counts = np.bincount(dst, minlength=n)
    rowptr = np.zeros(n + 1, np.int64)
    np.cumsum(counts, out=rowptr[1:])
    return ss, rowptr, counts


def _tiles_of(nodes):
    return [np.asarray(nodes[i:i + P]) for i in range(0, len(nodes), P)]


def _flat_src(node_tiles, dts, ss, rowptr, counts, npad_row):
    """Flat per-edge-slot src index list (tile-major, col-major, 128 lanes)
    with -1 for pad slots, plus the [128, ntiles] pad-count array."""
    nslot = int(np.sum(dts))
    flat = np.full((nslot, P), -1, np.int64)
    npad = np.zeros((P, len(node_tiles)), np.float32)
    col = 0
    for t, nodes in enumerate(node_tiles):
        dt = int(dts[t])
        for pi, node in enumerate(nodes):
            deg = int(counts[node])
            if deg:
                lo = rowptr[node]
                flat[col:col + deg, pi] = ss[lo:lo + deg]
            npad[pi, t] = -(dt - deg)
        for pi in range(len(nodes), P):
            npad[pi, t] = -dt
        col += dt
    assert col == nslot
    return flat.reshape(-1), npad


def _xts(x_aug_T, flat):
    """[K, nslot*128] fp16 ELL-expanded x^T (zero column for pad slots)."""
    return np.ascontiguousarray(x_aug_T[:, flat])


def _xtp(x, node_tiles, ntiles):
    """x^T columns for a node list, padded to ntiles*128 cols, fp16."""
    kdim = x.shape[1]
    out = np.zeros((kdim, ntiles * P), np.float16)
    for t, nodes in enumerate(node_tiles):
        out[:, t * P:t * P + len(nodes)] = x[nodes].T.astype(np.float16)
    return out


def preprocess(inputs):
    inp = {k: np.asarray(v) for k, v in inputs.items()}
    stu_id = inp["stu_id"].astype(np.int64)
    exer_id = inp["exer_id"].astype(np.int64)

    csr = {
        "st": _csr_by_dst(inp["ss0"].astype(np.int64), inp["sd0"].astype(np.int64), S_N),
        "ex0": _csr_by_dst(inp["es0"].astype(np.int64), inp["ed0"].astype(np.int64), E_N),
        "ex1": _csr_by_dst(inp["es1"].astype(np.int64), inp["ed1"].astype(np.int64), E_N),
        "kn": _csr_by_dst(inp["ks0"].astype(np.int64), inp["kd0"].astype(np.int64), K),
    }

    # per-core node tile lists; exercise share nodes degree-sorted + strided
    SH = E_N // NC                      # 2500
    SH_TILES = (SH + P - 1) // P        # 20
    BS_TILES = BC // P                  # 2
    NTP = {"ex0": SH_TILES + BS_TILES, "ex1": SH_TILES + BS_TILES,
           "st": BS_TILES, "kn": 1}

    tiles = {g: [] for g in GRAPHS}     # g -> [core][tile] node arrays
    for g in ("ex0", "ex1"):
        order = np.argsort(-csr[g][2], kind="stable")
        for c in range(NC):
            share = order[c::NC]
            assert len(share) == SH
            tiles[g].append(_tiles_of(share) +
                            _tiles_of(exer_id[c * BC:(c + 1) * BC]))
    for c in range(NC):
        tiles["st"].append(_tiles_of(stu_id[c * BC:(c + 1) * BC]))
        tiles["kn"].append(_tiles_of(np.arange(K)))

    # shared per-tile Dt = max over cores (SPMD: one program)
    dts = {}
    for g in GRAPHS:
        counts = csr[g][2]
        dts[g] = np.max(
            [[max(1, int(counts[t].max()) if len(t) else 1) for t in tiles[g][c]]
             for c in range(NC)], axis=0)

    meta = dict(dts=dts, NTP=NTP, SH=SH, SH_TILES=SH_TILES, BS_TILES=BS_TILES,
                nslot={g: int(dts[g].sum()) for g in GRAPHS})

    # ------- shared input arrays -------
    shared = {
        "w_ex0": inp["f3W0"].astype(np.float16),
        "w_ex1": inp["f3W1"].astype(np.float16),
        "w_st": inp["f1W0"].astype(np.float16),
        "w_kn": inp["f5W0"].astype(np.float16),
        "alr_ex0": np.concatenate([inp["f3al0"].reshape(1, 64), inp["f3ar0"].reshape(1, 64)], 1),
        "alr_ex1": np.concatenate([inp["f3al1"].reshape(1, 64), inp["f3ar1"].reshape(1, 64)], 1),
        "alr_st": np.concatenate([inp["f1al0"].reshape(1, 64), inp["f1ar0"].reshape(1, 64)], 1),
        "alr_kn": np.concatenate([inp["f5al0"].reshape(1, 64), inp["f5ar0"].reshape(1, 64)], 1),
        "semW": inp["f3sW"].astype(np.float32),
        "semb_col": inp["f3sb"].reshape(SEM, 1).astype(np.float32),
        "semq_col": inp["f3sq"].reshape(SEM, 1).astype(np.float32),
        "pWT_st": inp["f1pW"].T.astype(np.float32).copy(),
        "pb_st": inp["f1pb"].reshape(K, 1).astype(np.float32),
        "pWT_ex": inp["f3pW"].T.astype(np.float32).copy(),
        "pb_ex": inp["f3pb"].reshape(K, 1).astype(np.float32),
        "pW_kn": inp["f5pW"].astype(np.float32),
        "pb_kn_row": inp["f5pb"].reshape(1, K).astype(np.float32),
        "W1a": inp["W1"][:K].astype(np.float32),
        "W1b": inp["W1"][K:].astype(np.float32),
        "W2a": inp["W2"][:K].astype(np.float32),
        "W2b": inp["W2"][K:].astype(np.float32),
        "W3h": inp["W3"].astype(np.float16),
        "b3": inp["b3"].reshape(1, 1).astype(np.float32),
    }

    xsrc = {"ex0": inp["exer_t"], "ex1": inp["exer_t"],
            "st": inp["stu_t"], "kn": inp["kn_t"]}
    x_aug_T = {}
    for g in GRAPHS:
        xa = np.vstack([xsrc[g], np.zeros((1, K), np.float32)]).astype(np.float16)
        x_aug_T[g] = np.ascontiguousarray(xa.T)   # [K, N+1], col N = zeros

    # ------- per-core arrays -------
    in_maps = []
    for c in range(NC):
        m = dict(shared)
        for g in GRAPHS:
            ss, rowptr, counts = csr[g]
            flat, npad = _flat_src(tiles[g][c], dts[g], ss, rowptr, counts, None)
            flat = np.where(flat < 0, xsrc[g].shape[0], flat)
            m["xts_" + g] = _xts(x_aug_T[g], flat)
            m["npad_" + g] = npad
            m["xtp_" + g] = _xtp(xsrc[g], tiles[g][c], NTP[g])
        m["kn_rT"] = inp["kn_r"][c * BC:(c + 1) * BC].T.astype(np.float32).copy()
        in_maps.append(m)

    return meta, in_maps


# ----------------------------------------------------------------------------
# Bass program
# ----------------------------------------------------------------------------

PS_COLS = 24     # slot-cols per PSUM unit: z [P,24,64] (3 banks) + el [P,24,8]
FG = 4           # predictor batch rows per group


def build_program(meta):
    nc = bacc.Bacc("TRN2", num_devices=NC)
    dts = meta["dts"]
    NTP = meta["NTP"]
    SH = meta["SH"]
    SH_TILES, BS_TILES = meta["SH_TILES"], meta["BS_TILES"]
    nslot = meta["nslot"]
    MAXDT = -(-max(int(dts[g].max()) for g in GRAPHS) // 8) * 8

    ein = {}
    def EIN(name, shape, dt):
        ein[name] = nc.dram_tensor(name, list(shape), dt, kind="ExternalInput")
        return ein[name]

    for g in GRAPHS:
        EIN("w_" + g, (K, FD), F16)
        EIN("alr_" + g, (1, 128), F32)
        EIN("xts_" + g, (K, nslot[g] * P), F16)
        EIN("npad_" + g, (P, len(dts[g])), F32)
        EIN("xtp_" + g, (K, NTP[g] * P), F16)
    EIN("semW", (FD, SEM), F32); EIN("semb_col", (SEM, 1), F32); EIN("semq_col", (SEM, 1), F32)
    EIN("pWT_st", (K, FD), F32); EIN("pb_st", (K, 1), F32)
    EIN("pWT_ex", (K, FD), F32); EIN("pb_ex", (K, 1), F32)
    EIN("pW_kn", (FD, K), F32); EIN("pb_kn_row", (1, K), F32)
    EIN("W1a", (K, K), F32); EIN("W1b", (K, K), F32)
    EIN("W2a", (K, K), F32); EIN("W2b", (K, K), F32)
    EIN("W3h", (K, 1), F16); EIN("b3", (1, 1), F32)
    EIN("kn_rT", (K, BC), F32)

    out_d = nc.dram_tensor("out", [1, BC], F32, kind="ExternalOutput")

    cc_in = nc.dram_tensor("cc_in", [1, 16], F32, kind="Internal")
    cc_out = nc.dram_tensor("cc_out", [1, 16], F32, kind="Internal", addr_space="Shared")

    with tile.TileContext(nc) as tc:
        with tc.tile_pool(name="const", bufs=1) as cst, \
             tc.tile_pool(name="slab", bufs=1) as slab:
            nc.gpsimd.load_library(library_config.mlp)

            ident = cst.tile([P, P], F32, tag="ident", name="ident")
            make_identity(nc, ident[:])
            identh = cst.tile([P, P], F16, tag="identh", name="identh")
            nc.vector.tensor_copy(identh[:], ident[:])
            ones_col = cst.tile([P, 1], F32, tag="ones_col", name="ones_col")
            nc.vector.memset(ones_col[:], 1.0)
            ones_row = cst.tile([1, P], F32, tag="ones_row", name="ones_row")
            nc.vector.memset(ones_row[:], 1.0)

            # ---- load small weights ----
            def load(name, shape, dt):
                t = cst.tile(list(shape), dt, tag="ld_" + name, name="ld_" + name)
                nc.sync.dma_start(t[:], ein[name][:])
                return t
            w_g = {g: load("w_" + g, (K, FD), F16) for g in GRAPHS}
            alr = {g: load("alr_" + g, (1, 128), F32) for g in GRAPHS}
            npad_sb = {g: load("npad_" + g, (P, len(dts[g])), F32) for g in GRAPHS}
            semW = load("semW", (FD, SEM), F32)
            semb_col = load("semb_col", (SEM, 1), F32)
            semq_col = load("semq_col", (SEM, 1), F32)
            pWT_st = load("pWT_st", (K, FD), F32); pb_st = load("pb_st", (K, 1), F32)
            pWT_ex = load("pWT_ex", (K, FD), F32); pb_ex = load("pb_ex", (K, 1), F32)
            pW_kn = load("pW_kn", (FD, K), F32); pb_kn_row = load("pb_kn_row", (1, K), F32)
            W1a = load("W1a", (K, K), F32); W1b = load("W1b", (K, K), F32)
            W2a = load("W2a", (K, K), F32); W2b = load("W2b", (K, K), F32)
            W3h = load("W3h", (K, 1), F16); b3 = load("b3", (1, 1), F32)
            kn_rT = load("kn_rT", (K, BC), F32)

            # ---- fold al/ar into W: wcat = [W | W@al] fp16, war = W@ar ----
            wcat = {}   # [128, 72] f16: cols 0:64 W, 64:72 Wal
            war = {}    # [128, 8] f16
            with tc.tile_pool(name="bc_ps", bufs=2, space="PSUM") as bcp:
              for g in GRAPHS:
                alb = cst.tile([P, 128], F32, tag="alb", name="alb")
                alb_ps = bcp.tile([P, 128], F32, space="PSUM", tag="alb_ps", name="alb_ps")
                nc.tensor.matmul(alb_ps[:], lhsT=ones_row[:], rhs=alr[g][:])
                nc.vector.tensor_copy(alb[:], alb_ps[:])
                wf = cst.tile([P, FD], F32, tag="wf", name="wf")
                nc.vector.tensor_copy(wf[:], w_g[g][:])
                wtmp = cst.tile([P, FD], F32, tag="wtmp", name="wtmp")
                wc = cst.tile([P, 72], F16, tag="wcat_" + g, name="wcat_" + g)
                wcat[g] = wc
                nc.vector.tensor_copy(wc[:, 0:64], w_g[g][:])
                with nc.allow_low_precision(reason="8-elem head fold of fp16 weights"):
                    nc.vector.tensor_tensor(out=wtmp[:], in0=wf[:], in1=alb[:, 0:64], op=OP.mult)
                    nc.vector.tensor_reduce(out=wc[:, 64:72].bitcast(F16),
                                            in_=wtmp[:].rearrange("p (h f) -> p h f", h=H),
                                            axis=AX.X, op=OP.add)
                    wr = cst.tile([P, 8], F16, tag="war_" + g, name="war_" + g)
                    war[g] = wr
                    nc.vector.tensor_tensor(out=wtmp[:], in0=wf[:], in1=alb[:, 64:128], op=OP.mult)
                    nc.vector.tensor_reduce(out=wr[:], in_=wtmp[:].rearrange("p (h f) -> p h f", h=H),
                                            axis=AX.X, op=OP.add)

            # ---- er per (graph, tile): er[d, t, h] = (x[d] @ war)[h] ----
            er = {}
            with tc.tile_pool(name="pE", bufs=2) as pe, \
                 tc.tile_pool(name="pE_ps", bufs=4, space="PSUM") as pep:
                for g in GRAPHS:
                    ntp = NTP[g]
                    er_sb = slab.tile([P, ntp, 8], F32, tag="er_" + g, name="er_" + g)
                    er[g] = er_sb
                    xtp_sb = pe.tile([P, NTP["ex0"] * P], F16, tag="xtp_sb", name="xtp_sb")
                    nc.sync.dma_start(xtp_sb[:, 0:ntp * P], ein["xtp_" + g][:])
                    for t in range(ntp):
                        eps = pep.tile([P, 8], F32, space="PSUM", tag="eps", name="eps")
                        nc.tensor.matmul(eps[:], lhsT=xtp_sb[:, t * P:(t + 1) * P],
                                         rhs=war[g][:])
                        nc.vector.tensor_copy(er_sb[:, t, :], eps[:])

            # ---- Phase B: per-edge z via PE + edge softmax + aggregation ----
            zs = {g: slab.tile([P, NTP[g], FD], F32, tag="zs_" + g, name="zs_" + g)
                  for g in GRAPHS}
            m_slab = {g: slab.tile([P, NTP[g], 8], F32, tag="m_" + g, name="m_" + g)
                      for g in GRAPHS}
            s_slab = {g: slab.tile([P, NTP[g], 8], F32, tag="s_" + g, name="s_" + g)
                      for g in GRAPHS}

            def do_graph(g, pb, pbs, pzp):
                """Emit z matmuls + softmax for every tile of graph g.

                Small tiles (Dt <= PS_COLS): z and el stay resident in PSUM;
                the e-pipe reads el from PSUM and the weighted mult reads z
                from PSUM (no evacuation copies).  Big tiles copy per unit.
                """
                col0 = 0
                for t in range(NTP[g]):
                    Dt = int(dts[g][t])
                    DtP = -(-Dt // 8) * 8
                    small = Dt <= PS_COLS
                    z_el = pbs.tile([P, MAXDT, 64], F16, tag="z_el", name="z_el")
                    if not small:
                        el_t = pbs.tile([P, 8, MAXDT], F32, tag="el_t", name="el_t")
                    zp_keep = None
                    elp_keep = None
                    for lo in range(0, Dt, PS_COLS):
                        n_here = min(PS_COLS, Dt - lo)
                        xts_sb = pb.tile([P, PS_COLS * P], F16, tag="xts_sb", name="xts_sb")
                        nc.sync.dma_start(
                            xts_sb[:, 0:n_here * P],
                            ein["xts_" + g][:, (col0 + lo) * P:(col0 + lo + n_here) * P])
                        zp = pzp.tile([P, PS_COLS, 64], F32, space="PSUM",
                                      tag="zp", name="zp")
                        elp = pzp.tile([P, PS_COLS, 8], F32, space="PSUM",
                                       tag="elp", name="elp")
                        for ci in range(n_here):
                            lhsT = xts_sb[:, ci * P:(ci + 1) * P]
                            nc.tensor.matmul(zp[:, ci, :], lhsT=lhsT, rhs=wcat[g][:, 0:64])
                            nc.tensor.matmul(elp[:, ci, :], lhsT=lhsT, rhs=wcat[g][:, 64:72])
                        if small:
                            zp_keep, elp_keep = zp, elp
                        else:
                            nc.scalar.activation(out=z_el[:, lo:lo + n_here, :],
                                                 in_=zp[:, 0:n_here, :], func=AF.Copy)
                            nc.scalar.activation(
                                out=el_t[:].transpose([0, 2, 1])[:, lo:lo + n_here, :],
                                in_=elp[:, 0:n_here, :], func=AF.Copy)
                    # ---- edge softmax over the Dt slots (head-major e) ----
                    e = pbs.tile([P, 8, MAXDT], F32, tag="e_buf", name="e_buf")
                    if small:
                        nc.vector.tensor_tensor(
                            out=e[:].transpose([0, 2, 1])[:, 0:Dt, :],
                            in0=elp_keep[:, 0:Dt, :],
                            in1=er[g][:, t, :].unsqueeze(1).to_broadcast([P, Dt, 8]),
                            op=OP.add)
                    else:
                        nc.vector.tensor_tensor(
                            out=e[:, :, 0:Dt], in0=el_t[:, :, 0:Dt],
                            in1=er[g][:, t, :].unsqueeze(2).to_broadcast([P, 8, Dt]),
                            op=OP.add)
                    e2 = pbs.tile([P, 8, MAXDT], F32, tag="e2_buf", name="e2_buf")
                    nc.vector.scalar_tensor_tensor(out=e2[:, :, 0:Dt], in0=e[:, :, 0:Dt],
                                                   scalar=0.2, in1=e[:, :, 0:Dt],
                                                   op0=OP.mult, op1=OP.max)
                    m = m_slab[g][:, t, :]
                    nc.vector.tensor_reduce(out=m, in_=e2[:, :, 0:Dt],
                                            axis=AX.X, op=OP.max)
                    nc.vector.tensor_tensor(
                        out=e2[:, :, 0:Dt], in0=e2[:, :, 0:Dt],
                        in1=m.unsqueeze(2).to_broadcast([P, 8, Dt]), op=OP.subtract)
                    exb = pbs.tile([P, 8, MAXDT], F16, tag="exb_buf", name="exb_buf")
                    nc.scalar.activation(out=exb[:, :, 0:Dt], in_=e2[:, :, 0:Dt], func=AF.Exp)
                    s = s_slab[g][:, t, :]
                    nc.vector.tensor_reduce(out=s, in_=exb[:, :, 0:Dt],
                                            axis=AX.X, op=OP.add)
                    # weighted w = z * exb into SBUF (z from PSUM for small tiles)
                    z_src = (zp_keep[:, 0:Dt, :] if small else z_el[:, 0:Dt, :])
                    nc.vector.tensor_tensor(
                        out=z_el[:, 0:Dt, :].rearrange("p s (h f) -> p s h f", h=H),
                        in0=z_src.rearrange("p s (h f) -> p s h f", h=H),
                        in1=exb[:, :, 0:Dt].transpose([0, 2, 1]).unsqueeze(3)
                        .to_broadcast([P, Dt, 8, 8]),
                        op=OP.mult)
                    if DtP > Dt:
                        nc.gpsimd.memset(z_el[:, Dt:DtP, :], 0.0)
                    # 3 contiguous halvings then short strided tail reduce
                    cur = DtP
                    while cur > DtP // 8:
                        h2 = cur // 2
                        nc.gpsimd.tensor_tensor(out=z_el[:, 0:h2, :],
                                                in0=z_el[:, 0:h2, :],
                                                in1=z_el[:, h2:cur, :], op=OP.add)
                        cur = h2
                    v = zs[g][:, t, :]
                    nc.vector.tensor_reduce(
                        out=v, in_=z_el[:, 0:cur, :].transpose([0, 2, 1]),
                        axis=AX.X, op=OP.add)
                    col0 += Dt
                # ---- batched per-graph epilogue: pad fix, normalize, elu ----
                ntp = NTP[g]
                epb = pbs.tile([P, ntp, 8], F32, tag="epb_" + g, name="epb_" + g)
                nc.vector.scalar_tensor_tensor(out=epb[:], in0=er[g][:],
                                               scalar=0.2, in1=er[g][:],
                                               op0=OP.mult, op1=OP.max)
                nc.vector.tensor_tensor(out=epb[:], in0=epb[:], in1=m_slab[g][:],
                                        op=OP.subtract)
                pexb = pbs.tile([P, ntp, 8], F16, tag="pexb_" + g, name="pexb_" + g)
                nc.scalar.activation(out=pexb[:], in_=epb[:], func=AF.Exp)
                pexb32 = pbs.tile([P, ntp, 8], F32, tag="pexb32_" + g, name="pexb32_" + g)
                nc.vector.tensor_copy(pexb32[:], pexb[:])
                nc.vector.tensor_tensor(
                    out=pexb32[:], in0=pexb32[:],
                    in1=npad_sb[g][:].unsqueeze(2).to_broadcast([P, ntp, 8]),
                    op=OP.mult)
                nc.vector.tensor_tensor(out=s_slab[g][:], in0=s_slab[g][:],
                                        in1=pexb32[:], op=OP.add)
                nc.vector.tensor_scalar_add(s_slab[g][:], s_slab[g][:], 1e-9)
                nc.vector.reciprocal(s_slab[g][:], s_slab[g][:])
                nc.vector.tensor_tensor(
                    out=zs[g][:].rearrange("p t (h f) -> p t h f", h=H),
                    in0=zs[g][:].rearrange("p t (h f) -> p t h f", h=H),
                    in1=s_slab[g][:].unsqueeze(3).to_broadcast([P, ntp, 8, 8]),
                    op=OP.mult)
                # elu (batched)
                t1 = pbs.tile([P, ntp, FD], F32, tag="elu1_" + g, name="elu1_" + g)
                nc.vector.tensor_scalar_min(t1[:], zs[g][:], 0.0)
                t2 = pbs.tile([P, ntp, FD], F32, tag="elu2_" + g, name="elu2_" + g)
                nc.scalar.activation(out=t2[:], in_=t1[:], func=AF.Exp)
                nc.vector.tensor_tensor(out=zs[g][:], in0=zs[g][:], in1=t1[:],
                                        op=OP.subtract)
                nc.vector.scalar_tensor_tensor(out=zs[g][:], in0=t2[:], scalar=-1.0,
                                               in1=zs[g][:], op0=OP.add, op1=OP.add)

            zsT = {g: slab.tile([FD, NTP[g] * P], F32, tag="zsT_" + g, name="zsT_" + g)
                   for g in GRAPHS}

            def do_transposes(g, pcp):
                for t in range(NTP[g]):
                    tp = pcp.tile([FD, P], F32, space="PSUM", tag="tp_ps", name="tp_ps")
                    nc.tensor.transpose(out=tp[:], in_=zs[g][:, t, :], identity=ident[:])
                    if t % 2 == 0:
                        nc.scalar.copy(zsT[g][:, t * P:(t + 1) * P], tp[:])
                    else:
                        nc.vector.tensor_copy(zsT[g][:, t * P:(t + 1) * P], tp[:])

            with tc.tile_pool(name="pB", bufs=3) as pb, \
                 tc.tile_pool(name="pBs", bufs=2) as pbs, \
                 tc.tile_pool(name="pB_ps", bufs=2, space="PSUM") as pzp:
                do_graph("ex0", pb, pbs, pzp)
                do_graph("ex1", pb, pbs, pzp)

            with tc.tile_pool(name="pC_ps", bufs=4, space="PSUM") as pcp:
                do_transposes("ex0", pcp)
                do_transposes("ex1", pcp)

            # ---- semantic attention stats over the exercise share ----
            stats = cst.tile([1, 16], F32, tag="stats", name="stats")
            nc.vector.memset(stats[:], 0.0)
            with tc.tile_pool(name="pD", bufs=2) as pd, \
                 tc.tile_pool(name="pD_ps", bufs=4, space="PSUM") as pdp:
                parts = cst.tile([1, 16], F32, tag="parts", name="parts")
                for mi, g in enumerate(("ex0", "ex1")):
                    cw_list = []
                    lo = 0
                    while lo < SH:
                        cw = min(512, SH - lo)
                        cw_list.append((lo, cw))
                        lo += cw
                    for ci, (lo, cw) in enumerate(cw_list):
                        tps = pdp.tile([SEM, 512], F32, space="PSUM", tag="tps", name="tps")
                        nc.tensor.matmul(tps[:, 0:cw], lhsT=semW[:], rhs=zsT[g][:, lo:lo + cw])
                        tsb = pd.tile([SEM, 512], F32, tag="tsb", name="tsb")
                        nc.scalar.activation(out=tsb[:, 0:cw], in_=tps[:, 0:cw],
                                             func=AF.Tanh, bias=semb_col[:])
                        rps = pdp.tile([1, 512], F32, space="PSUM", tag="rps", name="rps")
                        nc.tensor.matmul(rps[:, 0:cw], lhsT=semq_col[:], rhs=tsb[:, 0:cw])
                        nc.vector.tensor_reduce(out=parts[:, mi * 8 + ci:mi * 8 + ci + 1],
                                                in_=rps[:, 0:cw], axis=AX.X, op=OP.add)
                    nc.vector.tensor_reduce(
                        out=stats[:, mi:mi + 1],
                        in_=parts[:, mi * 8:mi * 8 + len(cw_list)], axis=AX.X, op=OP.add)

            # ---- AllReduce the 2 stats scalars (overlapped with st/kn) ----
            nc.sync.dma_start(cc_in[:, 0:16], stats[:])
            nc.gpsimd.collective_compute(
                "AllReduce", OP.add,
                replica_groups=[list(range(NC))],
                ins=[cc_in[:, :]], outs=[cc_out[:, :]])

            with tc.tile_pool(name="pB2", bufs=3) as pb, \
                 tc.tile_pool(name="pB2s", bufs=2) as pbs, \
                 tc.tile_pool(name="pB2_ps", bufs=2, space="PSUM") as pzp:
                do_graph("st", pb, pbs, pzp)
                do_graph("kn", pb, pbs, pzp)

            with tc.tile_pool(name="pC2_ps", bufs=4, space="PSUM") as pcp:
                do_transposes("st", pcp)
                do_transposes("kn", pcp)

            gstats = cst.tile([1, 16], F32, tag="gstats", name="gstats")
            nc.sync.dma_start(gstats[:], cc_out[:, :])

            # ---- predictor prep ----
            beta_col = cst.tile([P, 2], F32, tag="beta_col", name="beta_col")
            bd = cst.tile([1, 2], F32, tag="bd", name="bd")
            nc.vector.tensor_tensor(out=bd[:, 0:1], in0=gstats[:, 0:1],
                                    in1=gstats[:, 1:2], op=OP.subtract)
            btmp = cst.tile([1, 2], F32, tag="btmp", name="btmp")
            nc.scalar.activation(out=btmp[:, 0:1], in_=bd[:, 0:1], func=AF.Sigmoid,
                                 scale=1.0 / E_N)
            nc.scalar.activation(out=btmp[:, 1:2], in_=bd[:, 0:1], func=AF.Sigmoid,
                                 scale=-1.0 / E_N)
            b3_col = cst.tile([P, 1], F32, tag="b3_col", name="b3_col")
            with tc.tile_pool(name="bc2_ps", bufs=2, space="PSUM") as bc2:
                bb_ps = bc2.tile([P, 4], F32, space="PSUM", tag="bb_ps", name="bb_ps")
                nc.tensor.matmul(bb_ps[:, 0:2], lhsT=ones_row[:], rhs=btmp[:])
                nc.tensor.matmul(bb_ps[:, 2:3], lhsT=ones_row[:], rhs=b3[:])
                nc.vector.tensor_copy(beta_col[:], bb_ps[:, 0:2])
                nc.vector.tensor_copy(b3_col[:], bb_ps[:, 2:3])

            # fused exercise b-slot features: zsFT = b0*zsT_ex0 + b1*zsT_ex1
            zsFT = cst.tile([FD, BC], F32, tag="zsFT", name="zsFT")
            bcol = SH_TILES * P
            nc.vector.tensor_scalar(out=zsFT[:], in0=zsT["ex0"][:, bcol:bcol + BC],
                                    scalar1=beta_col[0:FD, 0:1], scalar2=None,
                                    op0=OP.mult)
            nc.vector.scalar_tensor_tensor(out=zsFT[:], in0=zsT["ex1"][:, bcol:bcol + BC],
                                           scalar=beta_col[0:FD, 1:2], in1=zsFT[:],
                                           op0=OP.mult, op1=OP.add)

            q1jk = cst.tile([P, K], F16, tag="q1jk", name="q1jk")      # Q1 [j, k]
            st_sb = cst.tile([P, K], F16, tag="st_sb", name="st_sb")   # Q2T [k, j]
            m1_sb = cst.tile([FD, K], F32, tag="m1_sb", name="m1_sb")
            m2_sb = cst.tile([FD, K], F32, tag="m2_sb", name="m2_sb")
            c1t = cst.tile([P, 1], F32, tag="c1t", name="c1t")
            c2t = cst.tile([P, 1], F32, tag="c2t", name="c2t")
            kn1T = cst.tile([P, K], F32, tag="kn1T", name="kn1T")
            U1bk = cst.tile([P, 2, K], F16, tag="U1bk", name="U1bk")   # U1 [b, k]
            U2T = cst.tile([P, BC], F16, tag="U2T", name="U2T")
            with tc.tile_pool(name="pF_ps", bufs=2, space="PSUM") as pfp:
                kn1_ps = pfp.tile([P, K], F32, space="PSUM", tag="prep_ps", name="kn1_ps")
                nc.tensor.matmul(kn1_ps[:], lhsT=zsT["kn"][:], rhs=pW_kn[:],
                                 start=True, stop=False)
                nc.tensor.matmul(kn1_ps[:], lhsT=ones_row[:], rhs=pb_kn_row[:],
                                 start=False, stop=True)
                kn1_sb = cst.tile([P, K], F32, tag="kn1_sb", name="kn1_sb")
                nc.scalar.copy(kn1_sb[:], kn1_ps[:])
                kn1T_ps = pfp.tile([P, K], F32, space="PSUM", tag="prep_ps", name="kn1T_ps")
                nc.tensor.transpose(out=kn1T_ps[:], in_=kn1_sb[:], identity=ident[:])
                nc.scalar.copy(kn1T[:], kn1T_ps[:])

                qs_ps = pfp.tile([P, K], F32, space="PSUM", tag="prep_ps", name="qs_ps")
                nc.tensor.matmul(qs_ps[:], lhsT=kn1T[:], rhs=W1b[:])
                nc.scalar.copy(q1jk[:], qs_ps[:])
                qs2_ps = pfp.tile([P, K], F32, space="PSUM", tag="prep_ps", name="qs2_ps")
                nc.tensor.matmul(qs2_ps[:], lhsT=W2b[:], rhs=kn1T[:])
                nc.scalar.copy(st_sb[:], qs2_ps[:])

                m1_ps = pfp.tile([FD, K], F32, space="PSUM", tag="prep_ps", name="m1_ps")
                nc.tensor.matmul(m1_ps[:], lhsT=pWT_st[:], rhs=W1a[:])
                nc.scalar.copy(m1_sb[:], m1_ps[:])
                m2_ps = pfp.tile([FD, K], F32, space="PSUM", tag="prep_ps", name="m2_ps")
                nc.tensor.matmul(m2_ps[:], lhsT=pWT_ex[:], rhs=W2a[:])
                nc.scalar.copy(m2_sb[:], m2_ps[:])
                c1_ps = pfp.tile([P, 1], F32, space="PSUM", tag="prep_ps", name="c1_ps")
                nc.tensor.matmul(c1_ps[:], lhsT=W1a[:], rhs=pb_st[:])
                nc.vector.tensor_copy(c1t[:], c1_ps[:])
                c2_ps = pfp.tile([P, 1], F32, space="PSUM", tag="prep_ps", name="c2_ps")
                nc.tensor.matmul(c2_ps[:], lhsT=W2a[:], rhs=pb_ex[:])
                nc.vector.tensor_copy(c2t[:], c2_ps[:])

                for bh in range(2):
                    u1_ps = pfp.tile([P, K], F32, space="PSUM", tag="u_ps", name="u1_ps")
                    nc.tensor.matmul(u1_ps[:], lhsT=zsT["st"][:, bh * P:(bh + 1) * P],
                                     rhs=m1_sb[:])
                    nc.vector.tensor_copy(U1bk[:, bh, :], u1_ps[:])
                u2_ps = pfp.tile([P, BC], F32, space="PSUM", tag="u_ps", name="u2_ps")
                nc.tensor.matmul(u2_ps[:], lhsT=m2_sb[:], rhs=zsFT[:])
                nc.vector.tensor_copy(U2T[:], u2_ps[:])

            # ---- predictor main loop ----
            # pref[b,j,k] = sig(U1[b,k] + Q1[j,k] + c1[k]); layout [k, (g,j)]
            with tc.tile_pool(name="pG", bufs=3) as pg, \
                 tc.tile_pool(name="pG_ps", bufs=2, space="PSUM") as pgp, \
                 tc.tile_pool(name="pO_ps", bufs=1, space="PSUM") as pop:
                o_ps = pop.tile([P, BC], F32, space="PSUM", tag="o_ps", name="o_ps")
                for grp in range(BC // FG):
                    b0 = grp * FG
                    # pref on PE: out[k,(g,j)] = Q1[j,k] (dj) + U1[b0+g,k] (dg)
                    pr_ps = pgp.tile([P, FG, K], F32, space="PSUM", tag="pr_ps",
                                     name="pr_ps")
                    nc.tensor.matmul(
                        pr_ps[:], lhsT=q1jk[:],
                        rhs=identh[:].unsqueeze(1).to_broadcast([P, FG, K]),
                        start=True, stop=False)
                    nc.tensor.matmul(
                        pr_ps[:], lhsT=U1bk[:, b0 // P, :],
                        rhs=identh[:, b0 % P:b0 % P + FG].unsqueeze(2)
                        .to_broadcast([P, FG, K]),
                        start=False, stop=True)
                    pr_sb = pg.tile([P, FG, K], F16, tag="pr_sb", name="pr_sb")
                    nc.scalar.activation(out=pr_sb[:], in_=pr_ps[:], func=AF.Sigmoid,
                                         bias=c1t[:])
                    df_lin = pg.tile([P, FG, K], F16, tag="df_lin", name="df_lin")
                    nc.vector.tensor_tensor(
                        out=df_lin[:],
                        in0=st_sb[:].unsqueeze(1).to_broadcast([P, FG, K]),
                        in1=U2T[:, b0:b0 + FG].unsqueeze(2).to_broadcast([P, FG, K]),
                        op=OP.add)
                    df_sb = pg.tile([P, FG, K], F16, tag="df_sb", name="df_sb")
                    nc.scalar.activation(out=df_sb[:], in_=df_lin[:], func=AF.Sigmoid,
                                         bias=c2t[:])
                    d_sb = pg.tile([P, FG, K], F16, tag="d_sb", name="d_sb")
                    nc.vector.tensor_tensor(out=d_sb[:], in0=pr_sb[:], in1=df_sb[:],
                                            op=OP.subtract)
                    for lb in range(FG):
                        nc.tensor.matmul(o_ps[:, b0 + lb:b0 + lb + 1],
                                         lhsT=d_sb[:, lb, :], rhs=W3h[:])

                # ---- final ----
                with tc.tile_pool(name="pN_ps", bufs=1, space="PSUM") as pnp:
                    o_sb = pg.tile([P, BC], F32, tag="o_sb", name="o_sb")
                    nc.scalar.activation(out=o_sb[:], in_=o_ps[:], func=AF.Sigmoid,
                                         bias=b3_col[:])
                    om = pg.tile([P, BC], F32, tag="om", name="om")
                    nc.vector.tensor_tensor(out=om[:], in0=o_sb[:], in1=kn_rT[:], op=OP.mult)
                    nd_ps = pnp.tile([1, 2 * BC], F32, space="PSUM", tag="nd_ps", name="nd_ps")
                    nc.tensor.matmul(nd_ps[:, 0:BC], lhsT=ones_col[:], rhs=om[:])
                    nc.tensor.matmul(nd_ps[:, BC:2 * BC], lhsT=ones_col[:], rhs=kn_rT[:])
                    rcp = pg.tile([1, BC], F32, tag="rcp", name="rcp")
                    nc.vector.reciprocal(rcp[:], nd_ps[:, BC:2 * BC])
                    res = pg.tile([1, BC], F32, tag="res", name="res")
                    nc.vector.tensor_tensor(out=res[:], in0=nd_ps[:, 0:BC], in1=rcp[:],
                                            op=OP.mult)
                    nc.sync.dma_start(out_d[:], res[:])

    nc.compile()
    return nc


# ----------------------------------------------------------------------------
# Entry point
# ----------------------------------------------------------------------------

_TRACE = bool(int(os.environ.get("KERNEL_TRACE", "0")))


def kernel(**inputs):
    meta, in_maps = preprocess(inputs)
    nc = build_program(meta)
    res = bass_utils.run_bass_kernel_spmd(
        nc, in_maps, core_ids=list(range(NC)), trace=_TRACE)
    out = np.concatenate([r["out"].reshape(-1) for r in res.results])
    kernel.last_results = res
    return out.reshape(B, 1).astype(np.float32)


# revision 15
# speedup vs baseline: 6.3175x; 1.1317x over previous
"""Trainium2 Bass kernel for the HAN-based cognitive-diagnosis net.

Strategy (8 NeuronCores, SPMD — one program, per-core data):
  * Batch (2048) split 8x256 across cores. Each core computes GAT outputs
    for its own batch-slice node list plus a 1/8 share of all exercise
    nodes (for the global-mean semantic attention stats, AllReduce'd).
  * Edge phase without any device gather: the edge lists are known on the
    host, so the host pre-expands x^T into the ELL slot layout (one
    x-column per edge slot, zero column for pad slots).  The device
    computes per-edge [z(64) | el(8)] directly with PE matmuls
    (lhsT = xts slot tile, rhs = W folded with a_l), then runs the
    edge softmax + weighted aggregation on DVE/ACT with dst nodes on
    partitions.  Pad slots have z=0 (exact numerator); the softmax
    denominator is corrected analytically:
        s_real = s - npad[d] * exp(leaky(er[d]) - m[d])
    since every pad slot contributes exactly that one value.
  * Predictor exploits the rank structure pref[b,j,k] =
    sigmoid(U1[b,k] + Q1[j,k] + c1[k]): two small matmuls build U/Q once,
    the [B,K,K] tensors are pure broadcast-add + sigmoid, and the W3
    contraction runs on PE per batch row.
"""

import os
import numpy as np

import concourse.bass as bass
import concourse.bacc as bacc
import concourse.mybir as mybir
import concourse.tile as tile
from concourse import library_config
from concourse.masks import make_identity
from concourse import bass_utils

F32 = mybir.dt.float32
F16 = mybir.dt.float16

NC = 8
B = 2048
BC = B // NC          # 256 batch rows per core
K = 128
H, D, FD = 8, 8, 64
SEM = 128
S_N, E_N = 10000, 20000
P = 128

AX = mybir.AxisListType
OP = mybir.AluOpType
AF = mybir.ActivationFunctionType

GRAPHS = ("ex0", "ex1", "st", "kn")


# ----------------------------------------------------------------------------
# Host-side preprocessing (integer / layout only)
# ----------------------------------------------------------------------------

def _csr_by_dst(src, dst, n):
    order = np.argsort(dst, kind="stable")
    ss = src[order].astype(np.int64)
    counts = np.bincount(dst, minlength=n)
    rowptr = np.zeros(n + 1, np.int64)
    np.cumsum(counts, out=rowptr[1:])
    return ss, rowptr, counts


def _tiles_of(nodes):
    return [np.asarray(nodes[i:i + P]) for i in range(0, len(nodes), P)]


def _flat_src(node_tiles, dts, ss, rowptr, counts, npad_row):
    """Flat per-edge-slot src index list (tile-major, col-major, 128 lanes)
    with -1 for pad slots, plus the [128, ntiles] pad-count array."""
    nslot = int(np.sum(dts))
    flat = np.full((nslot, P), -1, np.int64)
    npad = np.zeros((P, len(node_tiles)), np.float32)
    col = 0
    for t, nodes in enumerate(node_tiles):
        dt = int(dts[t])
        for pi, node in enumerate(nodes):
            deg = int(counts[node])
            if deg:
                lo = rowptr[node]
                flat[col:col + deg, pi] = ss[lo:lo + deg]
            npad[pi, t] = -(dt - deg)
        for pi in range(len(nodes), P):
            npad[pi, t] = -dt
        col += dt
    assert col == nslot
    return flat.reshape(-1), npad


def _xts(x_aug_T, flat):
    """[K, nslot*128] fp16 ELL-expanded x^T (zero column for pad slots)."""
    return np.ascontiguousarray(x_aug_T[:, flat])


def _xtp(x, node_tiles, ntiles):
    """x^T columns for a node list, padded to ntiles*128 cols, fp16."""
    kdim = x.shape[1]
    out = np.zeros((kdim, ntiles * P), np.float16)
    for t, nodes in enumerate(node_tiles):
        out[:, t * P:t * P + len(nodes)] = x[nodes].T.astype(np.float16)
    return out


def preprocess(inputs):
    inp = {k: np.asarray(v) for k, v in inputs.items()}
    stu_id = inp["stu_id"].astype(np.int64)
    exer_id = inp["exer_id"].astype(np.int64)

    csr = {
        "st": _csr_by_dst(inp["ss0"].astype(np.int64), inp["sd0"].astype(np.int64), S_N),
        "ex0": _csr_by_dst(inp["es0"].astype(np.int64), inp["ed0"].astype(np.int64), E_N),
        "ex1": _csr_by_dst(inp["es1"].astype(np.int64), inp["ed1"].astype(np.int64), E_N),
        "kn": _csr_by_dst(inp["ks0"].astype(np.int64), inp["kd0"].astype(np.int64), K),
    }

    # per-core node tile lists; exercise share nodes degree-sorted + strided
    SH = E_N // NC                      # 2500
    SH_TILES = (SH + P - 1) // P        # 20
    BS_TILES = BC // P                  # 2
    NTP = {"ex0": SH_TILES + BS_TILES, "ex1": SH_TILES + BS_TILES,
           "st": BS_TILES, "kn": 1}

    tiles = {g: [] for g in GRAPHS}     # g -> [core][tile] node arrays
    for g in ("ex0", "ex1"):
        order = np.argsort(-csr[g][2], kind="stable")
        for c in range(NC):
            share = order[c::NC]
            assert len(share) == SH
            tiles[g].append(_tiles_of(share) +
                            _tiles_of(exer_id[c * BC:(c + 1) * BC]))
    for c in range(NC):
        tiles["st"].append(_tiles_of(stu_id[c * BC:(c + 1) * BC]))
        tiles["kn"].append(_tiles_of(np.arange(K)))

    # shared per-tile Dt = max over cores (SPMD: one program)
    dts = {}
    for g in GRAPHS:
        counts = csr[g][2]
        dts[g] = np.max(
            [[max(1, int(counts[t].max()) if len(t) else 1) for t in tiles[g][c]]
             for c in range(NC)], axis=0)

    meta = dict(dts=dts, NTP=NTP, SH=SH, SH_TILES=SH_TILES, BS_TILES=BS_TILES,
                nslot={g: int(dts[g].sum()) for g in GRAPHS})

    # ------- shared input arrays -------
    shared = {
        "w_ex0": inp["f3W0"].astype(np.float16),
        "w_ex1": inp["f3W1"].astype(np.float16),
        "w_st": inp["f1W0"].astype(np.float16),
        "w_kn": inp["f5W0"].astype(np.float16),
        "alr_ex0": np.concatenate([inp["f3al0"].reshape(1, 64), inp["f3ar0"].reshape(1, 64)], 1),
        "alr_ex1": np.concatenate([inp["f3al1"].reshape(1, 64), inp["f3ar1"].reshape(1, 64)], 1),
        "alr_st": np.concatenate([inp["f1al0"].reshape(1, 64), inp["f1ar0"].reshape(1, 64)], 1),
        "alr_kn": np.concatenate([inp["f5al0"].reshape(1, 64), inp["f5ar0"].reshape(1, 64)], 1),
        "semW": inp["f3sW"].astype(np.float32),
        "semb_col": inp["f3sb"].reshape(SEM, 1).astype(np.float32),
        "semq_col": inp["f3sq"].reshape(SEM, 1).astype(np.float32),
        "pWT_st": inp["f1pW"].T.astype(np.float32).copy(),
        "pb_st": inp["f1pb"].reshape(K, 1).astype(np.float32),
        "pWT_ex": inp["f3pW"].T.astype(np.float32).copy(),
        "pb_ex": inp["f3pb"].reshape(K, 1).astype(np.float32),
        "pW_kn": inp["f5pW"].astype(np.float32),
        "pb_kn_row": inp["f5pb"].reshape(1, K).astype(np.float32),
        "W1a": inp["W1"][:K].astype(np.float32),
        "W1b": inp["W1"][K:].astype(np.float32),
        "W2a": inp["W2"][:K].astype(np.float32),
        "W2b": inp["W2"][K:].astype(np.float32),
        "W3h": inp["W3"].astype(np.float16),
        "b3": inp["b3"].reshape(1, 1).astype(np.float32),
    }

    xsrc = {"ex0": inp["exer_t"], "ex1": inp["exer_t"],
            "st": inp["stu_t"], "kn": inp["kn_t"]}
    x_aug_T = {}
    for g in GRAPHS:
        xa = np.vstack([xsrc[g], np.zeros((1, K), np.float32)]).astype(np.float16)
        x_aug_T[g] = np.ascontiguousarray(xa.T)   # [K, N+1], col N = zeros

    # ------- per-core arrays -------
    in_maps = []
    for c in range(NC):
        m = dict(shared)
        for g in GRAPHS:
            ss, rowptr, counts = csr[g]
            flat, npad = _flat_src(tiles[g][c], dts[g], ss, rowptr, counts, None)
            flat = np.where(flat < 0, xsrc[g].shape[0], flat)
            m["xts_" + g] = _xts(x_aug_T[g], flat)
            m["npad_" + g] = npad
            m["xtp_" + g] = _xtp(xsrc[g], tiles[g][c], NTP[g])
        m["kn_rT"] = inp["kn_r"][c * BC:(c + 1) * BC].T.astype(np.float32).copy()
        in_maps.append(m)

    return meta, in_maps


# ----------------------------------------------------------------------------
# Bass program
# ----------------------------------------------------------------------------

PS_COLS = 24     # slot-cols per PSUM unit: z [P,24,64] (3 banks) + el [P,24,8]
FG = 4           # predictor batch rows per group


def build_program(meta):
    nc = bacc.Bacc("TRN2", num_devices=NC)
    dts = meta["dts"]
    NTP = meta["NTP"]
    SH = meta["SH"]
    SH_TILES, BS_TILES = meta["SH_TILES"], meta["BS_TILES"]
    nslot = meta["nslot"]
    MAXDT = -(-max(int(dts[g].max()) for g in GRAPHS) // 8) * 8

    ein = {}
    def EIN(name, shape, dt):
        ein[name] = nc.dram_tensor(name, list(shape), dt, kind="ExternalInput")
        return ein[name]

    for g in GRAPHS:
        EIN("w_" + g, (K, FD), F16)
        EIN("alr_" + g, (1, 128), F32)
        EIN("xts_" + g, (K, nslot[g] * P), F16)
        EIN("npad_" + g, (P, len(dts[g])), F32)
        EIN("xtp_" + g, (K, NTP[g] * P), F16)
    EIN("semW", (FD, SEM), F32); EIN("semb_col", (SEM, 1), F32); EIN("semq_col", (SEM, 1), F32)
    EIN("pWT_st", (K, FD), F32); EIN("pb_st", (K, 1), F32)
    EIN("pWT_ex", (K, FD), F32); EIN("pb_ex", (K, 1), F32)
    EIN("pW_kn", (FD, K), F32); EIN("pb_kn_row", (1, K), F32)
    EIN("W1a", (K, K), F32); EIN("W1b", (K, K), F32)
    EIN("W2a", (K, K), F32); EIN("W2b", (K, K), F32)
    EIN("W3h", (K, 1), F16); EIN("b3", (1, 1), F32)
    EIN("kn_rT", (K, BC), F32)

    out_d = nc.dram_tensor("out", [1, BC], F32, kind="ExternalOutput")

    cc_in = nc.dram_tensor("cc_in", [1, 16], F32, kind="Internal")
    cc_out = nc.dram_tensor("cc_out", [1, 16], F32, kind="Internal", addr_space="Shared")

    with tile.TileContext(nc) as tc:
        with tc.tile_pool(name="const", bufs=1) as cst, \
             tc.tile_pool(name="slab", bufs=1) as slab:
            nc.gpsimd.load_library(library_config.mlp)

            ident = cst.tile([P, P], F32, tag="ident", name="ident")
            make_identity(nc, ident[:])
            identh = cst.tile([P, P], F16, tag="identh", name="identh")
            nc.vector.tensor_copy(identh[:], ident[:])
            ones_col = cst.tile([P, 1], F32, tag="ones_col", name="ones_col")
            nc.vector.memset(ones_col[:], 1.0)
            ones_row = cst.tile([1, P], F32, tag="ones_row", name="ones_row")
            nc.vector.memset(ones_row[:], 1.0)

            # ---- load small weights ----
            def load(name, shape, dt):
                t = cst.tile(list(shape), dt, tag="ld_" + name, name="ld_" + name)
                nc.sync.dma_start(t[:], ein[name][:])
                return t
            w_g = {g: load("w_" + g, (K, FD), F16) for g in GRAPHS}
            alr = {g: load("alr_" + g, (1, 128), F32) for g in GRAPHS}
            npad_sb = {g: load("npad_" + g, (P, len(dts[g])), F32) for g in GRAPHS}
            semW = load("semW", (FD, SEM), F32)
            semb_col = load("semb_col", (SEM, 1), F32)
            semq_col = load("semq_col", (SEM, 1), F32)
            pWT_st = load("pWT_st", (K, FD), F32); pb_st = load("pb_st", (K, 1), F32)
            pWT_ex = load("pWT_ex", (K, FD), F32); pb_ex = load("pb_ex", (K, 1), F32)
            pW_kn = load("pW_kn", (FD, K), F32); pb_kn_row = load("pb_kn_row", (1, K), F32)
            W1a = load("W1a", (K, K), F32); W1b = load("W1b", (K, K), F32)
            W2a = load("W2a", (K, K), F32); W2b = load("W2b", (K, K), F32)
            W3h = load("W3h", (K, 1), F16); b3 = load("b3", (1, 1), F32)
            kn_rT = load("kn_rT", (K, BC), F32)

            # ---- fold al/ar into W: wcat = [W | W@al] fp16, war = W@ar ----
            wcat = {}   # [128, 72] f16: cols 0:64 W, 64:72 Wal
            war = {}    # [128, 8] f16
            with tc.tile_pool(name="bc_ps", bufs=2, space="PSUM") as bcp:
              for g in GRAPHS:
                alb = cst.tile([P, 128], F32, tag="alb", name="alb")
                alb_ps = bcp.tile([P, 128], F32, space="PSUM", tag="alb_ps", name="alb_ps")
                nc.tensor.matmul(alb_ps[:], lhsT=ones_row[:], rhs=alr[g][:])
                nc.vector.tensor_copy(alb[:], alb_ps[:])
                wf = cst.tile([P, FD], F32, tag="wf", name="wf")
                nc.vector.tensor_copy(wf[:], w_g[g][:])
                wtmp = cst.tile([P, FD], F32, tag="wtmp", name="wtmp")
                wc = cst.tile([P, 72], F16, tag="wcat_" + g, name="wcat_" + g)
                wcat[g] = wc
                nc.vector.tensor_copy(wc[:, 0:64], w_g[g][:])
                with nc.allow_low_precision(reason="8-elem head fold of fp16 weights"):
                    nc.vector.tensor_tensor(out=wtmp[:], in0=wf[:], in1=alb[:, 0:64], op=OP.mult)
                    nc.vector.tensor_reduce(out=wc[:, 64:72].bitcast(F16),
                                            in_=wtmp[:].rearrange("p (h f) -> p h f", h=H),
                                            axis=AX.X, op=OP.add)
                    wr = cst.tile([P, 8], F16, tag="war_" + g, name="war_" + g)
                    war[g] = wr
                    nc.vector.tensor_tensor(out=wtmp[:], in0=wf[:], in1=alb[:, 64:128], op=OP.mult)
                    nc.vector.tensor_reduce(out=wr[:], in_=wtmp[:].rearrange("p (h f) -> p h f", h=H),
                                            axis=AX.X, op=OP.add)

            # ---- er per (graph, tile): er[d, t, h] = (x[d] @ war)[h] ----
            er = {}
            with tc.tile_pool(name="pE", bufs=2) as pe, \
                 tc.tile_pool(name="pE_ps", bufs=4, space="PSUM") as pep:
                for g in GRAPHS:
                    ntp = NTP[g]
                    er_sb = slab.tile([P, ntp, 8], F32, tag="er_" + g, name="er_" + g)
                    er[g] = er_sb
                    xtp_sb = pe.tile([P, NTP["ex0"] * P], F16, tag="xtp_sb", name="xtp_sb")
                    nc.sync.dma_start(xtp_sb[:, 0:ntp * P], ein["xtp_" + g][:])
                    for t in range(ntp):
                        eps = pep.tile([P, 8], F32, space="PSUM", tag="eps", name="eps")
                        nc.tensor.matmul(eps[:], lhsT=xtp_sb[:, t * P:(t + 1) * P],
                                         rhs=war[g][:])
                        nc.vector.tensor_copy(er_sb[:, t, :], eps[:])

            # ---- Phase B: per-edge z via PE + edge softmax + aggregation ----
            zs = {g: slab.tile([P, NTP[g], FD], F32, tag="zs_" + g, name="zs_" + g)
                  for g in GRAPHS}
            m_slab = {g: slab.tile([P, NTP[g], 8], F32, tag="m_" + g, name="m_" + g)
                      for g in GRAPHS}
            s_slab = {g: slab.tile([P, NTP[g], 8], F32, tag="s_" + g, name="s_" + g)
                      for g in GRAPHS}

            def do_graph(g, pb, pbs, pzp):
                """Emit z matmuls + softmax for every tile of graph g.

                Small tiles (Dt <= PS_COLS): z and el stay resident in PSUM;
                the e-pipe reads el from PSUM and the weighted mult reads z
                from PSUM (no evacuation copies).  Big tiles copy per unit.
                """
                col0 = 0
                for t in range(NTP[g]):
                    Dt = int(dts[g][t])
                    DtP = -(-Dt // 8) * 8
                    small = Dt <= PS_COLS
                    z_el = pbs.tile([P, MAXDT, 64], F16, tag="z_el", name="z_el")
                    if not small:
                        el_t = pbs.tile([P, 8, MAXDT], F32, tag="el_t", name="el_t")
                    zp_keep = None
                    elp_keep = None
                    for lo in range(0, Dt, PS_COLS):
                        n_here = min(PS_COLS, Dt - lo)
                        xts_sb = pb.tile([P, PS_COLS * P], F16, tag="xts_sb", name="xts_sb")
                        nc.sync.dma_start(
                            xts_sb[:, 0:n_here * P],
                            ein["xts_" + g][:, (col0 + lo) * P:(col0 + lo + n_here) * P])
                        zp = pzp.tile([P, PS_COLS, 64], F32, space="PSUM",
                                      tag="zp", name="zp")
                        elp = pzp.tile([P, PS_COLS, 8], F32, space="PSUM",
                                       tag="elp", name="elp")
                        for ci in range(n_here):
                            lhsT = xts_sb[:, ci * P:(ci + 1) * P]
                            nc.tensor.matmul(zp[:, ci, :], lhsT=lhsT, rhs=wcat[g][:, 0:64])
                            nc.tensor.matmul(elp[:, ci, :], lhsT=lhsT, rhs=wcat[g][:, 64:72])
                        if small:
                            zp_keep, elp_keep = zp, elp
                        else:
                            nc.scalar.activation(out=z_el[:, lo:lo + n_here, :],
                                                 in_=zp[:, 0:n_here, :], func=AF.Copy)
                            nc.scalar.activation(
                                out=el_t[:].transpose([0, 2, 1])[:, lo:lo + n_here, :],
                                in_=elp[:, 0:n_here, :], func=AF.Copy)
                    # ---- edge softmax over the Dt slots (head-major e) ----
                    e = pbs.tile([P, 8, MAXDT], F32, tag="e_buf", name="e_buf")
                    if small:
                        nc.vector.tensor_tensor(
                            out=e[:].transpose([0, 2, 1])[:, 0:Dt, :],
                            in0=elp_keep[:, 0:Dt, :],
                            in1=er[g][:, t, :].unsqueeze(1).to_broadcast([P, Dt, 8]),
                            op=OP.add)
                    else:
                        nc.vector.tensor_tensor(
                            out=e[:, :, 0:Dt], in0=el_t[:, :, 0:Dt],
                            in1=er[g][:, t, :].unsqueeze(2).to_broadcast([P, 8, Dt]),
                            op=OP.add)
                    e2 = pbs.tile([P, 8, MAXDT], F32, tag="e2_buf", name="e2_buf")
                    nc.vector.scalar_tensor_tensor(out=e2[:, :, 0:Dt], in0=e[:, :, 0:Dt],
                                                   scalar=0.2, in1=e[:, :, 0:Dt],
                                                   op0=OP.mult, op1=OP.max)
                    m = m_slab[g][:, t, :]
                    nc.vector.tensor_reduce(out=m, in_=e2[:, :, 0:Dt],
                                            axis=AX.X, op=OP.max)
                    nc.vector.tensor_tensor(
                        out=e2[:, :, 0:Dt], in0=e2[:, :, 0:Dt],
                        in1=m.unsqueeze(2).to_broadcast([P, 8, Dt]), op=OP.subtract)
                    exb = pbs.tile([P, 8, MAXDT], F16, tag="exb_buf", name="exb_buf")
                    nc.scalar.activation(out=exb[:, :, 0:Dt], in_=e2[:, :, 0:Dt], func=AF.Exp)
                    s = s_slab[g][:, t, :]
                    nc.vector.tensor_reduce(out=s, in_=exb[:, :, 0:Dt],
                                            axis=AX.X, op=OP.add)
                    # weighted w = z * exb into SBUF (z from PSUM for small tiles)
                    z_src = (zp_keep[:, 0:Dt, :] if small else z_el[:, 0:Dt, :])
                    nc.vector.tensor_tensor(
                        out=z_el[:, 0:Dt, :].rearrange("p s (h f) -> p s h f", h=H),
                        in0=z_src.rearrange("p s (h f) -> p s h f", h=H),
                        in1=exb[:, :, 0:Dt].transpose([0, 2, 1]).unsqueeze(3)
                        .to_broadcast([P, Dt, 8, 8]),
                        op=OP.mult)
                    if DtP > Dt:
                        nc.gpsimd.memset(z_el[:, Dt:DtP, :], 0.0)
                    # 3 contiguous halvings then short strided tail reduce
                    cur = DtP
                    while cur > DtP // 8:
                        h2 = cur // 2
                        nc.vector.tensor_tensor(out=z_el[:, 0:h2, :],
                                                in0=z_el[:, 0:h2, :],
                                                in1=z_el[:, h2:cur, :], op=OP.add)
                        cur = h2
                    v = zs[g][:, t, :]
                    nc.vector.tensor_reduce(
                        out=v, in_=z_el[:, 0:cur, :].transpose([0, 2, 1]),
                        axis=AX.X, op=OP.add)
                    col0 += Dt
                # ---- batched per-graph epilogue: pad fix, normalize, elu ----
                ntp = NTP[g]
                epb = pbs.tile([P, ntp, 8], F32, tag="epb_" + g, name="epb_" + g)
                nc.vector.scalar_tensor_tensor(out=epb[:], in0=er[g][:],
                                               scalar=0.2, in1=er[g][:],
                                               op0=OP.mult, op1=OP.max)
                nc.vector.tensor_tensor(out=epb[:], in0=epb[:], in1=m_slab[g][:],
                                        op=OP.subtract)
                pexb = pbs.tile([P, ntp, 8], F16, tag="pexb_" + g, name="pexb_" + g)
                nc.scalar.activation(out=pexb[:], in_=epb[:], func=AF.Exp)
                pexb32 = pbs.tile([P, ntp, 8], F32, tag="pexb32_" + g, name="pexb32_" + g)
                nc.vector.tensor_copy(pexb32[:], pexb[:])
                nc.vector.tensor_tensor(
                    out=pexb32[:], in0=pexb32[:],
                    in1=npad_sb[g][:].unsqueeze(2).to_broadcast([P, ntp, 8]),
                    op=OP.mult)
                nc.vector.tensor_tensor(out=s_slab[g][:], in0=s_slab[g][:],
                                        in1=pexb32[:], op=OP.add)
                nc.vector.tensor_scalar_add(s_slab[g][:], s_slab[g][:], 1e-9)
                nc.vector.reciprocal(s_slab[g][:], s_slab[g][:])
                nc.vector.tensor_tensor(
                    out=zs[g][:].rearrange("p t (h f) -> p t h f", h=H),
                    in0=zs[g][:].rearrange("p t (h f) -> p t h f", h=H),
                    in1=s_slab[g][:].unsqueeze(3).to_broadcast([P, ntp, 8, 8]),
                    op=OP.mult)
                # elu (batched)
                t1 = pbs.tile([P, ntp, FD], F32, tag="elu1_" + g, name="elu1_" + g)
                nc.vector.tensor_scalar_min(t1[:], zs[g][:], 0.0)
                t2 = pbs.tile([P, ntp, FD], F32, tag="elu2_" + g, name="elu2_" + g)
                nc.scalar.activation(out=t2[:], in_=t1[:], func=AF.Exp)
                nc.vector.tensor_tensor(out=zs[g][:], in0=zs[g][:], in1=t1[:],
                                        op=OP.subtract)
                nc.vector.scalar_tensor_tensor(out=zs[g][:], in0=t2[:], scalar=-1.0,
                                               in1=zs[g][:], op0=OP.add, op1=OP.add)

            zsT = {g: slab.tile([FD, NTP[g] * P], F32, tag="zsT_" + g, name="zsT_" + g)
                   for g in GRAPHS}

            def do_transposes(g, pcp):
                for t in range(NTP[g]):
                    tp = pcp.tile([FD, P], F32, space="PSUM", tag="tp_ps", name="tp_ps")
                    nc.tensor.transpose(out=tp[:], in_=zs[g][:, t, :], identity=ident[:])
                    nc.scalar.copy(zsT[g][:, t * P:(t + 1) * P], tp[:])

            with tc.tile_pool(name="pB", bufs=3) as pb, \
                 tc.tile_pool(name="pBs", bufs=2) as pbs, \
                 tc.tile_pool(name="pB_ps", bufs=2, space="PSUM") as pzp:
                do_graph("ex0", pb, pbs, pzp)
                do_graph("ex1", pb, pbs, pzp)

            with tc.tile_pool(name="pC_ps", bufs=4, space="PSUM") as pcp:
                do_transposes("ex0", pcp)
                do_transposes("ex1", pcp)

            # ---- semantic attention stats over the exercise share ----
            stats = cst.tile([1, 16], F32, tag="stats", name="stats")
            nc.vector.memset(stats[:], 0.0)
            with tc.tile_pool(name="pD", bufs=2) as pd, \
                 tc.tile_pool(name="pD_ps", bufs=4, space="PSUM") as pdp:
                parts = cst.tile([1, 16], F32, tag="parts", name="parts")
                for mi, g in enumerate(("ex0", "ex1")):
                    cw_list = []
                    lo = 0
                    while lo < SH:
                        cw = min(512, SH - lo)
                        cw_list.append((lo, cw))
                        lo += cw
                    for ci, (lo, cw) in enumerate(cw_list):
                        tps = pdp.tile([SEM, 512], F32, space="PSUM", tag="tps", name="tps")
                        nc.tensor.matmul(tps[:, 0:cw], lhsT=semW[:], rhs=zsT[g][:, lo:lo + cw])
                        tsb = pd.tile([SEM, 512], F32, tag="tsb", name="tsb")
                        nc.scalar.activation(out=tsb[:, 0:cw], in_=tps[:, 0:cw],
                                             func=AF.Tanh, bias=semb_col[:])
                        rps = pdp.tile([1, 512], F32, space="PSUM", tag="rps", name="rps")
                        nc.tensor.matmul(rps[:, 0:cw], lhsT=semq_col[:], rhs=tsb[:, 0:cw])
                        nc.vector.tensor_reduce(out=parts[:, mi * 8 + ci:mi * 8 + ci + 1],
                                                in_=rps[:, 0:cw], axis=AX.X, op=OP.add)
                    nc.vector.tensor_reduce(
                        out=stats[:, mi:mi + 1],
                        in_=parts[:, mi * 8:mi * 8 + len(cw_list)], axis=AX.X, op=OP.add)

            # ---- AllReduce the 2 stats scalars (overlapped with st/kn) ----
            nc.sync.dma_start(cc_in[:, 0:16], stats[:])
            nc.gpsimd.collective_compute(
                "AllReduce", OP.add,
                replica_groups=[list(range(NC))],
                ins=[cc_in[:, :]], outs=[cc_out[:, :]])

            with tc.tile_pool(name="pB2", bufs=3) as pb, \
                 tc.tile_pool(name="pB2s", bufs=2) as pbs, \
                 tc.tile_pool(name="pB2_ps", bufs=2, space="PSUM") as pzp:
                do_graph("st", pb, pbs, pzp)
                do_graph("kn", pb, pbs, pzp)

            with tc.tile_pool(name="pC2_ps", bufs=4, space="PSUM") as pcp:
                do_transposes("st", pcp)
                do_transposes("kn", pcp)

            gstats = cst.tile([1, 16], F32, tag="gstats", name="gstats")
            nc.sync.dma_start(gstats[:], cc_out[:, :])

            # ---- predictor prep ----
            beta_col = cst.tile([P, 2], F32, tag="beta_col", name="beta_col")
            bd = cst.tile([1, 2], F32, tag="bd", name="bd")
            nc.vector.tensor_tensor(out=bd[:, 0:1], in0=gstats[:, 0:1],
                                    in1=gstats[:, 1:2], op=OP.subtract)
            btmp = cst.tile([1, 2], F32, tag="btmp", name="btmp")
            nc.scalar.activation(out=btmp[:, 0:1], in_=bd[:, 0:1], func=AF.Sigmoid,
                                 scale=1.0 / E_N)
            nc.scalar.activation(out=btmp[:, 1:2], in_=bd[:, 0:1], func=AF.Sigmoid,
                                 scale=-1.0 / E_N)
            b3_col = cst.tile([P, 1], F32, tag="b3_col", name="b3_col")
            with tc.tile_pool(name="bc2_ps", bufs=2, space="PSUM") as bc2:
                bb_ps = bc2.tile([P, 4], F32, space="PSUM", tag="bb_ps", name="bb_ps")
                nc.tensor.matmul(bb_ps[:, 0:2], lhsT=ones_row[:], rhs=btmp[:])
                nc.tensor.matmul(bb_ps[:, 2:3], lhsT=ones_row[:], rhs=b3[:])
                nc.vector.tensor_copy(beta_col[:], bb_ps[:, 0:2])
                nc.vector.tensor_copy(b3_col[:], bb_ps[:, 2:3])

            # fused exercise b-slot features: zsFT = b0*zsT_ex0 + b1*zsT_ex1
            zsFT = cst.tile([FD, BC], F32, tag="zsFT", name="zsFT")
            bcol = SH_TILES * P
            nc.vector.tensor_scalar(out=zsFT[:], in0=zsT["ex0"][:, bcol:bcol + BC],
                                    scalar1=beta_col[0:FD, 0:1], scalar2=None,
                                    op0=OP.mult)
            nc.vector.scalar_tensor_tensor(out=zsFT[:], in0=zsT["ex1"][:, bcol:bcol + BC],
                                           scalar=beta_col[0:FD, 1:2], in1=zsFT[:],
                                           op0=OP.mult, op1=OP.add)

            q1jk = cst.tile([P, K], F16, tag="q1jk", name="q1jk")      # Q1 [j, k]
            st_sb = cst.tile([P, K], F16, tag="st_sb", name="st_sb")   # Q2T [k, j]
            m1_sb = cst.tile([FD, K], F32, tag="m1_sb", name="m1_sb")
            m2_sb = cst.tile([FD, K], F32, tag="m2_sb", name="m2_sb")
            c1t = cst.tile([P, 1], F32, tag="c1t", name="c1t")
            c2t = cst.tile([P, 1], F32, tag="c2t", name="c2t")
            kn1T = cst.tile([P, K], F32, tag="kn1T", name="kn1T")
            U1bk = cst.tile([P, 2, K], F16, tag="U1bk", name="U1bk")   # U1 [b, k]
            U2T = cst.tile([P, BC], F16, tag="U2T", name="U2T")
            with tc.tile_pool(name="pF_ps", bufs=2, space="PSUM") as pfp:
                kn1_ps = pfp.tile([P, K], F32, space="PSUM", tag="prep_ps", name="kn1_ps")
                nc.tensor.matmul(kn1_ps[:], lhsT=zsT["kn"][:], rhs=pW_kn[:],
                                 start=True, stop=False)
                nc.tensor.matmul(kn1_ps[:], lhsT=ones_row[:], rhs=pb_kn_row[:],
                                 start=False, stop=True)
                kn1_sb = cst.tile([P, K], F32, tag="kn1_sb", name="kn1_sb")
                nc.scalar.copy(kn1_sb[:], kn1_ps[:])
                kn1T_ps = pfp.tile([P, K], F32, space="PSUM", tag="prep_ps", name="kn1T_ps")
                nc.tensor.transpose(out=kn1T_ps[:], in_=kn1_sb[:], identity=ident[:])
                nc.scalar.copy(kn1T[:], kn1T_ps[:])

                qs_ps = pfp.tile([P, K], F32, space="PSUM", tag="prep_ps", name="qs_ps")
                nc.tensor.matmul(qs_ps[:], lhsT=kn1T[:], rhs=W1b[:])
                nc.scalar.copy(q1jk[:], qs_ps[:])
                qs2_ps = pfp.tile([P, K], F32, space="PSUM", tag="prep_ps", name="qs2_ps")
                nc.tensor.matmul(qs2_ps[:], lhsT=W2b[:], rhs=kn1T[:])
                nc.scalar.copy(st_sb[:], qs2_ps[:])

                m1_ps = pfp.tile([FD, K], F32, space="PSUM", tag="prep_ps", name="m1_ps")
                nc.tensor.matmul(m1_ps[:], lhsT=pWT_st[:], rhs=W1a[:])
                nc.scalar.copy(m1_sb[:], m1_ps[:])
                m2_ps = pfp.tile([FD, K], F32, space="PSUM", tag="prep_ps", name="m2_ps")
                nc.tensor.matmul(m2_ps[:], lhsT=pWT_ex[:], rhs=W2a[:])
                nc.scalar.copy(m2_sb[:], m2_ps[:])
                c1_ps = pfp.tile([P, 1], F32, space="PSUM", tag="prep_ps", name="c1_ps")
                nc.tensor.matmul(c1_ps[:], lhsT=W1a[:], rhs=pb_st[:])
                nc.vector.tensor_copy(c1t[:], c1_ps[:])
                c2_ps = pfp.tile([P, 1], F32, space="PSUM", tag="prep_ps", name="c2_ps")
                nc.tensor.matmul(c2_ps[:], lhsT=W2a[:], rhs=pb_ex[:])
                nc.vector.tensor_copy(c2t[:], c2_ps[:])

                for bh in range(2):
                    u1_ps = pfp.tile([P, K], F32, space="PSUM", tag="u_ps", name="u1_ps")
                    nc.tensor.matmul(u1_ps[:], lhsT=zsT["st"][:, bh * P:(bh + 1) * P],
                                     rhs=m1_sb[:])
                    nc.vector.tensor_copy(U1bk[:, bh, :], u1_ps[:])
                u2_ps = pfp.tile([P, BC], F32, space="PSUM", tag="u_ps", name="u2_ps")
                nc.tensor.matmul(u2_ps[:], lhsT=m2_sb[:], rhs=zsFT[:])
                nc.vector.tensor_copy(U2T[:], u2_ps[:])

            # ---- predictor main loop ----
            # pref[b,j,k] = sig(U1[b,k] + Q1[j,k] + c1[k]); layout [k, (g,j)]
            with tc.tile_pool(name="pG", bufs=3) as pg, \
                 tc.tile_pool(name="pG_ps", bufs=2, space="PSUM") as pgp, \
                 tc.tile_pool(name="pO_ps", bufs=1, space="PSUM") as pop:
                o_ps = pop.tile([P, BC], F32, space="PSUM", tag="o_ps", name="o_ps")
                for grp in range(BC // FG):
                    b0 = grp * FG
                    # pref on PE: out[k,(g,j)] = Q1[j,k] (dj) + U1[b0+g,k] (dg)
                    pr_ps = pgp.tile([P, FG, K], F32, space="PSUM", tag="pr_ps",
                                     name="pr_ps")
                    nc.tensor.matmul(
                        pr_ps[:], lhsT=q1jk[:],
                        rhs=identh[:].unsqueeze(1).to_broadcast([P, FG, K]),
                        start=True, stop=False)
                    nc.tensor.matmul(
                        pr_ps[:], lhsT=U1bk[:, b0 // P, :],
                        rhs=identh[:, b0 % P:b0 % P + FG].unsqueeze(2)
                        .to_broadcast([P, FG, K]),
                        start=False, stop=True)
                    pr_sb = pg.tile([P, FG, K], F16, tag="pr_sb", name="pr_sb")
                    nc.scalar.activation(out=pr_sb[:], in_=pr_ps[:], func=AF.Sigmoid,
                                         bias=c1t[:])
                    df_lin = pg.tile([P, FG, K], F16, tag="df_lin", name="df_lin")
                    nc.vector.tensor_tensor(
                        out=df_lin[:],
                        in0=st_sb[:].unsqueeze(1).to_broadcast([P, FG, K]),
                        in1=U2T[:, b0:b0 + FG].unsqueeze(2).to_broadcast([P, FG, K]),
                        op=OP.add)
                    df_sb = pg.tile([P, FG, K], F16, tag="df_sb", name="df_sb")
                    nc.scalar.activation(out=df_sb[:], in_=df_lin[:], func=AF.Sigmoid,
                                         bias=c2t[:])
                    d_sb = pg.tile([P, FG, K], F16, tag="d_sb", name="d_sb")
                    nc.vector.tensor_tensor(out=d_sb[:], in0=pr_sb[:], in1=df_sb[:],
                                            op=OP.subtract)
                    for lb in range(FG):
                        nc.tensor.matmul(o_ps[:, b0 + lb:b0 + lb + 1],
                                         lhsT=d_sb[:, lb, :], rhs=W3h[:])

                # ---- final ----
                with tc.tile_pool(name="pN_ps", bufs=1, space="PSUM") as pnp:
                    o_sb = pg.tile([P, BC], F32, tag="o_sb", name="o_sb")
                    nc.scalar.activation(out=o_sb[:], in_=o_ps[:], func=AF.Sigmoid,
                                         bias=b3_col[:])
                    om = pg.tile([P, BC], F32, tag="om", name="om")
                    nc.vector.tensor_tensor(out=om[:], in0=o_sb[:], in1=kn_rT[:], op=OP.mult)
                    nd_ps = pnp.tile([1, 2 * BC], F32, space="PSUM", tag="nd_ps", name="nd_ps")
                    nc.tensor.matmul(nd_ps[:, 0:BC], lhsT=ones_col[:], rhs=om[:])
                    nc.tensor.matmul(nd_ps[:, BC:2 * BC], lhsT=ones_col[:], rhs=kn_rT[:])
                    rcp = pg.tile([1, BC], F32, tag="rcp", name="rcp")
                    nc.vector.reciprocal(rcp[:], nd_ps[:, BC:2 * BC])
                    res = pg.tile([1, BC], F32, tag="res", name="res")
                    nc.vector.tensor_tensor(out=res[:], in0=nd_ps[:, 0:BC], in1=rcp[:],
                                            op=OP.mult)
                    nc.sync.dma_start(out_d[:], res[:])

    nc.compile()
    return nc


# ----------------------------------------------------------------------------
# Entry point
# ----------------------------------------------------------------------------

_TRACE = bool(int(os.environ.get("KERNEL_TRACE", "0")))


def kernel(**inputs):
    meta, in_maps = preprocess(inputs)
    nc = build_program(meta)
    res = bass_utils.run_bass_kernel_spmd(
        nc, in_maps, core_ids=list(range(NC)), trace=_TRACE)
    out = np.concatenate([r["out"].reshape(-1) for r in res.results])
    kernel.last_results = res
    return out.reshape(B, 1).astype(np.float32)


# revision 19
# speedup vs baseline: 6.4074x; 1.0142x over previous
"""Trainium2 Bass kernel for the HAN-based cognitive-diagnosis net.

Strategy (8 NeuronCores, SPMD — one program, per-core data):
  * Batch (2048) split 8x256 across cores. Each core computes GAT outputs
    for its own batch-slice node list plus a 1/8 share of all exercise
    nodes (for the global-mean semantic attention stats, AllReduce'd).
  * Edge phase without any device gather: the edge lists are known on the
    host, so the host pre-expands x^T into the ELL slot layout (one
    x-column per edge slot, zero column for pad slots).  The device
    computes per-edge [z(64) | el(8)] directly with PE matmuls
    (lhsT = xts slot tile, rhs = W folded with a_l), then runs the
    edge softmax + weighted aggregation on DVE/ACT with dst nodes on
    partitions.  Pad slots have z=0 (exact numerator); the softmax
    denominator is corrected analytically:
        s_real = s - npad[d] * exp(leaky(er[d]) - m[d])
    since every pad slot contributes exactly that one value.
  * Predictor exploits the rank structure pref[b,j,k] =
    sigmoid(U1[b,k] + Q1[j,k] + c1[k]): two small matmuls build U/Q once,
    the [B,K,K] tensors are pure broadcast-add + sigmoid, and the W3
    contraction runs on PE per batch row.
"""

import os
import numpy as np

import concourse.bass as bass
import concourse.bacc as bacc
import concourse.mybir as mybir
import concourse.tile as tile
from concourse import library_config
from concourse.masks import make_identity
from concourse import bass_utils

F32 = mybir.dt.float32
F16 = mybir.dt.float16

NC = 8
B = 2048
BC = B // NC          # 256 batch rows per core
K = 128
H, D, FD = 8, 8, 64
SEM = 128
S_N, E_N = 10000, 20000
P = 128

AX = mybir.AxisListType
OP = mybir.AluOpType
AF = mybir.ActivationFunctionType

GRAPHS = ("ex0", "ex1", "st", "kn")


# ----------------------------------------------------------------------------
# Host-side preprocessing (integer / layout only)
# ----------------------------------------------------------------------------

def _csr_by_dst(src, dst, n):
    order = np.argsort(dst, kind="stable")
    ss = src[order].astype(np.int64)
    counts = np.bincount(dst, minlength=n)
    rowptr = np.zeros(n + 1, np.int64)
    np.cumsum(counts, out=rowptr[1:])
    return ss, rowptr, counts


def _tiles_of(nodes):
    return [np.asarray(nodes[i:i + P]) for i in range(0, len(nodes), P)]


def _flat_src(node_tiles, dts, ss, rowptr, counts, npad_row):
    """Flat per-edge-slot src index list (tile-major, col-major, 128 lanes)
    with -1 for pad slots, plus the [128, ntiles] pad-count array."""
    nslot = int(np.sum(dts))
    flat = np.full((nslot, P), -1, np.int64)
    npad = np.zeros((P, len(node_tiles)), np.float32)
    col = 0
    for t, nodes in enumerate(node_tiles):
        dt = int(dts[t])
        for pi, node in enumerate(nodes):
            deg = int(counts[node])
            if deg:
                lo = rowptr[node]
                flat[col:col + deg, pi] = ss[lo:lo + deg]
            npad[pi, t] = -(dt - deg)
        for pi in range(len(nodes), P):
            npad[pi, t] = -dt
        col += dt
    assert col == nslot
    return flat.reshape(-1), npad


def _xts(x_aug_T, flat):
    """[K, nslot*128] fp16 ELL-expanded x^T (zero column for pad slots)."""
    return np.ascontiguousarray(x_aug_T[:, flat])


def _xtp(x, node_tiles, ntiles):
    """x^T columns for a node list, padded to ntiles*128 cols, fp16."""
    kdim = x.shape[1]
    out = np.zeros((kdim, ntiles * P), np.float16)
    for t, nodes in enumerate(node_tiles):
        out[:, t * P:t * P + len(nodes)] = x[nodes].T.astype(np.float16)
    return out


def preprocess(inputs):
    inp = {k: np.asarray(v) for k, v in inputs.items()}
    stu_id = inp["stu_id"].astype(np.int64)
    exer_id = inp["exer_id"].astype(np.int64)

    csr = {
        "st": _csr_by_dst(inp["ss0"].astype(np.int64), inp["sd0"].astype(np.int64), S_N),
        "ex0": _csr_by_dst(inp["es0"].astype(np.int64), inp["ed0"].astype(np.int64), E_N),
        "ex1": _csr_by_dst(inp["es1"].astype(np.int64), inp["ed1"].astype(np.int64), E_N),
        "kn": _csr_by_dst(inp["ks0"].astype(np.int64), inp["kd0"].astype(np.int64), K),
    }

    # per-core node tile lists; exercise share nodes degree-sorted + strided
    SH = E_N // NC                      # 2500
    SH_TILES = (SH + P - 1) // P        # 20
    BS_TILES = BC // P                  # 2
    NTP = {"ex0": SH_TILES + BS_TILES, "ex1": SH_TILES + BS_TILES,
           "st": BS_TILES, "kn": 1}

    tiles = {g: [] for g in GRAPHS}     # g -> [core][tile] node arrays
    for g in ("ex0", "ex1"):
        order = np.argsort(-csr[g][2], kind="stable")
        for c in range(NC):
            share = order[c::NC]
            assert len(share) == SH
            tiles[g].append(_tiles_of(share) +
                            _tiles_of(exer_id[c * BC:(c + 1) * BC]))
    for c in range(NC):
        tiles["st"].append(_tiles_of(stu_id[c * BC:(c + 1) * BC]))
        tiles["kn"].append(_tiles_of(np.arange(K)))

    # shared per-tile Dt = max over cores (SPMD: one program)
    dts = {}
    for g in GRAPHS:
        counts = csr[g][2]
        dts[g] = np.max(
            [[max(1, int(counts[t].max()) if len(t) else 1) for t in tiles[g][c]]
             for c in range(NC)], axis=0)

    meta = dict(dts=dts, NTP=NTP, SH=SH, SH_TILES=SH_TILES, BS_TILES=BS_TILES,
                nslot={g: int(dts[g].sum()) for g in GRAPHS})

    # ------- shared input arrays -------
    shared = {
        "w_ex0": inp["f3W0"].astype(np.float16),
        "w_ex1": inp["f3W1"].astype(np.float16),
        "w_st": inp["f1W0"].astype(np.float16),
        "w_kn": inp["f5W0"].astype(np.float16),
        "alr_ex0": np.concatenate([inp["f3al0"].reshape(1, 64), inp["f3ar0"].reshape(1, 64)], 1),
        "alr_ex1": np.concatenate([inp["f3al1"].reshape(1, 64), inp["f3ar1"].reshape(1, 64)], 1),
        "alr_st": np.concatenate([inp["f1al0"].reshape(1, 64), inp["f1ar0"].reshape(1, 64)], 1),
        "alr_kn": np.concatenate([inp["f5al0"].reshape(1, 64), inp["f5ar0"].reshape(1, 64)], 1),
        "semW": inp["f3sW"].astype(np.float32),
        "semb_col": inp["f3sb"].reshape(SEM, 1).astype(np.float32),
        "semq_col": inp["f3sq"].reshape(SEM, 1).astype(np.float32),
        "pWT_st": inp["f1pW"].T.astype(np.float32).copy(),
        "pb_st": inp["f1pb"].reshape(K, 1).astype(np.float32),
        "pWT_ex": inp["f3pW"].T.astype(np.float32).copy(),
        "pb_ex": inp["f3pb"].reshape(K, 1).astype(np.float32),
        "pW_kn": inp["f5pW"].astype(np.float32),
        "pb_kn_row": inp["f5pb"].reshape(1, K).astype(np.float32),
        "W1a": inp["W1"][:K].astype(np.float32),
        "W1b": inp["W1"][K:].astype(np.float32),
        "W2a": inp["W2"][:K].astype(np.float32),
        "W2b": inp["W2"][K:].astype(np.float32),
        "W3h": inp["W3"].astype(np.float16),
        "b3": inp["b3"].reshape(1, 1).astype(np.float32),
    }

    xsrc = {"ex0": inp["exer_t"], "ex1": inp["exer_t"],
            "st": inp["stu_t"], "kn": inp["kn_t"]}
    x_aug_T = {}
    for g in GRAPHS:
        xa = np.vstack([xsrc[g], np.zeros((1, K), np.float32)]).astype(np.float16)
        x_aug_T[g] = np.ascontiguousarray(xa.T)   # [K, N+1], col N = zeros

    # ------- per-core arrays -------
    in_maps = []
    for c in range(NC):
        m = dict(shared)
        for g in GRAPHS:
            ss, rowptr, counts = csr[g]
            flat, npad = _flat_src(tiles[g][c], dts[g], ss, rowptr, counts, None)
            flat = np.where(flat < 0, xsrc[g].shape[0], flat)
            m["xts_" + g] = _xts(x_aug_T[g], flat)
            m["npad_" + g] = npad
            m["xtp_" + g] = _xtp(xsrc[g], tiles[g][c], NTP[g])
        m["kn_rT"] = inp["kn_r"][c * BC:(c + 1) * BC].T.astype(np.float32).copy()
        in_maps.append(m)

    return meta, in_maps


# ----------------------------------------------------------------------------
# Bass program
# ----------------------------------------------------------------------------

PS_COLS = 24     # slot-cols per PSUM unit: z [P,24,64] (3 banks) + el [P,24,8]
FG = 4           # predictor batch rows per group


def build_program(meta):
    nc = bacc.Bacc("TRN2", num_devices=NC)
    dts = meta["dts"]
    NTP = meta["NTP"]
    SH = meta["SH"]
    SH_TILES, BS_TILES = meta["SH_TILES"], meta["BS_TILES"]
    nslot = meta["nslot"]
    MAXDT = -(-max(int(dts[g].max()) for g in GRAPHS) // 8) * 8

    ein = {}
    def EIN(name, shape, dt):
        ein[name] = nc.dram_tensor(name, list(shape), dt, kind="ExternalInput")
        return ein[name]

    for g in GRAPHS:
        EIN("w_" + g, (K, FD), F16)
        EIN("alr_" + g, (1, 128), F32)
        EIN("xts_" + g, (K, nslot[g] * P), F16)
        EIN("npad_" + g, (P, len(dts[g])), F32)
        EIN("xtp_" + g, (K, NTP[g] * P), F16)
    EIN("semW", (FD, SEM), F32); EIN("semb_col", (SEM, 1), F32); EIN("semq_col", (SEM, 1), F32)
    EIN("pWT_st", (K, FD), F32); EIN("pb_st", (K, 1), F32)
    EIN("pWT_ex", (K, FD), F32); EIN("pb_ex", (K, 1), F32)
    EIN("pW_kn", (FD, K), F32); EIN("pb_kn_row", (1, K), F32)
    EIN("W1a", (K, K), F32); EIN("W1b", (K, K), F32)
    EIN("W2a", (K, K), F32); EIN("W2b", (K, K), F32)
    EIN("W3h", (K, 1), F16); EIN("b3", (1, 1), F32)
    EIN("kn_rT", (K, BC), F32)

    out_d = nc.dram_tensor("out", [1, BC], F32, kind="ExternalOutput")

    cc_in = nc.dram_tensor("cc_in", [1, 16], F32, kind="Internal")
    cc_out = nc.dram_tensor("cc_out", [1, 16], F32, kind="Internal", addr_space="Shared")

    with tile.TileContext(nc) as tc:
        with tc.tile_pool(name="const", bufs=1) as cst, \
             tc.tile_pool(name="slab", bufs=1) as slab:
            nc.gpsimd.load_library(library_config.mlp)

            ident = cst.tile([P, P], F32, tag="ident", name="ident")
            make_identity(nc, ident[:])
            identh = cst.tile([P, P], F16, tag="identh", name="identh")
            nc.vector.tensor_copy(identh[:], ident[:])
            ones_col = cst.tile([P, 1], F32, tag="ones_col", name="ones_col")
            nc.vector.memset(ones_col[:], 1.0)
            ones_row = cst.tile([1, P], F32, tag="ones_row", name="ones_row")
            nc.vector.memset(ones_row[:], 1.0)

            # ---- load small weights ----
            def load(name, shape, dt):
                t = cst.tile(list(shape), dt, tag="ld_" + name, name="ld_" + name)
                nc.sync.dma_start(t[:], ein[name][:])
                return t
            w_g = {g: load("w_" + g, (K, FD), F16) for g in GRAPHS}
            alr = {g: load("alr_" + g, (1, 128), F32) for g in GRAPHS}
            npad_sb = {g: load("npad_" + g, (P, len(dts[g])), F32) for g in GRAPHS}
            semW = load("semW", (FD, SEM), F32)
            semb_col = load("semb_col", (SEM, 1), F32)
            semq_col = load("semq_col", (SEM, 1), F32)
            pWT_st = load("pWT_st", (K, FD), F32); pb_st = load("pb_st", (K, 1), F32)
            pWT_ex = load("pWT_ex", (K, FD), F32); pb_ex = load("pb_ex", (K, 1), F32)
            pW_kn = load("pW_kn", (FD, K), F32); pb_kn_row = load("pb_kn_row", (1, K), F32)
            W1a = load("W1a", (K, K), F32); W1b = load("W1b", (K, K), F32)
            W2a = load("W2a", (K, K), F32); W2b = load("W2b", (K, K), F32)
            W3h = load("W3h", (K, 1), F16); b3 = load("b3", (1, 1), F32)
            kn_rT = load("kn_rT", (K, BC), F32)

            # ---- fold al/ar into W: wcat = [W | W@al] fp16, war = W@ar ----
            wcat = {}   # [128, 72] f16: cols 0:64 W, 64:72 Wal
            war = {}    # [128, 8] f16
            with tc.tile_pool(name="bc_ps", bufs=2, space="PSUM") as bcp:
              for g in GRAPHS:
                alb = cst.tile([P, 128], F32, tag="alb", name="alb")
                alb_ps = bcp.tile([P, 128], F32, space="PSUM", tag="alb_ps", name="alb_ps")
                nc.tensor.matmul(alb_ps[:], lhsT=ones_row[:], rhs=alr[g][:])
                nc.vector.tensor_copy(alb[:], alb_ps[:])
                wf = cst.tile([P, FD], F32, tag="wf", name="wf")
                nc.vector.tensor_copy(wf[:], w_g[g][:])
                wtmp = cst.tile([P, FD], F32, tag="wtmp", name="wtmp")
                wc = cst.tile([P, 72], F16, tag="wcat_" + g, name="wcat_" + g)
                wcat[g] = wc
                nc.vector.tensor_copy(wc[:, 0:64], w_g[g][:])
                with nc.allow_low_precision(reason="8-elem head fold of fp16 weights"):
                    nc.vector.tensor_tensor(out=wtmp[:], in0=wf[:], in1=alb[:, 0:64], op=OP.mult)
                    nc.vector.tensor_reduce(out=wc[:, 64:72].bitcast(F16),
                                            in_=wtmp[:].rearrange("p (h f) -> p h f", h=H),
                                            axis=AX.X, op=OP.add)
                    wr = cst.tile([P, 8], F16, tag="war_" + g, name="war_" + g)
                    war[g] = wr
                    nc.vector.tensor_tensor(out=wtmp[:], in0=wf[:], in1=alb[:, 64:128], op=OP.mult)
                    nc.vector.tensor_reduce(out=wr[:], in_=wtmp[:].rearrange("p (h f) -> p h f", h=H),
                                            axis=AX.X, op=OP.add)

            # ---- er per (graph, tile): er[d, t, h] = (x[d] @ war)[h] ----
            er = {}
            with tc.tile_pool(name="pE", bufs=2) as pe, \
                 tc.tile_pool(name="pE_ps", bufs=4, space="PSUM") as pep:
                for g in GRAPHS:
                    ntp = NTP[g]
                    er_sb = slab.tile([P, ntp, 8], F32, tag="er_" + g, name="er_" + g)
                    er[g] = er_sb
                    xtp_sb = pe.tile([P, NTP["ex0"] * P], F16, tag="xtp_sb", name="xtp_sb")
                    nc.sync.dma_start(xtp_sb[:, 0:ntp * P], ein["xtp_" + g][:])
                    for t in range(ntp):
                        eps = pep.tile([P, 8], F32, space="PSUM", tag="eps", name="eps")
                        nc.tensor.matmul(eps[:], lhsT=xtp_sb[:, t * P:(t + 1) * P],
                                         rhs=war[g][:])
                        nc.vector.tensor_copy(er_sb[:, t, :], eps[:])

            # ---- Phase B: per-edge z via PE + edge softmax + aggregation ----
            zs = {g: slab.tile([P, NTP[g], FD], F32, tag="zs_" + g, name="zs_" + g)
                  for g in GRAPHS}
            s_slab = {g: slab.tile([P, NTP[g], 8], F32, tag="s_" + g, name="s_" + g)
                      for g in GRAPHS}

            def do_graph(g, pb, pbs, pzp):
                """Emit z matmuls + softmax for every tile of graph g.

                Small tiles (Dt <= PS_COLS): z and el stay resident in PSUM;
                the e-pipe reads el from PSUM and the weighted mult reads z
                from PSUM (no evacuation copies).  Big tiles copy per unit.
                """
                col0 = 0
                for t in range(NTP[g]):
                    Dt = int(dts[g][t])
                    DtP = -(-Dt // 8) * 8
                    small = Dt <= PS_COLS
                    z_el = pbs.tile([P, MAXDT, 64], F16, tag="z_el", name="z_el")
                    if not small:
                        el_t = pbs.tile([P, 8, MAXDT], F32, tag="el_t", name="el_t")
                    zp_keep = None
                    elp_keep = None
                    for lo in range(0, Dt, PS_COLS):
                        n_here = min(PS_COLS, Dt - lo)
                        xts_sb = pb.tile([P, PS_COLS * P], F16, tag="xts_sb", name="xts_sb")
                        nc.sync.dma_start(
                            xts_sb[:, 0:n_here * P],
                            ein["xts_" + g][:, (col0 + lo) * P:(col0 + lo + n_here) * P])
                        zp = pzp.tile([P, PS_COLS, 64], F32, space="PSUM",
                                      tag="zp", name="zp")
                        elp = pzp.tile([P, PS_COLS, 8], F32, space="PSUM",
                                       tag="elp", name="elp")
                        for ci in range(n_here):
                            lhsT = xts_sb[:, ci * P:(ci + 1) * P]
                            nc.tensor.matmul(zp[:, ci, :], lhsT=lhsT, rhs=wcat[g][:, 0:64])
                            nc.tensor.matmul(elp[:, ci, :], lhsT=lhsT, rhs=wcat[g][:, 64:72])
                        if small:
                            zp_keep, elp_keep = zp, elp
                        else:
                            nc.scalar.activation(out=z_el[:, lo:lo + n_here, :],
                                                 in_=zp[:, 0:n_here, :], func=AF.Copy)
                            nc.scalar.activation(
                                out=el_t[:].transpose([0, 2, 1])[:, lo:lo + n_here, :],
                                in_=elp[:, 0:n_here, :], func=AF.Copy)
                    # ---- edge softmax over the Dt slots (head-major e) ----
                    e = pbs.tile([P, 8, MAXDT], F32, tag="e_buf", name="e_buf")
                    if small:
                        nc.vector.tensor_tensor(
                            out=e[:].transpose([0, 2, 1])[:, 0:Dt, :],
                            in0=elp_keep[:, 0:Dt, :],
                            in1=er[g][:, t, :].unsqueeze(1).to_broadcast([P, Dt, 8]),
                            op=OP.add)
                    else:
                        nc.vector.tensor_tensor(
                            out=e[:, :, 0:Dt], in0=el_t[:, :, 0:Dt],
                            in1=er[g][:, t, :].unsqueeze(2).to_broadcast([P, 8, Dt]),
                            op=OP.add)
                    nc.vector.scalar_tensor_tensor(out=e[:, :, 0:Dt], in0=e[:, :, 0:Dt],
                                                   scalar=0.2, in1=e[:, :, 0:Dt],
                                                   op0=OP.mult, op1=OP.max)
                    nc.scalar.activation(out=e[:, :, 0:Dt], in_=e[:, :, 0:Dt],
                                          func=AF.Exp)
                    s = s_slab[g][:, t, :]
                    nc.vector.tensor_reduce(out=s, in_=e[:, :, 0:Dt],
                                            axis=AX.X, op=OP.add)
                    # weighted w = z * exb into SBUF (z from PSUM for small tiles)
                    z_src = (zp_keep[:, 0:Dt, :] if small else z_el[:, 0:Dt, :])
                    nc.vector.tensor_tensor(
                        out=z_el[:, 0:Dt, :].rearrange("p s (h f) -> p s h f", h=H),
                        in0=z_src.rearrange("p s (h f) -> p s h f", h=H),
                        in1=e[:, :, 0:Dt].transpose([0, 2, 1]).unsqueeze(3)
                        .to_broadcast([P, Dt, 8, 8]),
                        op=OP.mult)
                    if DtP > Dt:
                        nc.gpsimd.memset(z_el[:, Dt:DtP, :], 0.0)
                    # 3 contiguous halvings then short strided tail reduce
                    cur = DtP
                    while cur > DtP // 8:
                        h2 = cur // 2
                        nc.vector.tensor_tensor(out=z_el[:, 0:h2, :],
                                                in0=z_el[:, 0:h2, :],
                                                in1=z_el[:, h2:cur, :], op=OP.add)
                        cur = h2
                    v = zs[g][:, t, :]
                    nc.vector.tensor_reduce(
                        out=v, in_=z_el[:, 0:cur, :].transpose([0, 2, 1]),
                        axis=AX.X, op=OP.add)
                    col0 += Dt
                # ---- batched per-graph epilogue: pad fix, normalize, elu ----
                ntp = NTP[g]
                epb = pbs.tile([P, ntp, 8], F32, tag="epb_" + g, name="epb_" + g)
                nc.vector.scalar_tensor_tensor(out=epb[:], in0=er[g][:],
                                               scalar=0.2, in1=er[g][:],
                                               op0=OP.mult, op1=OP.max)
                nc.scalar.activation(out=epb[:], in_=epb[:], func=AF.Exp)
                nc.vector.tensor_tensor(
                    out=epb[:], in0=epb[:],
                    in1=npad_sb[g][:].unsqueeze(2).to_broadcast([P, ntp, 8]),
                    op=OP.mult)
                nc.vector.tensor_tensor(out=s_slab[g][:], in0=s_slab[g][:],
                                        in1=epb[:], op=OP.add)
                nc.vector.tensor_scalar_add(s_slab[g][:], s_slab[g][:], 1e-9)
                nc.vector.reciprocal(s_slab[g][:], s_slab[g][:])
                nc.vector.tensor_tensor(
                    out=zs[g][:].rearrange("p t (h f) -> p t h f", h=H),
                    in0=zs[g][:].rearrange("p t (h f) -> p t h f", h=H),
                    in1=s_slab[g][:].unsqueeze(3).to_broadcast([P, ntp, 8, 8]),
                    op=OP.mult)
                # elu (batched)
                t1 = pbs.tile([P, ntp, FD], F32, tag="elu1_" + g, name="elu1_" + g)
                nc.vector.tensor_scalar_min(t1[:], zs[g][:], 0.0)
                t2 = pbs.tile([P, ntp, FD], F32, tag="elu2_" + g, name="elu2_" + g)
                nc.scalar.activation(out=t2[:], in_=t1[:], func=AF.Exp)
                nc.vector.tensor_tensor(out=zs[g][:], in0=zs[g][:], in1=t1[:],
                                        op=OP.subtract)
                nc.vector.scalar_tensor_tensor(out=zs[g][:], in0=t2[:], scalar=-1.0,
                                               in1=zs[g][:], op0=OP.add, op1=OP.add)

            zsT = {g: slab.tile([FD, NTP[g] * P], F32, tag="zsT_" + g, name="zsT_" + g)
                   for g in GRAPHS}

            def do_transposes(g, pcp):
                for t in range(NTP[g]):
                    tp = pcp.tile([FD, P], F32, space="PSUM", tag="tp_ps", name="tp_ps")
                    nc.tensor.transpose(out=tp[:], in_=zs[g][:, t, :], identity=ident[:])
                    nc.scalar.copy(zsT[g][:, t * P:(t + 1) * P], tp[:])

            stats = cst.tile([1, 16], F32, tag="stats", name="stats")
            nc.vector.memset(stats[:], 0.0)
            with tc.tile_pool(name="pB", bufs=3) as pb, \
                 tc.tile_pool(name="pBs", bufs=2) as pbs:
                with tc.tile_pool(name="pB_ps", bufs=2, space="PSUM") as pzp:
                    do_graph("ex0", pb, pbs, pzp)
                    do_graph("ex1", pb, pbs, pzp)
                    do_graph("st", pb, pbs, pzp)
                    do_graph("kn", pb, pbs, pzp)

                with tc.tile_pool(name="pC_ps", bufs=4, space="PSUM") as pcp:
                    for g in GRAPHS:
                        do_transposes(g, pcp)

                # ---- semantic attention stats over the exercise share ----
                with tc.tile_pool(name="pD_ps", bufs=4, space="PSUM") as pdp:
                    parts = cst.tile([1, 16], F32, tag="parts", name="parts")
                    for mi, g in enumerate(("ex0", "ex1")):
                        cw_list = []
                        lo = 0
                        while lo < SH:
                            cw = min(512, SH - lo)
                            cw_list.append((lo, cw))
                            lo += cw
                        for ci, (lo, cw) in enumerate(cw_list):
                            tps = pdp.tile([SEM, 512], F32, space="PSUM", tag="tps", name="tps")
                            nc.tensor.matmul(tps[:, 0:cw], lhsT=semW[:], rhs=zsT[g][:, lo:lo + cw])
                            tsb = pbs.tile([SEM, 512], F32, tag="tsb", name="tsb")
                            nc.scalar.activation(out=tsb[:, 0:cw], in_=tps[:, 0:cw],
                                                 func=AF.Tanh, bias=semb_col[:])
                            rps = pdp.tile([1, 512], F32, space="PSUM", tag="rps", name="rps")
                            nc.tensor.matmul(rps[:, 0:cw], lhsT=semq_col[:], rhs=tsb[:, 0:cw])
                            nc.vector.tensor_reduce(out=parts[:, mi * 8 + ci:mi * 8 + ci + 1],
                                                    in_=rps[:, 0:cw], axis=AX.X, op=OP.add)
                        nc.vector.tensor_reduce(
                            out=stats[:, mi:mi + 1],
                            in_=parts[:, mi * 8:mi * 8 + len(cw_list)], axis=AX.X, op=OP.add)

            # ---- AllReduce the 2 stats scalars (overlapped with prep) ----
            nc.sync.dma_start(cc_in[:, 0:16], stats[:])
            nc.gpsimd.collective_compute(
                "AllReduce", OP.add,
                replica_groups=[list(range(NC))],
                ins=[cc_in[:, :]], outs=[cc_out[:, :]])

            # ---- predictor prep (beta-independent parts first; the gstats
            # read + beta + zsFT + U2T slot in after, hiding the AllReduce) ----
            q1jk = cst.tile([P, K], F16, tag="q1jk", name="q1jk")      # Q1 [j, k]
            st_sb = cst.tile([P, K], F16, tag="st_sb", name="st_sb")   # Q2T [k, j]
            m1_sb = cst.tile([FD, K], F32, tag="m1_sb", name="m1_sb")
            m2_sb = cst.tile([FD, K], F32, tag="m2_sb", name="m2_sb")
            c1t = cst.tile([P, 1], F32, tag="c1t", name="c1t")
            c2t = cst.tile([P, 1], F32, tag="c2t", name="c2t")
            kn1T = cst.tile([P, K], F32, tag="kn1T", name="kn1T")
            U1bk = cst.tile([P, 2, K], F16, tag="U1bk", name="U1bk")   # U1 [b, k]
            U2T = cst.tile([P, BC], F16, tag="U2T", name="U2T")
            with tc.tile_pool(name="pF_ps", bufs=2, space="PSUM") as pfp:
                kn1_ps = pfp.tile([P, K], F32, space="PSUM", tag="prep_ps", name="kn1_ps")
                nc.tensor.matmul(kn1_ps[:], lhsT=zsT["kn"][:], rhs=pW_kn[:],
                                 start=True, stop=False)
                nc.tensor.matmul(kn1_ps[:], lhsT=ones_row[:], rhs=pb_kn_row[:],
                                 start=False, stop=True)
                kn1_sb = cst.tile([P, K], F32, tag="kn1_sb", name="kn1_sb")
                nc.scalar.copy(kn1_sb[:], kn1_ps[:])
                kn1T_ps = pfp.tile([P, K], F32, space="PSUM", tag="prep_ps", name="kn1T_ps")
                nc.tensor.transpose(out=kn1T_ps[:], in_=kn1_sb[:], identity=ident[:])
                nc.scalar.copy(kn1T[:], kn1T_ps[:])

                qs_ps = pfp.tile([P, K], F32, space="PSUM", tag="prep_ps", name="qs_ps")
                nc.tensor.matmul(qs_ps[:], lhsT=kn1T[:], rhs=W1b[:])
                nc.scalar.copy(q1jk[:], qs_ps[:])
                qs2_ps = pfp.tile([P, K], F32, space="PSUM", tag="prep_ps", name="qs2_ps")
                nc.tensor.matmul(qs2_ps[:], lhsT=W2b[:], rhs=kn1T[:])
                nc.scalar.copy(st_sb[:], qs2_ps[:])

                m1_ps = pfp.tile([FD, K], F32, space="PSUM", tag="prep_ps", name="m1_ps")
                nc.tensor.matmul(m1_ps[:], lhsT=pWT_st[:], rhs=W1a[:])
                nc.scalar.copy(m1_sb[:], m1_ps[:])
                m2_ps = pfp.tile([FD, K], F32, space="PSUM", tag="prep_ps", name="m2_ps")
                nc.tensor.matmul(m2_ps[:], lhsT=pWT_ex[:], rhs=W2a[:])
                nc.scalar.copy(m2_sb[:], m2_ps[:])
                c1_ps = pfp.tile([P, 1], F32, space="PSUM", tag="prep_ps", name="c1_ps")
                nc.tensor.matmul(c1_ps[:], lhsT=W1a[:], rhs=pb_st[:])
                nc.vector.tensor_copy(c1t[:], c1_ps[:])
                c2_ps = pfp.tile([P, 1], F32, space="PSUM", tag="prep_ps", name="c2_ps")
                nc.tensor.matmul(c2_ps[:], lhsT=W2a[:], rhs=pb_ex[:])
                nc.vector.tensor_copy(c2t[:], c2_ps[:])

                for bh in range(2):
                    u1_ps = pfp.tile([P, K], F32, space="PSUM", tag="u_ps", name="u1_ps")
                    nc.tensor.matmul(u1_ps[:], lhsT=zsT["st"][:, bh * P:(bh + 1) * P],
                                     rhs=m1_sb[:])
                    nc.vector.tensor_copy(U1bk[:, bh, :], u1_ps[:])
                # collective result -> beta -> fused features -> U2T
                gstats = cst.tile([1, 16], F32, tag="gstats", name="gstats")
                nc.sync.dma_start(gstats[:], cc_out[:, :])
                beta_col = cst.tile([P, 2], F32, tag="beta_col", name="beta_col")
                bd = cst.tile([1, 2], F32, tag="bd", name="bd")
                nc.vector.tensor_tensor(out=bd[:, 0:1], in0=gstats[:, 0:1],
                                        in1=gstats[:, 1:2], op=OP.subtract)
                btmp = cst.tile([1, 2], F32, tag="btmp", name="btmp")
                nc.scalar.activation(out=btmp[:, 0:1], in_=bd[:, 0:1], func=AF.Sigmoid,
                                     scale=1.0 / E_N)
                nc.scalar.activation(out=btmp[:, 1:2], in_=bd[:, 0:1], func=AF.Sigmoid,
                                     scale=-1.0 / E_N)
                b3_col = cst.tile([P, 1], F32, tag="b3_col", name="b3_col")
                bb_ps = pfp.tile([P, 4], F32, space="PSUM", tag="bb_ps", name="bb_ps")
                nc.tensor.matmul(bb_ps[:, 0:2], lhsT=ones_row[:], rhs=btmp[:])
                nc.tensor.matmul(bb_ps[:, 2:3], lhsT=ones_row[:], rhs=b3[:])
                nc.vector.tensor_copy(beta_col[:], bb_ps[:, 0:2])
                nc.vector.tensor_copy(b3_col[:], bb_ps[:, 2:3])
                zsFT = cst.tile([FD, BC], F32, tag="zsFT", name="zsFT")
                bcol = SH_TILES * P
                nc.vector.tensor_scalar(out=zsFT[:], in0=zsT["ex0"][:, bcol:bcol + BC],
                                        scalar1=beta_col[0:FD, 0:1], scalar2=None,
                                        op0=OP.mult)
                nc.vector.scalar_tensor_tensor(out=zsFT[:], in0=zsT["ex1"][:, bcol:bcol + BC],
                                               scalar=beta_col[0:FD, 1:2], in1=zsFT[:],
                                               op0=OP.mult, op1=OP.add)
                u2_ps = pfp.tile([P, BC], F32, space="PSUM", tag="u_ps", name="u2_ps")
                nc.tensor.matmul(u2_ps[:], lhsT=m2_sb[:], rhs=zsFT[:])
                nc.vector.tensor_copy(U2T[:], u2_ps[:])

            # ---- predictor main loop ----
            # pref[b,j,k] = sig(U1[b,k] + Q1[j,k] + c1[k]); layout [k, (g,j)]
            with tc.tile_pool(name="pG", bufs=3) as pg, \
                 tc.tile_pool(name="pG_ps", bufs=2, space="PSUM") as pgp, \
                 tc.tile_pool(name="pO_ps", bufs=1, space="PSUM") as pop:
                o_ps = pop.tile([P, BC], F32, space="PSUM", tag="o_ps", name="o_ps")
                for grp in range(BC // FG):
                    b0 = grp * FG
                    # pref on PE: out[k,(g,j)] = Q1[j,k] (dj) + U1[b0+g,k] (dg)
                    pr_ps = pgp.tile([P, FG, K], F32, space="PSUM", tag="pr_ps",
                                     name="pr_ps")
                    nc.tensor.matmul(
                        pr_ps[:], lhsT=q1jk[:],
                        rhs=identh[:].unsqueeze(1).to_broadcast([P, FG, K]),
                        start=True, stop=False)
                    nc.tensor.matmul(
                        pr_ps[:], lhsT=U1bk[:, b0 // P, :],
                        rhs=identh[:, b0 % P:b0 % P + FG].unsqueeze(2)
                        .to_broadcast([P, FG, K]),
                        start=False, stop=True)
                    pr_sb = pg.tile([P, FG, K], F16, tag="pr_sb", name="pr_sb")
                    nc.scalar.activation(out=pr_sb[:], in_=pr_ps[:], func=AF.Sigmoid,
                                         bias=c1t[:])
                    df_lin = pg.tile([P, FG, K], F16, tag="df_lin", name="df_lin")
                    nc.vector.tensor_tensor(
                        out=df_lin[:],
                        in0=st_sb[:].unsqueeze(1).to_broadcast([P, FG, K]),
                        in1=U2T[:, b0:b0 + FG].unsqueeze(2).to_broadcast([P, FG, K]),
                        op=OP.add)
                    df_sb = pg.tile([P, FG, K], F16, tag="df_sb", name="df_sb")
                    nc.scalar.activation(out=df_sb[:], in_=df_lin[:], func=AF.Sigmoid,
                                         bias=c2t[:])
                    d_sb = pg.tile([P, FG, K], F16, tag="d_sb", name="d_sb")
                    nc.vector.tensor_tensor(out=d_sb[:], in0=pr_sb[:], in1=df_sb[:],
                                            op=OP.subtract)
                    for lb in range(FG):
                        nc.tensor.matmul(o_ps[:, b0 + lb:b0 + lb + 1],
                                         lhsT=d_sb[:, lb, :], rhs=W3h[:])

                # ---- final ----
                with tc.tile_pool(name="pN_ps", bufs=1, space="PSUM") as pnp:
                    o_sb = pg.tile([P, BC], F32, tag="o_sb", name="o_sb")
                    nc.scalar.activation(out=o_sb[:], in_=o_ps[:], func=AF.Sigmoid,
                                         bias=b3_col[:])
                    om = pg.tile([P, BC], F32, tag="om", name="om")
                    nc.vector.tensor_tensor(out=om[:], in0=o_sb[:], in1=kn_rT[:], op=OP.mult)
                    nd_ps = pnp.tile([1, 2 * BC], F32, space="PSUM", tag="nd_ps", name="nd_ps")
                    nc.tensor.matmul(nd_ps[:, 0:BC], lhsT=ones_col[:], rhs=om[:])
                    nc.tensor.matmul(nd_ps[:, BC:2 * BC], lhsT=ones_col[:], rhs=kn_rT[:])
                    rcp = pg.tile([1, BC], F32, tag="rcp", name="rcp")
                    nc.vector.reciprocal(rcp[:], nd_ps[:, BC:2 * BC])
                    res = pg.tile([1, BC], F32, tag="res", name="res")
                    nc.vector.tensor_tensor(out=res[:], in0=nd_ps[:, 0:BC], in1=rcp[:],
                                            op=OP.mult)
                    nc.sync.dma_start(out_d[:], res[:])

    nc.compile()
    return nc


# ----------------------------------------------------------------------------
# Entry point
# ----------------------------------------------------------------------------

_TRACE = bool(int(os.environ.get("KERNEL_TRACE", "0")))


def kernel(**inputs):
    meta, in_maps = preprocess(inputs)
    nc = build_program(meta)
    res = bass_utils.run_bass_kernel_spmd(
        nc, in_maps, core_ids=list(range(NC)), trace=_TRACE)
    out = np.concatenate([r["out"].reshape(-1) for r in res.results])
    kernel.last_results = res
    return out.reshape(B, 1).astype(np.float32)


# revision 20
# speedup vs baseline: 6.4736x; 1.0103x over previous
"""Trainium2 Bass kernel for the HAN-based cognitive-diagnosis net.

Strategy (8 NeuronCores, SPMD — one program, per-core data):
  * Batch (2048) split 8x256 across cores. Each core computes GAT outputs
    for its own batch-slice node list plus a 1/8 share of all exercise
    nodes (for the global-mean semantic attention stats, AllReduce'd).
  * Edge phase without any device gather: the edge lists are known on the
    host, so the host pre-expands x^T into the ELL slot layout (one
    x-column per edge slot, zero column for pad slots).  The device
    computes per-edge [z(64) | el(8)] directly with PE matmuls
    (lhsT = xts slot tile, rhs = W folded with a_l), then runs the
    edge softmax + weighted aggregation on DVE/ACT with dst nodes on
    partitions.  Pad slots have z=0 (exact numerator); the softmax
    denominator is corrected analytically:
        s_real = s - npad[d] * exp(leaky(er[d]) - m[d])
    since every pad slot contributes exactly that one value.
  * Predictor exploits the rank structure pref[b,j,k] =
    sigmoid(U1[b,k] + Q1[j,k] + c1[k]): two small matmuls build U/Q once,
    the [B,K,K] tensors are pure broadcast-add + sigmoid, and the W3
    contraction runs on PE per batch row.
"""

import os
import numpy as np

import concourse.bass as bass
import concourse.bacc as bacc
import concourse.mybir as mybir
import concourse.tile as tile
from concourse import library_config
from concourse.masks import make_identity
from concourse import bass_utils

F32 = mybir.dt.float32
F16 = mybir.dt.float16

NC = 8
B = 2048
BC = B // NC          # 256 batch rows per core
K = 128
H, D, FD = 8, 8, 64
SEM = 128
S_N, E_N = 10000, 20000
P = 128

AX = mybir.AxisListType
OP = mybir.AluOpType
AF = mybir.ActivationFunctionType

GRAPHS = ("ex0", "ex1", "st", "kn")


# ----------------------------------------------------------------------------
# Host-side preprocessing (integer / layout only)
# ----------------------------------------------------------------------------

def _csr_by_dst(src, dst, n):
    order = np.argsort(dst, kind="stable")
    ss = src[order].astype(np.int64)
    counts = np.bincount(dst, minlength=n)
    rowptr = np.zeros(n + 1, np.int64)
    np.cumsum(counts, out=rowptr[1:])
    return ss, rowptr, counts


def _tiles_of(nodes):
    return [np.asarray(nodes[i:i + P]) for i in range(0, len(nodes), P)]


def _flat_src(node_tiles, dts, ss, rowptr, counts, npad_row):
    """Flat per-edge-slot src index list (tile-major, col-major, 128 lanes)
    with -1 for pad slots, plus the [128, ntiles] pad-count array."""
    nslot = int(np.sum(dts))
    flat = np.full((nslot, P), -1, np.int64)
    npad = np.zeros((P, len(node_tiles)), np.float32)
    col = 0
    for t, nodes in enumerate(node_tiles):
        dt = int(dts[t])
        for pi, node in enumerate(nodes):
            deg = int(counts[node])
            if deg:
                lo = rowptr[node]
                flat[col:col + deg, pi] = ss[lo:lo + deg]
            npad[pi, t] = -(dt - deg)
        for pi in range(len(nodes), P):
            npad[pi, t] = -dt
        col += dt
    assert col == nslot
    return flat.reshape(-1), npad


def _xts(x_aug_T, flat):
    """[K, nslot*128] fp16 ELL-expanded x^T (zero column for pad slots)."""
    return np.ascontiguousarray(x_aug_T[:, flat])


def _xtp(x, node_tiles, ntiles):
    """x^T columns for a node list, padded to ntiles*128 cols, fp16."""
    kdim = x.shape[1]
    out = np.zeros((kdim, ntiles * P), np.float16)
    for t, nodes in enumerate(node_tiles):
        out[:, t * P:t * P + len(nodes)] = x[nodes].T.astype(np.float16)
    return out


def preprocess(inputs):
    inp = {k: np.asarray(v) for k, v in inputs.items()}
    stu_id = inp["stu_id"].astype(np.int64)
    exer_id = inp["exer_id"].astype(np.int64)

    csr = {
        "st": _csr_by_dst(inp["ss0"].astype(np.int64), inp["sd0"].astype(np.int64), S_N),
        "ex0": _csr_by_dst(inp["es0"].astype(np.int64), inp["ed0"].astype(np.int64), E_N),
        "ex1": _csr_by_dst(inp["es1"].astype(np.int64), inp["ed1"].astype(np.int64), E_N),
        "kn": _csr_by_dst(inp["ks0"].astype(np.int64), inp["kd0"].astype(np.int64), K),
    }

    # per-core node tile lists; exercise share nodes degree-sorted + strided
    SH = E_N // NC                      # 2500
    SH_TILES = (SH + P - 1) // P        # 20
    BS_TILES = BC // P                  # 2
    NTP = {"ex0": SH_TILES + BS_TILES, "ex1": SH_TILES + BS_TILES,
           "st": BS_TILES, "kn": 1}

    tiles = {g: [] for g in GRAPHS}     # g -> [core][tile] node arrays
    for g in ("ex0", "ex1"):
        order = np.argsort(-csr[g][2], kind="stable")
        for c in range(NC):
            share = order[c::NC]
            assert len(share) == SH
            tiles[g].append(_tiles_of(share) +
                            _tiles_of(exer_id[c * BC:(c + 1) * BC]))
    for c in range(NC):
        tiles["st"].append(_tiles_of(stu_id[c * BC:(c + 1) * BC]))
        tiles["kn"].append(_tiles_of(np.arange(K)))

    # shared per-tile Dt = max over cores (SPMD: one program)
    dts = {}
    for g in GRAPHS:
        counts = csr[g][2]
        dts[g] = np.max(
            [[max(1, int(counts[t].max()) if len(t) else 1) for t in tiles[g][c]]
             for c in range(NC)], axis=0)

    meta = dict(dts=dts, NTP=NTP, SH=SH, SH_TILES=SH_TILES, BS_TILES=BS_TILES,
                nslot={g: int(dts[g].sum()) for g in GRAPHS})

    # ------- shared input arrays -------
    shared = {
        "w_ex0": inp["f3W0"].astype(np.float16),
        "w_ex1": inp["f3W1"].astype(np.float16),
        "w_st": inp["f1W0"].astype(np.float16),
        "w_kn": inp["f5W0"].astype(np.float16),
        "alr_ex0": np.concatenate([inp["f3al0"].reshape(1, 64), inp["f3ar0"].reshape(1, 64)], 1),
        "alr_ex1": np.concatenate([inp["f3al1"].reshape(1, 64), inp["f3ar1"].reshape(1, 64)], 1),
        "alr_st": np.concatenate([inp["f1al0"].reshape(1, 64), inp["f1ar0"].reshape(1, 64)], 1),
        "alr_kn": np.concatenate([inp["f5al0"].reshape(1, 64), inp["f5ar0"].reshape(1, 64)], 1),
        "semW": inp["f3sW"].astype(np.float32),
        "semb_col": inp["f3sb"].reshape(SEM, 1).astype(np.float32),
        "semq_col": inp["f3sq"].reshape(SEM, 1).astype(np.float32),
        "pWT_st": inp["f1pW"].T.astype(np.float32).copy(),
        "pb_st": inp["f1pb"].reshape(K, 1).astype(np.float32),
        "pWT_ex": inp["f3pW"].T.astype(np.float32).copy(),
        "pb_ex": inp["f3pb"].reshape(K, 1).astype(np.float32),
        "pW_kn": inp["f5pW"].astype(np.float32),
        "pb_kn_row": inp["f5pb"].reshape(1, K).astype(np.float32),
        "W1a": inp["W1"][:K].astype(np.float32),
        "W1b": inp["W1"][K:].astype(np.float32),
        "W2a": inp["W2"][:K].astype(np.float32),
        "W2b": inp["W2"][K:].astype(np.float32),
        "W3h": inp["W3"].astype(np.float16),
        "b3": inp["b3"].reshape(1, 1).astype(np.float32),
    }

    xsrc = {"ex0": inp["exer_t"], "ex1": inp["exer_t"],
            "st": inp["stu_t"], "kn": inp["kn_t"]}
    x_aug_T = {}
    for g in GRAPHS:
        xa = np.vstack([xsrc[g], np.zeros((1, K), np.float32)]).astype(np.float16)
        x_aug_T[g] = np.ascontiguousarray(xa.T)   # [K, N+1], col N = zeros

    # ------- per-core arrays -------
    in_maps = []
    for c in range(NC):
        m = dict(shared)
        for g in GRAPHS:
            ss, rowptr, counts = csr[g]
            flat, npad = _flat_src(tiles[g][c], dts[g], ss, rowptr, counts, None)
            flat = np.where(flat < 0, xsrc[g].shape[0], flat)
            m["xts_" + g] = _xts(x_aug_T[g], flat)
            m["npad_" + g] = npad
            m["xtp_" + g] = _xtp(xsrc[g], tiles[g][c], NTP[g])
        m["kn_rT"] = inp["kn_r"][c * BC:(c + 1) * BC].T.astype(np.float32).copy()
        in_maps.append(m)

    return meta, in_maps


# ----------------------------------------------------------------------------
# Bass program
# ----------------------------------------------------------------------------

PS_COLS = 24     # slot-cols per PSUM unit: z [P,24,64] (3 banks) + el [P,24,8]
FG = 4           # predictor batch rows per group


def build_program(meta):
    nc = bacc.Bacc("TRN2", num_devices=NC)
    dts = meta["dts"]
    NTP = meta["NTP"]
    SH = meta["SH"]
    SH_TILES, BS_TILES = meta["SH_TILES"], meta["BS_TILES"]
    nslot = meta["nslot"]
    MAXDT = -(-max(int(dts[g].max()) for g in GRAPHS) // 8) * 8

    ein = {}
    def EIN(name, shape, dt):
        ein[name] = nc.dram_tensor(name, list(shape), dt, kind="ExternalInput")
        return ein[name]

    for g in GRAPHS:
        EIN("w_" + g, (K, FD), F16)
        EIN("alr_" + g, (1, 128), F32)
        EIN("xts_" + g, (K, nslot[g] * P), F16)
        EIN("npad_" + g, (P, len(dts[g])), F32)
        EIN("xtp_" + g, (K, NTP[g] * P), F16)
    EIN("semW", (FD, SEM), F32); EIN("semb_col", (SEM, 1), F32); EIN("semq_col", (SEM, 1), F32)
    EIN("pWT_st", (K, FD), F32); EIN("pb_st", (K, 1), F32)
    EIN("pWT_ex", (K, FD), F32); EIN("pb_ex", (K, 1), F32)
    EIN("pW_kn", (FD, K), F32); EIN("pb_kn_row", (1, K), F32)
    EIN("W1a", (K, K), F32); EIN("W1b", (K, K), F32)
    EIN("W2a", (K, K), F32); EIN("W2b", (K, K), F32)
    EIN("W3h", (K, 1), F16); EIN("b3", (1, 1), F32)
    EIN("kn_rT", (K, BC), F32)

    out_d = nc.dram_tensor("out", [1, BC], F32, kind="ExternalOutput")

    cc_in = nc.dram_tensor("cc_in", [1, 16], F32, kind="Internal")
    cc_out = nc.dram_tensor("cc_out", [1, 16], F32, kind="Internal", addr_space="Shared")

    with tile.TileContext(nc) as tc:
        with tc.tile_pool(name="const", bufs=1) as cst, \
             tc.tile_pool(name="slab", bufs=1) as slab:
            nc.gpsimd.load_library(library_config.mlp)

            ident = cst.tile([P, P], F32, tag="ident", name="ident")
            make_identity(nc, ident[:])
            identh = cst.tile([P, P], F16, tag="identh", name="identh")
            nc.vector.tensor_copy(identh[:], ident[:])
            ones_col = cst.tile([P, 1], F32, tag="ones_col", name="ones_col")
            nc.vector.memset(ones_col[:], 1.0)
            ones_row = cst.tile([1, P], F32, tag="ones_row", name="ones_row")
            nc.vector.memset(ones_row[:], 1.0)

            # ---- load small weights ----
            def load(name, shape, dt):
                t = cst.tile(list(shape), dt, tag="ld_" + name, name="ld_" + name)
                nc.sync.dma_start(t[:], ein[name][:])
                return t
            w_g = {g: load("w_" + g, (K, FD), F16) for g in GRAPHS}
            alr = {g: load("alr_" + g, (1, 128), F32) for g in GRAPHS}
            npad_sb = {g: load("npad_" + g, (P, len(dts[g])), F32) for g in GRAPHS}
            semW = load("semW", (FD, SEM), F32)
            semb_col = load("semb_col", (SEM, 1), F32)
            semq_col = load("semq_col", (SEM, 1), F32)
            pWT_st = load("pWT_st", (K, FD), F32); pb_st = load("pb_st", (K, 1), F32)
            pWT_ex = load("pWT_ex", (K, FD), F32); pb_ex = load("pb_ex", (K, 1), F32)
            pW_kn = load("pW_kn", (FD, K), F32); pb_kn_row = load("pb_kn_row", (1, K), F32)
            W1a = load("W1a", (K, K), F32); W1b = load("W1b", (K, K), F32)
            W2a = load("W2a", (K, K), F32); W2b = load("W2b", (K, K), F32)
            W3h = load("W3h", (K, 1), F16); b3 = load("b3", (1, 1), F32)
            kn_rT = load("kn_rT", (K, BC), F32)

            # ---- fold al/ar into W: wcat = [W | W@al] fp16, war = W@ar ----
            wcat = {}   # [128, 72] f16: cols 0:64 W, 64:72 Wal
            war = {}    # [128, 8] f16
            with tc.tile_pool(name="bc_ps", bufs=2, space="PSUM") as bcp:
              for g in GRAPHS:
                alb = cst.tile([P, 128], F32, tag="alb", name="alb")
                alb_ps = bcp.tile([P, 128], F32, space="PSUM", tag="alb_ps", name="alb_ps")
                nc.tensor.matmul(alb_ps[:], lhsT=ones_row[:], rhs=alr[g][:])
                nc.vector.tensor_copy(alb[:], alb_ps[:])
                wf = cst.tile([P, FD], F32, tag="wf", name="wf")
                nc.vector.tensor_copy(wf[:], w_g[g][:])
                wtmp = cst.tile([P, FD], F32, tag="wtmp", name="wtmp")
                wc = cst.tile([P, 72], F16, tag="wcat_" + g, name="wcat_" + g)
                wcat[g] = wc
                nc.vector.tensor_copy(wc[:, 0:64], w_g[g][:])
                with nc.allow_low_precision(reason="8-elem head fold of fp16 weights"):
                    nc.vector.tensor_tensor(out=wtmp[:], in0=wf[:], in1=alb[:, 0:64], op=OP.mult)
                    nc.vector.tensor_reduce(out=wc[:, 64:72].bitcast(F16),
                                            in_=wtmp[:].rearrange("p (h f) -> p h f", h=H),
                                            axis=AX.X, op=OP.add)
                    wr = cst.tile([P, 8], F16, tag="war_" + g, name="war_" + g)
                    war[g] = wr
                    nc.vector.tensor_tensor(out=wtmp[:], in0=wf[:], in1=alb[:, 64:128], op=OP.mult)
                    nc.vector.tensor_reduce(out=wr[:], in_=wtmp[:].rearrange("p (h f) -> p h f", h=H),
                                            axis=AX.X, op=OP.add)

            # ---- er per (graph, tile): er[d, t, h] = (x[d] @ war)[h] ----
            er = {}
            with tc.tile_pool(name="pE", bufs=2) as pe, \
                 tc.tile_pool(name="pE_ps", bufs=4, space="PSUM") as pep:
                for g in GRAPHS:
                    ntp = NTP[g]
                    er_sb = slab.tile([P, ntp, 8], F32, tag="er_" + g, name="er_" + g)
                    er[g] = er_sb
                    xtp_sb = pe.tile([P, NTP["ex0"] * P], F16, tag="xtp_sb", name="xtp_sb")
                    nc.sync.dma_start(xtp_sb[:, 0:ntp * P], ein["xtp_" + g][:])
                    for t in range(ntp):
                        eps = pep.tile([P, 8], F32, space="PSUM", tag="eps", name="eps")
                        nc.tensor.matmul(eps[:], lhsT=xtp_sb[:, t * P:(t + 1) * P],
                                         rhs=war[g][:])
                        nc.vector.tensor_copy(er_sb[:, t, :], eps[:])

            # ---- Phase B: per-edge z via PE + edge softmax + aggregation ----
            zs = {g: slab.tile([P, NTP[g], FD], F32, tag="zs_" + g, name="zs_" + g)
                  for g in GRAPHS}
            s_slab = {g: slab.tile([P, NTP[g], 8], F32, tag="s_" + g, name="s_" + g)
                      for g in GRAPHS}

            def do_graph(g, pb, pbs, pzp):
                """Emit z matmuls + softmax for every tile of graph g.

                Small tiles (Dt <= PS_COLS): z and el stay resident in PSUM;
                the e-pipe reads el from PSUM and the weighted mult reads z
                from PSUM (no evacuation copies).  Big tiles copy per unit.
                """
                col0 = 0
                for t in range(NTP[g]):
                    Dt = int(dts[g][t])
                    DtP = -(-Dt // 8) * 8
                    small = Dt <= PS_COLS
                    z_el = pbs.tile([P, MAXDT, 64], F16, tag="z_el", name="z_el")
                    if not small:
                        el_t = pbs.tile([P, 8, MAXDT], F32, tag="el_t", name="el_t")
                    zp_keep = None
                    elp_keep = None
                    for lo in range(0, Dt, PS_COLS):
                        n_here = min(PS_COLS, Dt - lo)
                        xts_sb = pb.tile([P, PS_COLS * P], F16, tag="xts_sb", name="xts_sb")
                        nc.sync.dma_start(
                            xts_sb[:, 0:n_here * P],
                            ein["xts_" + g][:, (col0 + lo) * P:(col0 + lo + n_here) * P])
                        zp = pzp.tile([P, PS_COLS, 64], F32, space="PSUM",
                                      tag="zp", name="zp")
                        elp = pzp.tile([P, PS_COLS, 8], F32, space="PSUM",
                                       tag="elp", name="elp")
                        for ci in range(n_here):
                            lhsT = xts_sb[:, ci * P:(ci + 1) * P]
                            nc.tensor.matmul(zp[:, ci, :], lhsT=lhsT, rhs=wcat[g][:, 0:64])
                            nc.tensor.matmul(elp[:, ci, :], lhsT=lhsT, rhs=wcat[g][:, 64:72])
                        if small:
                            zp_keep, elp_keep = zp, elp
                        else:
                            nc.scalar.activation(out=z_el[:, lo:lo + n_here, :],
                                                 in_=zp[:, 0:n_here, :], func=AF.Copy)
                            nc.scalar.activation(
                                out=el_t[:].transpose([0, 2, 1])[:, lo:lo + n_here, :],
                                in_=elp[:, 0:n_here, :], func=AF.Copy)
                    # ---- edge softmax over the Dt slots (head-major e) ----
                    e = pbs.tile([P, 8, MAXDT], F32, tag="e_buf", name="e_buf")
                    if small:
                        nc.vector.tensor_tensor(
                            out=e[:].transpose([0, 2, 1])[:, 0:Dt, :],
                            in0=elp_keep[:, 0:Dt, :],
                            in1=er[g][:, t, :].unsqueeze(1).to_broadcast([P, Dt, 8]),
                            op=OP.add)
                    else:
                        nc.vector.tensor_tensor(
                            out=e[:, :, 0:Dt], in0=el_t[:, :, 0:Dt],
                            in1=er[g][:, t, :].unsqueeze(2).to_broadcast([P, 8, Dt]),
                            op=OP.add)
                    nc.vector.scalar_tensor_tensor(out=e[:, :, 0:Dt], in0=e[:, :, 0:Dt],
                                                   scalar=0.2, in1=e[:, :, 0:Dt],
                                                   op0=OP.mult, op1=OP.max)
                    nc.scalar.activation(out=e[:, :, 0:Dt], in_=e[:, :, 0:Dt],
                                          func=AF.Exp)
                    s = s_slab[g][:, t, :]
                    nc.vector.tensor_reduce(out=s, in_=e[:, :, 0:Dt],
                                            axis=AX.X, op=OP.add)
                    # weighted w = z * exb into SBUF (z from PSUM for small tiles)
                    z_src = (zp_keep[:, 0:Dt, :] if small else z_el[:, 0:Dt, :])
                    nc.vector.tensor_tensor(
                        out=z_el[:, 0:Dt, :].rearrange("p s (h f) -> p s h f", h=H),
                        in0=z_src.rearrange("p s (h f) -> p s h f", h=H),
                        in1=e[:, :, 0:Dt].transpose([0, 2, 1]).unsqueeze(3)
                        .to_broadcast([P, Dt, 8, 8]),
                        op=OP.mult)
                    if DtP > Dt:
                        nc.gpsimd.memset(z_el[:, Dt:DtP, :], 0.0)
                    # 3 contiguous halvings then short strided tail reduce
                    cur = DtP
                    while cur > DtP // 8:
                        h2 = cur // 2
                        nc.vector.tensor_tensor(out=z_el[:, 0:h2, :],
                                                in0=z_el[:, 0:h2, :],
                                                in1=z_el[:, h2:cur, :], op=OP.add)
                        cur = h2
                    v = zs[g][:, t, :]
                    nc.vector.tensor_reduce(
                        out=v, in_=z_el[:, 0:cur, :].transpose([0, 2, 1]),
                        axis=AX.X, op=OP.add)
                    col0 += Dt
                # ---- batched per-graph epilogue: pad fix, normalize, elu ----
                ntp = NTP[g]
                epb = pbs.tile([P, ntp, 8], F32, tag="epb_" + g, name="epb_" + g)
                nc.vector.scalar_tensor_tensor(out=epb[:], in0=er[g][:],
                                               scalar=0.2, in1=er[g][:],
                                               op0=OP.mult, op1=OP.max)
                nc.scalar.activation(out=epb[:], in_=epb[:], func=AF.Exp)
                nc.vector.tensor_tensor(
                    out=epb[:], in0=epb[:],
                    in1=npad_sb[g][:].unsqueeze(2).to_broadcast([P, ntp, 8]),
                    op=OP.mult)
                nc.vector.tensor_tensor(out=s_slab[g][:], in0=s_slab[g][:],
                                        in1=epb[:], op=OP.add)
                nc.vector.tensor_scalar_add(s_slab[g][:], s_slab[g][:], 1e-9)
                nc.vector.reciprocal(s_slab[g][:], s_slab[g][:])
                nc.vector.tensor_tensor(
                    out=zs[g][:].rearrange("p t (h f) -> p t h f", h=H),
                    in0=zs[g][:].rearrange("p t (h f) -> p t h f", h=H),
                    in1=s_slab[g][:].unsqueeze(3).to_broadcast([P, ntp, 8, 8]),
                    op=OP.mult)
                # elu (batched)
                t1 = pbs.tile([P, ntp, FD], F32, tag="elu1_" + g, name="elu1_" + g)
                nc.vector.tensor_scalar_min(t1[:], zs[g][:], 0.0)
                t2 = pbs.tile([P, ntp, FD], F32, tag="elu2_" + g, name="elu2_" + g)
                nc.scalar.activation(out=t2[:], in_=t1[:], func=AF.Exp)
                nc.vector.tensor_tensor(out=zs[g][:], in0=zs[g][:], in1=t1[:],
                                        op=OP.subtract)
                nc.vector.scalar_tensor_tensor(out=zs[g][:], in0=t2[:], scalar=-1.0,
                                               in1=zs[g][:], op0=OP.add, op1=OP.add)

            zsT = {g: slab.tile([FD, NTP[g] * P], F32, tag="zsT_" + g, name="zsT_" + g)
                   for g in GRAPHS}

            def do_transposes(g, pcp):
                for t in range(NTP[g]):
                    tp = pcp.tile([FD, P], F32, space="PSUM", tag="tp_ps", name="tp_ps")
                    nc.tensor.transpose(out=tp[:], in_=zs[g][:, t, :], identity=ident[:])
                    nc.scalar.copy(zsT[g][:, t * P:(t + 1) * P], tp[:])

            stats = cst.tile([1, 16], F32, tag="stats", name="stats")
            nc.vector.memset(stats[:], 0.0)
            with tc.tile_pool(name="pB", bufs=3) as pb, \
                 tc.tile_pool(name="pBs", bufs=2) as pbs:
                with tc.tile_pool(name="pB_ps", bufs=2, space="PSUM") as pzp:
                    do_graph("ex0", pb, pbs, pzp)
                    do_graph("ex1", pb, pbs, pzp)
                    do_graph("st", pb, pbs, pzp)
                    do_graph("kn", pb, pbs, pzp)

                with tc.tile_pool(name="pC_ps", bufs=4, space="PSUM") as pcp:
                    for g in GRAPHS:
                        do_transposes(g, pcp)

                # ---- semantic attention stats over the exercise share ----
                with tc.tile_pool(name="pD_ps", bufs=4, space="PSUM") as pdp:
                    parts = cst.tile([1, 16], F32, tag="parts", name="parts")
                    for mi, g in enumerate(("ex0", "ex1")):
                        cw_list = []
                        lo = 0
                        while lo < SH:
                            cw = min(512, SH - lo)
                            cw_list.append((lo, cw))
                            lo += cw
                        for ci, (lo, cw) in enumerate(cw_list):
                            tps = pdp.tile([SEM, 512], F32, space="PSUM", tag="tps", name="tps")
                            nc.tensor.matmul(tps[:, 0:cw], lhsT=semW[:], rhs=zsT[g][:, lo:lo + cw])
                            tsb = pbs.tile([SEM, 512], F32, tag="tsb", name="tsb")
                            nc.scalar.activation(out=tsb[:, 0:cw], in_=tps[:, 0:cw],
                                                 func=AF.Tanh, bias=semb_col[:])
                            rps = pdp.tile([1, 512], F32, space="PSUM", tag="rps", name="rps")
                            nc.tensor.matmul(rps[:, 0:cw], lhsT=semq_col[:], rhs=tsb[:, 0:cw])
                            nc.vector.tensor_reduce(out=parts[:, mi * 8 + ci:mi * 8 + ci + 1],
                                                    in_=rps[:, 0:cw], axis=AX.X, op=OP.add)
                        nc.vector.tensor_reduce(
                            out=stats[:, mi:mi + 1],
                            in_=parts[:, mi * 8:mi * 8 + len(cw_list)], axis=AX.X, op=OP.add)

            # ---- AllReduce the 2 stats scalars (overlapped with prep) ----
            nc.sync.dma_start(cc_in[:, 0:16], stats[:])
            nc.gpsimd.collective_compute(
                "AllReduce", OP.add,
                replica_groups=[list(range(NC))],
                ins=[cc_in[:, :]], outs=[cc_out[:, :]])

            # ---- predictor prep (beta-independent parts first; the gstats
            # read + beta + zsFT + U2T slot in after, hiding the AllReduce) ----
            q1jk = cst.tile([P, K], F16, tag="q1jk", name="q1jk")      # Q1 [j, k]
            st_sb = cst.tile([P, K], F16, tag="st_sb", name="st_sb")   # Q2T [k, j]
            m1_sb = cst.tile([FD, K], F32, tag="m1_sb", name="m1_sb")
            m2_sb = cst.tile([FD, K], F32, tag="m2_sb", name="m2_sb")
            c1t = cst.tile([P, 1], F32, tag="c1t", name="c1t")
            c2t = cst.tile([P, 1], F32, tag="c2t", name="c2t")
            kn1T = cst.tile([P, K], F32, tag="kn1T", name="kn1T")
            U1bk = cst.tile([P, 2, K], F16, tag="U1bk", name="U1bk")   # U1 [b, k]
            U2T = cst.tile([P, BC], F16, tag="U2T", name="U2T")
            with tc.tile_pool(name="pF_ps", bufs=2, space="PSUM") as pfp:
                kn1_ps = pfp.tile([P, K], F32, space="PSUM", tag="prep_ps", name="kn1_ps")
                nc.tensor.matmul(kn1_ps[:], lhsT=zsT["kn"][:], rhs=pW_kn[:],
                                 start=True, stop=False)
                nc.tensor.matmul(kn1_ps[:], lhsT=ones_row[:], rhs=pb_kn_row[:],
                                 start=False, stop=True)
                kn1_sb = cst.tile([P, K], F32, tag="kn1_sb", name="kn1_sb")
                nc.scalar.copy(kn1_sb[:], kn1_ps[:])
                kn1T_ps = pfp.tile([P, K], F32, space="PSUM", tag="prep_ps", name="kn1T_ps")
                nc.tensor.transpose(out=kn1T_ps[:], in_=kn1_sb[:], identity=ident[:])
                nc.scalar.copy(kn1T[:], kn1T_ps[:])

                qs_ps = pfp.tile([P, K], F32, space="PSUM", tag="prep_ps", name="qs_ps")
                nc.tensor.matmul(qs_ps[:], lhsT=kn1T[:], rhs=W1b[:])
                nc.scalar.copy(q1jk[:], qs_ps[:])
                qs2_ps = pfp.tile([P, K], F32, space="PSUM", tag="prep_ps", name="qs2_ps")
                nc.tensor.matmul(qs2_ps[:], lhsT=W2b[:], rhs=kn1T[:])
                nc.scalar.copy(st_sb[:], qs2_ps[:])

                m1_ps = pfp.tile([FD, K], F32, space="PSUM", tag="prep_ps", name="m1_ps")
                nc.tensor.matmul(m1_ps[:], lhsT=pWT_st[:], rhs=W1a[:])
                nc.scalar.copy(m1_sb[:], m1_ps[:])
                m2_ps = pfp.tile([FD, K], F32, space="PSUM", tag="prep_ps", name="m2_ps")
                nc.tensor.matmul(m2_ps[:], lhsT=pWT_ex[:], rhs=W2a[:])
                nc.scalar.copy(m2_sb[:], m2_ps[:])
                c1_ps = pfp.tile([P, 1], F32, space="PSUM", tag="prep_ps", name="c1_ps")
                nc.tensor.matmul(c1_ps[:], lhsT=W1a[:], rhs=pb_st[:])
                nc.vector.tensor_copy(c1t[:], c1_ps[:])
                c2_ps = pfp.tile([P, 1], F32, space="PSUM", tag="prep_ps", name="c2_ps")
                nc.tensor.matmul(c2_ps[:], lhsT=W2a[:], rhs=pb_ex[:])
                nc.vector.tensor_copy(c2t[:], c2_ps[:])

                for bh in range(2):
                    u1_ps = pfp.tile([P, K], F32, space="PSUM", tag="u_ps", name="u1_ps")
                    nc.tensor.matmul(u1_ps[:], lhsT=zsT["st"][:, bh * P:(bh + 1) * P],
                                     rhs=m1_sb[:])
                    nc.vector.tensor_copy(U1bk[:, bh, :], u1_ps[:])
                # collective result -> beta -> fused features -> U2T
                gstats = cst.tile([1, 16], F32, tag="gstats", name="gstats")
                nc.sync.dma_start(gstats[:], cc_out[:, :])
                beta_col = cst.tile([P, 2], F32, tag="beta_col", name="beta_col")
                bd = cst.tile([1, 2], F32, tag="bd", name="bd")
                nc.vector.tensor_tensor(out=bd[:, 0:1], in0=gstats[:, 0:1],
                                        in1=gstats[:, 1:2], op=OP.subtract)
                btmp = cst.tile([1, 2], F32, tag="btmp", name="btmp")
                nc.scalar.activation(out=btmp[:, 0:1], in_=bd[:, 0:1], func=AF.Sigmoid,
                                     scale=1.0 / E_N)
                nc.scalar.activation(out=btmp[:, 1:2], in_=bd[:, 0:1], func=AF.Sigmoid,
                                     scale=-1.0 / E_N)
                b3_col = cst.tile([P, 1], F32, tag="b3_col", name="b3_col")
                bb_ps = pfp.tile([P, 4], F32, space="PSUM", tag="bb_ps", name="bb_ps")
                nc.tensor.matmul(bb_ps[:, 0:2], lhsT=ones_row[:], rhs=btmp[:])
                nc.tensor.matmul(bb_ps[:, 2:3], lhsT=ones_row[:], rhs=b3[:])
                nc.vector.tensor_copy(beta_col[:], bb_ps[:, 0:2])
                nc.vector.tensor_copy(b3_col[:], bb_ps[:, 2:3])
                zsFT = cst.tile([FD, BC], F32, tag="zsFT", name="zsFT")
                bcol = SH_TILES * P
                nc.vector.tensor_scalar(out=zsFT[:], in0=zsT["ex0"][:, bcol:bcol + BC],
                                        scalar1=beta_col[0:FD, 0:1], scalar2=None,
                                        op0=OP.mult)
                nc.vector.scalar_tensor_tensor(out=zsFT[:], in0=zsT["ex1"][:, bcol:bcol + BC],
                                               scalar=beta_col[0:FD, 1:2], in1=zsFT[:],
                                               op0=OP.mult, op1=OP.add)
                u2_ps = pfp.tile([P, BC], F32, space="PSUM", tag="u_ps", name="u2_ps")
                nc.tensor.matmul(u2_ps[:], lhsT=m2_sb[:], rhs=zsFT[:])
                nc.vector.tensor_copy(U2T[:], u2_ps[:])

            # ---- predictor main loop ----
            # pref[b,j,k] = sig(U1[b,k] + Q1[j,k] + c1[k]); layout [k, (g,j)]
            FCH = 32   # groups per chunk; chunk-0 pr work hides the AllReduce
            with tc.tile_pool(name="pG", bufs=6) as pg, \
                 tc.tile_pool(name="pGpr", bufs=FCH) as pgpr, \
                 tc.tile_pool(name="pG_ps", bufs=4, space="PSUM") as pgp, \
                 tc.tile_pool(name="pO_ps", bufs=1, space="PSUM") as pop:
                o_ps = pop.tile([P, BC], F32, space="PSUM", tag="o_ps", name="o_ps")
                for c0 in range(0, BC // FG, FCH):
                    prs = []
                    for grp in range(c0, c0 + FCH):
                        b0 = grp * FG
                        # pref on PE: out[k,(g,j)] = Q1[j,k] (dj) + U1[b0+g,k] (dg)
                        pr_ps = pgp.tile([P, FG, K], F32, space="PSUM", tag="pr_ps",
                                         name="pr_ps")
                        nc.tensor.matmul(
                            pr_ps[:], lhsT=q1jk[:],
                            rhs=identh[:].unsqueeze(1).to_broadcast([P, FG, K]),
                            start=True, stop=False)
                        nc.tensor.matmul(
                            pr_ps[:], lhsT=U1bk[:, b0 // P, :],
                            rhs=identh[:, b0 % P:b0 % P + FG].unsqueeze(2)
                            .to_broadcast([P, FG, K]),
                            start=False, stop=True)
                        pr_sb = pgpr.tile([P, FG, K], F16, tag="pr_sb", name="pr_sb")
                        nc.scalar.activation(out=pr_sb[:], in_=pr_ps[:], func=AF.Sigmoid,
                                             bias=c1t[:])
                        prs.append(pr_sb)
                    for gi, grp in enumerate(range(c0, c0 + FCH)):
                        b0 = grp * FG
                        df_lin = pg.tile([P, FG, K], F16, tag="df_lin", name="df_lin")
                        nc.vector.tensor_tensor(
                            out=df_lin[:],
                            in0=st_sb[:].unsqueeze(1).to_broadcast([P, FG, K]),
                            in1=U2T[:, b0:b0 + FG].unsqueeze(2).to_broadcast([P, FG, K]),
                            op=OP.add)
                        df_sb = pg.tile([P, FG, K], F16, tag="df_sb", name="df_sb")
                        nc.scalar.activation(out=df_sb[:], in_=df_lin[:], func=AF.Sigmoid,
                                             bias=c2t[:])
                        d_sb = pg.tile([P, FG, K], F16, tag="d_sb", name="d_sb")
                        nc.vector.tensor_tensor(out=d_sb[:], in0=prs[gi][:], in1=df_sb[:],
                                                op=OP.subtract)
                        for lb in range(FG):
                            nc.tensor.matmul(o_ps[:, b0 + lb:b0 + lb + 1],
                                             lhsT=d_sb[:, lb, :], rhs=W3h[:])

                # ---- final ----
                with tc.tile_pool(name="pN_ps", bufs=1, space="PSUM") as pnp:
                    o_sb = pg.tile([P, BC], F32, tag="o_sb", name="o_sb")
                    nc.scalar.activation(out=o_sb[:], in_=o_ps[:], func=AF.Sigmoid,
                                         bias=b3_col[:])
                    om = pg.tile([P, BC], F32, tag="om", name="om")
                    nc.vector.tensor_tensor(out=om[:], in0=o_sb[:], in1=kn_rT[:], op=OP.mult)
                    nd_ps = pnp.tile([1, 2 * BC], F32, space="PSUM", tag="nd_ps", name="nd_ps")
                    nc.tensor.matmul(nd_ps[:, 0:BC], lhsT=ones_col[:], rhs=om[:])
                    nc.tensor.matmul(nd_ps[:, BC:2 * BC], lhsT=ones_col[:], rhs=kn_rT[:])
                    rcp = pg.tile([1, BC], F32, tag="rcp", name="rcp")
                    nc.vector.reciprocal(rcp[:], nd_ps[:, BC:2 * BC])
                    res = pg.tile([1, BC], F32, tag="res", name="res")
                    nc.vector.tensor_tensor(out=res[:], in0=nd_ps[:, 0:BC], in1=rcp[:],
                                            op=OP.mult)
                    nc.sync.dma_start(out_d[:], res[:])

    nc.compile()
    return nc


# ----------------------------------------------------------------------------
# Entry point
# ----------------------------------------------------------------------------

_TRACE = bool(int(os.environ.get("KERNEL_TRACE", "0")))


def kernel(**inputs):
    meta, in_maps = preprocess(inputs)
    nc = build_program(meta)
    res = bass_utils.run_bass_kernel_spmd(
        nc, in_maps, core_ids=list(range(NC)), trace=_TRACE)
    out = np.concatenate([r["out"].reshape(-1) for r in res.results])
    kernel.last_results = res
    return out.reshape(B, 1).astype(np.float32)
